# revision 28
# baseline (speedup 1.0000x reference)
"""Distributed Trainium2 Bass kernel for BitNet-style attention block.

Sharding: sequence-parallel projections + (batch x kv-head) parallel attention,
stitched with per-batch AllToAll collectives (split so comm overlaps compute).

Per core (core i):
  A. RMSNorm + per-token absmax quantization of its 512-token chunk.
  B. qkv projection as exact integer bf16 matmul against host-prequantized
     ternary weights, dequant, RoPE on q/k, scatter into per-batch A2A bufs.
  C. AllToAll #1a/#1b -> core i holds full-sequence q/k/v for kv-head i of
     each batch; causal attention (transposed scores, exp on ACT, ones-column
     rowsums, deferred normalization). Batch 1 attention overlaps A2A #2a.
  D. AllToAll #2a/#2b -> core i holds its token chunk of all 32 heads;
     per-token quantization + integer matmul with ternary output weights.
"""
import sys
sys.path.insert(0, "/opt/trn_rl_repo")
import numpy as np
import ml_dtypes
import concourse.bass as bass
import concourse.tile as tile
from concourse import bacc, mybir
from concourse import bass_utils
from concourse.masks import make_identity

f32 = mybir.dt.float32
bf16 = mybir.dt.bfloat16
FT = mybir.ActivationFunctionType
ALU = mybir.AluOpType

B, S, H = 2, 2048, 2048
NH, NKV, HD = 32, 8, 64
G = NH // NKV                    # 4
QKV_O = (NH + 2 * NKV) * HD      # 3072
EPS = 1e-5
THETA = 10000.0
C = 8
SC = S // C                      # 256 positions per core
TOK = B * SC                     # 512 token rows per core
MAGIC = float(1.5 * 2.0 ** 23)   # RNE integer rounding for |v| < 2^22
NT = TOK // 128                  # 4 token tiles
NHT = H // 128                   # 16 h-tiles
NKT = S // 128                   # 16 kj tiles

# per-batch a2a1 payload cols: [q 256 | k 64 | kdup 64 | v 64]
A1W = G * HD + 3 * HD            # 448
CK = 256
CKD = 320
CV = 384


def _dap(t_ap, extra, dims):
    return bass.AP(tensor=t_ap.tensor, offset=t_ap.offset + extra, ap=[list(d) for d in dims])


def build_nc():
    nc = bacc.Bacc("TRN2", target_bir_lowering=False, debug=False, num_devices=C)

    x_in = nc.dram_tensor("x", [TOK, H], f32, kind="ExternalInput")
    wn_in = nc.dram_tensor("wn", [1, H], f32, kind="ExternalInput")
    # contiguous stripes: row ((ng*16+j)*128 + h_local), 512 cols each
    wq1t_in = nc.dram_tensor("wq1t", [(QKV_O // 512) * NHT * 128, 512], bf16, kind="ExternalInput")
    wq2t_in = nc.dram_tensor("wq2t", [(H // 512) * NHT * 128, 512], bf16, kind="ExternalInput")
    cos_in = nc.dram_tensor("cosb", [SC, 8 * 32], f32, kind="ExternalInput")
    sin_in = nc.dram_tensor("sinb", [SC, 8 * 32], f32, kind="ExternalInput")
    tri_in = nc.dram_tensor("trimask", [128, 128], bf16, kind="ExternalInput")
    sw1_in = nc.dram_tensor("sw1", [1, 1], f32, kind="ExternalInput")
    sw2_in = nc.dram_tensor("sw2", [1, 1], f32, kind="ExternalInput")
    out_ext = nc.dram_tensor("out", [TOK, H], f32, kind="ExternalOutput")

    X = x_in.ap()
    WQ1 = wq1t_in.ap()
    WQ2 = wq2t_in.ap()
    OUT = out_ext.ap()

    with tile.TileContext(nc) as tc:
        from contextlib import ExitStack
        with ExitStack() as top:
            dram = top.enter_context(tc.tile_pool(name="dram", bufs=1, space="DRAM"))
            const = top.enter_context(tc.tile_pool(name="const", bufs=1))
            smalls = top.enter_context(tc.tile_pool(name="smalls", bufs=1))
            psA = top.enter_context(tc.tile_pool(name="psA", bufs=4, space="PSUM"))
            psS = top.enter_context(tc.tile_pool(name="psS", bufs=2, space="PSUM"))

            # ---------------- DRAM scratch ----------------
            a1i = [dram.tile([C * SC, A1W], bf16, name=f"a1i_{b}") for b in range(B)]
            a1o = [dram.tile([C * SC, A1W], bf16, name=f"a1o_{b}") for b in range(B)]
            a2i = [dram.tile([C * SC, G * HD], bf16, name=f"a2i_{b}") for b in range(B)]
            a2o = [dram.tile([C * SC, G * HD], bf16, name=f"a2o_{b}") for b in range(B)]

            # ---------------- constants ----------------
            wnorm_b = const.tile([128, H], f32)
            nc.sync.dma_start(out=wnorm_b[:], in_=_dap(wn_in.ap(), 0, [[0, 128], [1, H]]))
            trim = const.tile([128, 128], bf16)
            nc.sync.dma_start(out=trim[:], in_=tri_in.ap()[:, :])
            sw1b = const.tile([128, 1], f32)
            nc.sync.dma_start(out=sw1b[:], in_=_dap(sw1_in.ap(), 0, [[0, 128], [1, 1]]))
            sw2b = const.tile([128, 1], f32)
            nc.sync.dma_start(out=sw2b[:], in_=_dap(sw2_in.ap(), 0, [[0, 128], [1, 1]]))
            epsb = const.tile([128, 1], f32)
            nc.vector.memset(epsb[:], EPS)
            ident = const.tile([128, 128], bf16)
            make_identity(nc, ident[:])

            d1s = [smalls.tile([128, 1], f32, name=f"d1_{m}") for m in range(NT)]
            d2s = [smalls.tile([128, 1], f32, name=f"d2_{m}") for m in range(NT)]

            xqT_pool = top.enter_context(tc.tile_pool(name="xqT", bufs=NHT))

            # ================= Stage A: RMSNorm + quantize =================
            with ExitStack() as sa:
                pA = sa.enter_context(tc.tile_pool(name="pA", bufs=2))
                pXQ = sa.enter_context(tc.tile_pool(name="pXQ", bufs=NT))
                pSc = sa.enter_context(tc.tile_pool(name="pASc", bufs=4))
                xqms = []
                for m in range(NT):
                    xa = pA.tile([128, H], f32, tag="xa")
                    nc.sync.dma_start(out=xa[:], in_=X[m * 128:(m + 1) * 128, :])
                    sq = pA.tile([128, H], f32, tag="sq")
                    ssq = pSc.tile([128, 1], f32, tag="ssq")
                    nc.scalar.activation(out=sq[:], in_=xa[:], func=FT.Square, accum_out=ssq[:])
                    xw = pA.tile([128, H], f32, tag="xw")
                    nc.vector.tensor_tensor(xw[:], xa[:], wnorm_b[:], ALU.mult)
                    std = pSc.tile([128, 1], f32, tag="std")
                    nc.scalar.activation(out=std[:], in_=ssq[:], func=FT.Sqrt,
                                         bias=epsb[:], scale=1.0 / H)
                    rstd = pSc.tile([128, 1], f32, tag="rstd")
                    nc.vector.reciprocal(rstd[:], std[:])
                    mx = pSc.tile([128, 1], f32, tag="mx")
                    nc.vector.tensor_reduce(mx[:], xw[:], mybir.AxisListType.X, ALU.max,
                                            apply_absolute_value=True)
                    mp = pSc.tile([128, 1], f32, tag="mp")
                    nc.vector.tensor_scalar(mp[:], mx[:], rstd[:], 1e-5, ALU.mult, ALU.max)
                    nc.vector.tensor_tensor(d1s[m][:], mp[:], sw1b[:], ALU.mult)
                    rmp = pSc.tile([128, 1], f32, tag="rmp")
                    nc.vector.reciprocal(rmp[:], mp[:])
                    csc = pSc.tile([128, 1], f32, tag="csc")
                    nc.vector.tensor_scalar(csc[:], rmp[:], rstd[:], 127.0, ALU.mult, ALU.mult)
                    t1 = pA.tile([128, H], f32, tag="t1")
                    nc.gpsimd.tensor_scalar(t1[:], xw[:], csc[:], MAGIC, ALU.mult, ALU.add)
                    xqm = pXQ.tile([128, H], bf16, tag="xqm", name=f"xqm_{m}")
                    nc.vector.tensor_scalar(xqm[:], t1[:], MAGIC, None, ALU.subtract)
                    xqms.append(xqm)

                # transposed activations via PE (keeps the DMA queues free)
                xqT = []
                for j in range(NHT):
                    t = xqT_pool.tile([128, TOK], bf16, name=f"xqT_{j}", tag="xqT")
                    xqT.append(t)
                for m in range(NT):
                    for j in range(NHT):
                        tp = psS.tile([128, 128], bf16, tag="st", name=f"tp_{m}_{j}")
                        nc.tensor.transpose(tp[:], xqms[m][:, j * 128:(j + 1) * 128], ident[:])
                        nc.vector.tensor_copy(xqT[j][:, m * 128:(m + 1) * 128], tp[:])

            # ================= Stage B: qkv matmul + RoPE + scatter ========
            with ExitStack() as sb:
                pW = sb.enter_context(tc.tile_pool(name="pW", bufs=4))
                pQC = sb.enter_context(tc.tile_pool(name="pQC", bufs=3))
                pRT = sb.enter_context(tc.tile_pool(name="pRT", bufs=2))
                pSend = sb.enter_context(tc.tile_pool(name="pSend", bufs=NT))
                pCos = sb.enter_context(tc.tile_pool(name="pCos", bufs=1))

                cosr = []
                sinr = []
                for par in range(2):
                    ct = pCos.tile([128, 8 * 32], f32, name=f"cosr_{par}")
                    nc.sync.dma_start(out=ct[:], in_=cos_in.ap()[par * 128:(par + 1) * 128, :])
                    st_ = pCos.tile([128, 8 * 32], f32, name=f"sinr_{par}")
                    nc.sync.dma_start(out=st_[:], in_=sin_in.ap()[par * 128:(par + 1) * 128, :])
                    cosr.append(ct)
                    sinr.append(st_)

                sends = [pSend.tile([128, QKV_O], bf16, name=f"sends_{m}", tag="sends")
                         for m in range(NT)]

                NQC = QKV_O // 512   # 6 chunks of 512
                for bh in range(B):
                    for ng in range(NQC):
                        psq = [psA.tile([128, 512], f32, tag="acc", name=f"qkvp_{bh}_{ng}_{mm}")
                               for mm in range(2)]
                        for j in range(NHT):
                            wt = pW.tile([128, 512], bf16, tag="w1")
                            r0 = (ng * NHT + j) * 128
                            nc.sync.dma_start(out=wt[:], in_=WQ1[r0:r0 + 128, :])
                            for mm in range(2):
                                m = 2 * bh + mm
                                nc.tensor.matmul(psq[mm][:],
                                                 xqT[j][:, m * 128:(m + 1) * 128], wt[:],
                                                 start=(j == 0), stop=(j == NHT - 1))
                        for mm in range(2):
                            m = 2 * bh + mm
                            par = m % 2
                            if ng < 5:
                                qc_t = pQC.tile([128, 512], f32, tag="qc")
                                nc.vector.tensor_scalar(qc_t[:], psq[mm][:], d1s[m][:],
                                                        None, ALU.mult)
                                xv = qc_t[:].rearrange("p (h t d) -> p h t d", t=2, d=32)
                                xr = xv[:, :, 0, :]
                                xi = xv[:, :, 1, :]
                                cv = cosr[par][:].rearrange("p (h d) -> p h d", d=32)
                                sv = sinr[par][:].rearrange("p (h d) -> p h d", d=32)
                                ov = sends[m][:, ng * 512:(ng + 1) * 512].rearrange(
                                    "p (h t d) -> p h t d", t=2, d=32)
                                o_r = ov[:, :, 0, :]
                                o_i = ov[:, :, 1, :]
                                ta = pRT.tile([128, 256], f32, tag="ta")
                                tb = pRT.tile([128, 256], f32, tag="tb")
                                tav = ta[:].rearrange("p (h d) -> p h d", d=32)
                                tbv = tb[:].rearrange("p (h d) -> p h d", d=32)
                                nc.vector.tensor_tensor(tav, xr, cv, ALU.mult)
                                nc.vector.tensor_tensor(tbv, xi, sv, ALU.mult)
                                nc.vector.tensor_tensor(o_r, tav, tbv, ALU.subtract)
                                nc.vector.tensor_tensor(tav, xr, sv, ALU.mult)
                                nc.vector.tensor_tensor(tbv, xi, cv, ALU.mult)
                                nc.vector.tensor_tensor(o_i, tav, tbv, ALU.add)
                            else:
                                nc.vector.tensor_scalar(sends[m][:, ng * 512:(ng + 1) * 512],
                                                        psq[mm][:], d1s[m][:], None, ALU.mult)

                    # scatter this batch's sends, then fire its AllToAll
                    for mm in range(2):
                        m = 2 * bh + mm
                        par = m % 2
                        dst = a1i[bh][:]
                        base = par * 128 * A1W
                        nc.sync.dma_start(
                            out=_dap(dst, base + 0,
                                     [[A1W, 128], [SC * A1W, 8], [1, 256]]),
                            in_=sends[m][:, 0:2048].rearrange("p (j c) -> p j c", j=8))
                        for koff in (CK, CKD):
                            nc.sync.dma_start(
                                out=_dap(dst, base + koff,
                                         [[A1W, 128], [SC * A1W, 8], [1, 64]]),
                                in_=sends[m][:, 2048:2560].rearrange("p (j c) -> p j c", j=8))
                        nc.sync.dma_start(
                            out=_dap(dst, base + CV,
                                     [[A1W, 128], [SC * A1W, 8], [1, 64]]),
                            in_=sends[m][:, 2560:3072].rearrange("p (j c) -> p j c", j=8))
                    nc.gpsimd.collective_compute(
                        "AllToAll", ALU.bypass, replica_groups=[list(range(C))],
                        ins=[a1i[bh][:].opt()], outs=[a1o[bh][:].opt()])

            # ================= Stage C: attention =========================
            with ExitStack() as sc:
                pQT = sc.enter_context(tc.tile_pool(name="pQT", bufs=4))
                pKT = sc.enter_context(tc.tile_pool(name="pKT", bufs=2))
                pVA = sc.enter_context(tc.tile_pool(name="pVA", bufs=2 * NKT))
                pEX = sc.enter_context(tc.tile_pool(name="pEX", bufs=NKT))
                pOB = sc.enter_context(tc.tile_pool(name="pOB", bufs=2 * NKT))
                pR = sc.enter_context(tc.tile_pool(name="pR", bufs=8))

                for b in range(B):
                    src = a1o[b]
                    qTb = []
                    for hp in range(2):
                        t = pQT.tile([128, S], bf16, name=f"qT_{b}_{hp}", tag="qT")
                        nc.sync.dma_start(out=t[:], in_=src[:, hp * 128:(hp + 1) * 128],
                                          transpose=True)
                        qTb.append(t)
                    KB = pKT.tile([128, S], bf16, name=f"KB_{b}", tag="kT")
                    nc.sync.dma_start(out=KB[:], in_=src[:, CK:CK + 128], transpose=True)
                    vab = []
                    for kt in range(NKT):
                        t = pVA.tile([128, 65], bf16, name=f"va_{b}_{kt}", tag="va")
                        nc.sync.dma_start(out=t[:, 0:64],
                                          in_=src[kt * 128:(kt + 1) * 128, CV:CV + 64])
                        nc.vector.memset(t[:, 64:65], 1.0)
                        vab.append(t)
                    obs = [pOB.tile([128, G * HD], bf16, name=f"ob_{b}_{qt}", tag="ob")
                           for qt in range(NKT)]

                    for hp in range(2):
                        qTx = qTb[hp]
                        for qc in range(4):
                            exs = []
                            for kt in range(4 * qc + 4):
                                dpos = max(0, kt * 128 - qc * 512)
                                st = psS.tile([128, 1024], f32, tag="st",
                                              name=f"st_{b}_{hp}_{qc}_{kt}")
                                nc.tensor.matmul(
                                    st[:, dpos:512],
                                    KB[0:64, kt * 128:(kt + 1) * 128],
                                    qTx[0:64, qc * 512 + dpos:(qc + 1) * 512],
                                    start=True, stop=True)
                                nc.tensor.matmul(
                                    st[:, 512 + dpos:1024],
                                    KB[64:128, kt * 128:(kt + 1) * 128],
                                    qTx[64:128, qc * 512 + dpos:(qc + 1) * 512],
                                    start=True, stop=True, tile_position=(64, 0))
                                ex = pEX.tile([128, 1024], bf16, tag="ex",
                                              name=f"ex_{b}_{hp}_{qc}_{kt}")
                                stv = st[:].rearrange("p (h q) -> p h q", h=2)[:, :, dpos:512]
                                exv = ex[:].rearrange("p (h q) -> p h q", h=2)[:, :, dpos:512]
                                nc.scalar.activation(out=exv, in_=stv, func=FT.Exp, scale=0.125)
                                if kt >= 4 * qc:
                                    for h in range(2):
                                        sl = ex[:, h * 512 + dpos:h * 512 + dpos + 128]
                                        nc.vector.tensor_tensor(sl, sl, trim[:], ALU.mult)
                                exs.append(ex)
                            for h in range(2):
                                for qtl in range(4):
                                    qt = 4 * qc + qtl
                                    op = psA.tile([128, 65], f32, tag="acc",
                                                  name=f"op_{b}_{hp}_{qc}_{h}_{qtl}")
                                    for kt in range(qt + 1):
                                        nc.tensor.matmul(
                                            op[:],
                                            exs[kt][:, h * 512 + qtl * 128:h * 512 + (qtl + 1) * 128],
                                            vab[kt][:],
                                            start=(kt == 0), stop=(kt == qt))
                                    r = pR.tile([128, 1], f32, tag="r")
                                    nc.vector.reciprocal(r[:], op[:, 64:65])
                                    hg = hp * 2 + h
                                    nc.vector.tensor_scalar(
                                        obs[qt][:, hg * 64:(hg + 1) * 64],
                                        op[:, 0:64], r[:], None, ALU.mult)
                    for qt in range(NKT):
                        j = qt // 2
                        rowbase = j * SC + (qt % 2) * 128
                        nc.sync.dma_start(out=a2i[b][rowbase:rowbase + 128, :], in_=obs[qt][:])
                    nc.gpsimd.collective_compute(
                        "AllToAll", ALU.bypass, replica_groups=[list(range(C))],
                        ins=[a2i[b][:].opt()], outs=[a2o[b][:].opt()])

            # ================= Stage D: out projection ====================
            with ExitStack() as sd:
                pD = sd.enter_context(tc.tile_pool(name="pD", bufs=2))
                pDs = sd.enter_context(tc.tile_pool(name="pDs", bufs=4))
                pXT2 = sd.enter_context(tc.tile_pool(name="pXT2", bufs=NHT * 2))
                pW2 = sd.enter_context(tc.tile_pool(name="pW2", bufs=4))
                pO = sd.enter_context(tc.tile_pool(name="pO", bufs=3))

                xq2T = [[None] * 2 for _ in range(NHT)]
                for bh in range(2):
                    for j in range(NHT):
                        xq2T[j][bh] = pXT2.tile([128, 256], bf16,
                                                name=f"xq2T_{j}_{bh}", tag="xq2T")
                for m in range(NT):
                    b = m // 2
                    r0 = (m % 2) * 128
                    x2 = pD.tile([128, H], bf16, tag="x2")
                    nc.sync.dma_start(
                        out=x2[:],
                        in_=_dap(a2o[b][:], r0 * 256,
                                 [[256, 128], [SC * 256, 8], [1, 256]]))
                    mx2 = pDs.tile([128, 1], f32, tag="mx2")
                    nc.vector.tensor_reduce(mx2[:], x2[:], mybir.AxisListType.X, ALU.max,
                                            apply_absolute_value=True)
                    mp2 = pDs.tile([128, 1], f32, tag="mp2")
                    nc.vector.tensor_scalar(mp2[:], mx2[:], 1e-5, None, ALU.max)
                    nc.vector.tensor_tensor(d2s[m][:], mp2[:], sw2b[:], ALU.mult)
                    rm2 = pDs.tile([128, 1], f32, tag="rm2")
                    nc.vector.reciprocal(rm2[:], mp2[:])
                    c2 = pDs.tile([128, 1], f32, tag="c2")
                    nc.vector.tensor_scalar(c2[:], rm2[:], 127.0, None, ALU.mult)
                    t2 = pD.tile([128, H], f32, tag="t2")
                    nc.gpsimd.tensor_scalar(t2[:], x2[:], c2[:], MAGIC, ALU.mult, ALU.add)
                    xq2 = pD.tile([128, H], bf16, tag="xq2")
                    nc.vector.tensor_scalar(xq2[:], t2[:], MAGIC, None, ALU.subtract)
                    for j in range(NHT):
                        tp = psS.tile([128, 128], bf16, tag="st", name=f"tp2_{m}_{j}")
                        nc.tensor.transpose(tp[:], xq2[:, j * 128:(j + 1) * 128], ident[:])
                        nc.vector.tensor_copy(
                            xq2T[j][b][:, (m % 2) * 128:(m % 2 + 1) * 128], tp[:])

                for bh in range(B):
                    for ng in range(H // 512):
                        ps2 = [psA.tile([128, 512], f32, tag="acc", name=f"ps2_{bh}_{ng}_{mm}")
                               for mm in range(2)]
                        for j in range(NHT):
                            wt = pW2.tile([128, 512], bf16, tag="w2")
                            r0 = (ng * NHT + j) * 128
                            nc.sync.dma_start(out=wt[:], in_=WQ2[r0:r0 + 128, :])
                            for mm in range(2):
                                nc.tensor.matmul(
                                    ps2[mm][:],
                                    xq2T[j][bh][:, mm * 128:(mm + 1) * 128],
                                    wt[:], start=(j == 0), stop=(j == NHT - 1))
                        for mm in range(2):
                            m = 2 * bh + mm
                            ot = pO.tile([128, 512], f32, tag="ot")
                            nc.vector.tensor_scalar(ot[:], ps2[mm][:], d2s[m][:], None, ALU.mult)
                            nc.sync.dma_start(
                                out=OUT[m * 128:(m + 1) * 128, ng * 512:(ng + 1) * 512],
                                in_=ot[:])

    nc.compile()
    return nc


_NC_CACHE = {}


def _get_nc():
    if "nc" not in _NC_CACHE:
        _NC_CACHE["nc"] = build_nc()
    return _NC_CACHE["nc"]


def _stripe(wt, nchunk):
    """[H, O] -> [(O//512)*16*128, 512] contiguous (ng, j)-stripe layout."""
    Hh, O = wt.shape
    a = wt.reshape(NHT, 128, O // 512, 512)          # [j, h, ng, c]
    a = a.transpose(2, 0, 1, 3)                      # [ng, j, h, c]
    return np.ascontiguousarray(a.reshape(-1, 512))


def kernel(x, w_norm, w_qkv, w_out):
    x = np.asarray(x, dtype=np.float32)
    w_norm = np.asarray(w_norm, dtype=np.float32)
    w_qkv = np.asarray(w_qkv, dtype=np.float32)
    w_out = np.asarray(w_out, dtype=np.float32)

    def tern(w):
        ws = np.float32(1.0) / np.clip(np.mean(np.abs(w)), np.float32(1e-5), None).astype(np.float32)
        wq = np.clip(np.round(w * ws), -1.0, 1.0).astype(np.float32)
        return wq, (np.float32(1.0) / ws).astype(np.float32)

    wq1, s_w1 = tern(w_qkv)
    wq2, s_w2 = tern(w_out)
    wq1t = _stripe(np.ascontiguousarray(wq1.T), QKV_O // 512).astype(ml_dtypes.bfloat16)
    wq2t = _stripe(np.ascontiguousarray(wq2.T), H // 512).astype(ml_dtypes.bfloat16)

    inv_freq = (1.0 / THETA ** (np.arange(0, HD, 2, dtype=np.float32) / HD)).astype(np.float32)
    t_pos = np.arange(S, dtype=np.float32)
    freqs = t_pos[:, None] * inv_freq[None, :]
    cos_full = np.cos(freqs).astype(np.float32)
    sin_full = np.sin(freqs).astype(np.float32)

    trimask = np.triu(np.ones((128, 128), np.float32)).astype(ml_dtypes.bfloat16)
    sw1 = np.array([[s_w1 / np.float32(127.0)]], dtype=np.float32)
    sw2 = np.array([[s_w2 / np.float32(127.0)]], dtype=np.float32)
    wn2d = w_norm.reshape(1, H)

    in_maps = []
    for i in range(C):
        xc = np.ascontiguousarray(
            np.concatenate([x[0, i * SC:(i + 1) * SC, :], x[1, i * SC:(i + 1) * SC, :]], axis=0))
        in_maps.append({
            "x": xc,
            "wn": wn2d,
            "wq1t": wq1t,
            "wq2t": wq2t,
            "cosb": np.ascontiguousarray(np.tile(cos_full[i * SC:(i + 1) * SC, :], (1, 8))),
            "sinb": np.ascontiguousarray(np.tile(sin_full[i * SC:(i + 1) * SC, :], (1, 8))),
            "trimask": trimask,
            "sw1": sw1,
            "sw2": sw2,
        })

    nc = _get_nc()
    res = bass_utils.run_bass_kernel_spmd(nc, in_maps, core_ids=list(range(C)))

    out = np.empty((B, S, H), dtype=np.float32)
    for i in range(C):
        ci = res.results[i]["out"]
        for b in range(B):
            out[b, i * SC:(i + 1) * SC, :] = ci[b * SC:(b + 1) * SC, :]
    return out


# revision 33
# speedup vs baseline: 1.2060x; 1.2060x over previous
"""Distributed Trainium2 Bass kernel for BitNet-style attention block.

Sharding: sequence-parallel projections + (batch x kv-head) parallel attention,
stitched with per-batch AllToAll collectives (split so comm overlaps compute).

Per core (core i):
  A. RMSNorm + per-token absmax quantization of its 512-token chunk.
  B. qkv projection as exact integer bf16 matmul against host-prequantized
     ternary weights, dequant, RoPE on q/k, scatter into per-batch A2A bufs.
  C. AllToAll #1a/#1b -> core i holds full-sequence q/k/v for kv-head i of
     each batch; causal attention (transposed scores, exp on ACT, ones-column
     rowsums, deferred normalization). Batch 1 attention overlaps A2A #2a.
  D. AllToAll #2a/#2b -> core i holds its token chunk of all 32 heads;
     per-token quantization + integer matmul with ternary output weights.
"""
import sys
sys.path.insert(0, "/opt/trn_rl_repo")
import numpy as np
import ml_dtypes
import concourse.bass as bass
import concourse.tile as tile
from concourse import bacc, mybir
from concourse import bass_utils
from concourse.masks import make_identity

f32 = mybir.dt.float32
bf16 = mybir.dt.bfloat16
FT = mybir.ActivationFunctionType
ALU = mybir.AluOpType

B, S, H = 2, 2048, 2048
NH, NKV, HD = 32, 8, 64
G = NH // NKV                    # 4
QKV_O = (NH + 2 * NKV) * HD      # 3072
EPS = 1e-5
THETA = 10000.0
C = 8
SC = S // C                      # 256 positions per core
TOK = B * SC                     # 512 token rows per core
MAGIC = float(1.5 * 2.0 ** 23)   # RNE integer rounding for |v| < 2^22
NT = TOK // 128                  # 4 token tiles
NHT = H // 128                   # 16 h-tiles
NKT = S // 128                   # 16 kj tiles

# a2a1 split into two column-group collectives:
#   kv: per-batch [k 64 | kdup 64 | v 64] -> [8, 256, 384]
#   q:  per-batch [q 256]                 -> [8, 256, 512]
KVW = 2 * 3 * HD                 # 384
QW = 2 * G * HD                  # 512


def _dap(t_ap, extra, dims):
    return bass.AP(tensor=t_ap.tensor, offset=t_ap.offset + extra, ap=[list(d) for d in dims])


def build_nc():
    nc = bacc.Bacc("TRN2", target_bir_lowering=False, debug=False, num_devices=C)

    x_in = nc.dram_tensor("x", [TOK, H], f32, kind="ExternalInput")
    wn_in = nc.dram_tensor("wn", [1, H], f32, kind="ExternalInput")
    # contiguous stripes: row ((ng*16+j)*128 + h_local), 512 cols each
    wq1t_in = nc.dram_tensor("wq1t", [(QKV_O // 512) * NHT * 128, 512], bf16, kind="ExternalInput")
    wq2t_in = nc.dram_tensor("wq2t", [(H // 512) * NHT * 128, 512], bf16, kind="ExternalInput")
    cos_in = nc.dram_tensor("cosb", [SC, 8 * 32], f32, kind="ExternalInput")
    sin_in = nc.dram_tensor("sinb", [SC, 8 * 32], f32, kind="ExternalInput")
    tri_in = nc.dram_tensor("trimask", [128, 128], bf16, kind="ExternalInput")
    sw1_in = nc.dram_tensor("sw1", [1, 1], f32, kind="ExternalInput")
    sw2_in = nc.dram_tensor("sw2", [1, 1], f32, kind="ExternalInput")
    out_ext = nc.dram_tensor("out", [TOK, H], f32, kind="ExternalOutput")

    X = x_in.ap()
    WQ1 = wq1t_in.ap()
    WQ2 = wq2t_in.ap()
    OUT = out_ext.ap()

    with tile.TileContext(nc) as tc:
        from contextlib import ExitStack
        with ExitStack() as top:
            dram = top.enter_context(tc.tile_pool(name="dram", bufs=1, space="DRAM"))
            const = top.enter_context(tc.tile_pool(name="const", bufs=1))
            smalls = top.enter_context(tc.tile_pool(name="smalls", bufs=1))
            psA = top.enter_context(tc.tile_pool(name="psA", bufs=4, space="PSUM"))
            psS = top.enter_context(tc.tile_pool(name="psS", bufs=2, space="PSUM"))

            # ---------------- DRAM scratch ----------------
            akv_i = dram.tile([C * SC, KVW], bf16, name="akv_i")
            akv_o = dram.tile([C * SC, KVW], bf16, name="akv_o")
            aq_i = dram.tile([C * SC, QW], bf16, name="aq_i")
            aq_o = dram.tile([C * SC, QW], bf16, name="aq_o")
            a2i = [dram.tile([C * SC, G * HD], bf16, name=f"a2i_{b}") for b in range(B)]
            a2o = [dram.tile([C * SC, G * HD], bf16, name=f"a2o_{b}") for b in range(B)]

            # ---------------- constants ----------------
            wnorm_b = const.tile([128, H], f32)
            nc.sync.dma_start(out=wnorm_b[:], in_=_dap(wn_in.ap(), 0, [[0, 128], [1, H]]))
            trim = const.tile([128, 128], bf16)
            nc.sync.dma_start(out=trim[:], in_=tri_in.ap()[:, :])
            sw1b = const.tile([128, 1], f32)
            nc.sync.dma_start(out=sw1b[:], in_=_dap(sw1_in.ap(), 0, [[0, 128], [1, 1]]))
            sw2b = const.tile([128, 1], f32)
            nc.sync.dma_start(out=sw2b[:], in_=_dap(sw2_in.ap(), 0, [[0, 128], [1, 1]]))
            epsb = const.tile([128, 1], f32)
            nc.vector.memset(epsb[:], EPS)
            ident = const.tile([128, 128], bf16)
            make_identity(nc, ident[:])

            d1s = [smalls.tile([128, 1], f32, name=f"d1_{m}") for m in range(NT)]
            d2s = [smalls.tile([128, 1], f32, name=f"d2_{m}") for m in range(NT)]

            xqT_pool = top.enter_context(tc.tile_pool(name="xqT", bufs=NHT))

            # ================= Stage A: RMSNorm + quantize =================
            with ExitStack() as sa:
                pA = sa.enter_context(tc.tile_pool(name="pA", bufs=2))
                pXQ = sa.enter_context(tc.tile_pool(name="pXQ", bufs=NT))
                pSc = sa.enter_context(tc.tile_pool(name="pASc", bufs=4))
                xqms = []
                for m in range(NT):
                    xa = pA.tile([128, H], f32, tag="xa")
                    nc.sync.dma_start(out=xa[:], in_=X[m * 128:(m + 1) * 128, :])
                    sq = pA.tile([128, H], f32, tag="sq")
                    ssq = pSc.tile([128, 1], f32, tag="ssq")
                    nc.scalar.activation(out=sq[:], in_=xa[:], func=FT.Square, accum_out=ssq[:])
                    xw = pA.tile([128, H], f32, tag="xw")
                    nc.vector.tensor_tensor(xw[:], xa[:], wnorm_b[:], ALU.mult)
                    std = pSc.tile([128, 1], f32, tag="std")
                    nc.scalar.activation(out=std[:], in_=ssq[:], func=FT.Sqrt,
                                         bias=epsb[:], scale=1.0 / H)
                    rstd = pSc.tile([128, 1], f32, tag="rstd")
                    nc.vector.reciprocal(rstd[:], std[:])
                    mx = pSc.tile([128, 1], f32, tag="mx")
                    nc.vector.tensor_reduce(mx[:], xw[:], mybir.AxisListType.X, ALU.max,
                                            apply_absolute_value=True)
                    mp = pSc.tile([128, 1], f32, tag="mp")
                    nc.vector.tensor_scalar(mp[:], mx[:], rstd[:], 1e-5, ALU.mult, ALU.max)
                    nc.vector.tensor_tensor(d1s[m][:], mp[:], sw1b[:], ALU.mult)
                    rmp = pSc.tile([128, 1], f32, tag="rmp")
                    nc.vector.reciprocal(rmp[:], mp[:])
                    csc = pSc.tile([128, 1], f32, tag="csc")
                    nc.vector.tensor_scalar(csc[:], rmp[:], rstd[:], 127.0, ALU.mult, ALU.mult)
                    t1 = pA.tile([128, H], f32, tag="t1")
                    nc.gpsimd.tensor_scalar(t1[:], xw[:], csc[:], MAGIC, ALU.mult, ALU.add)
                    xqm = pXQ.tile([128, H], bf16, tag="xqm", name=f"xqm_{m}")
                    nc.vector.tensor_scalar(xqm[:], t1[:], MAGIC, None, ALU.subtract)
                    xqms.append(xqm)

                # transposed activations via PE (keeps the DMA queues free)
                xqT = []
                for j in range(NHT):
                    t = xqT_pool.tile([128, TOK], bf16, name=f"xqT_{j}", tag="xqT")
                    xqT.append(t)
                for m in range(NT):
                    for j in range(NHT):
                        tp = psS.tile([128, 128], bf16, tag="st", name=f"tp_{m}_{j}")
                        nc.tensor.transpose(tp[:], xqms[m][:, j * 128:(j + 1) * 128], ident[:])
                        nc.vector.tensor_copy(xqT[j][:, m * 128:(m + 1) * 128], tp[:])

            # ================= Stage B: qkv matmul + RoPE + scatter ========
            with ExitStack() as sb:
                pW = sb.enter_context(tc.tile_pool(name="pW", bufs=4))
                pQC = sb.enter_context(tc.tile_pool(name="pQC", bufs=3))
                pRT = sb.enter_context(tc.tile_pool(name="pRT", bufs=2))
                pSend = sb.enter_context(tc.tile_pool(name="pSend", bufs=NT))
                pCos = sb.enter_context(tc.tile_pool(name="pCos", bufs=1))

                cosr = []
                sinr = []
                for par in range(2):
                    ct = pCos.tile([128, 8 * 32], f32, name=f"cosr_{par}")
                    nc.sync.dma_start(out=ct[:], in_=cos_in.ap()[par * 128:(par + 1) * 128, :])
                    st_ = pCos.tile([128, 8 * 32], f32, name=f"sinr_{par}")
                    nc.sync.dma_start(out=st_[:], in_=sin_in.ap()[par * 128:(par + 1) * 128, :])
                    cosr.append(ct)
                    sinr.append(st_)

                sends = [pSend.tile([128, QKV_O], bf16, name=f"sends_{m}", tag="sends")
                         for m in range(NT)]

                NQC = QKV_O // 512   # 6 chunks of 512; process k/v chunks first
                for ng in (4, 5, 0, 1, 2, 3):
                    psq = [psA.tile([128, 512], f32, tag="acc", name=f"qkvp_{ng}_{m}")
                           for m in range(NT)]
                    for j in range(NHT):
                        wt = pW.tile([128, 512], bf16, tag="w1")
                        r0 = (ng * NHT + j) * 128
                        nc.sync.dma_start(out=wt[:], in_=WQ1[r0:r0 + 128, :])
                        for m in range(NT):
                            nc.tensor.matmul(psq[m][:], xqT[j][:, m * 128:(m + 1) * 128], wt[:],
                                             start=(j == 0), stop=(j == NHT - 1))
                    for m in range(NT):
                        par = m % 2
                        if ng < 5:
                            qc_t = pQC.tile([128, 512], f32, tag="qc")
                            nc.vector.tensor_scalar(qc_t[:], psq[m][:], d1s[m][:], None, ALU.mult)
                            xv = qc_t[:].rearrange("p (h t d) -> p h t d", t=2, d=32)
                            xr = xv[:, :, 0, :]
                            xi = xv[:, :, 1, :]
                            cv = cosr[par][:].rearrange("p (h d) -> p h d", d=32)
                            sv = sinr[par][:].rearrange("p (h d) -> p h d", d=32)
                            ov = sends[m][:, ng * 512:(ng + 1) * 512].rearrange(
                                "p (h t d) -> p h t d", t=2, d=32)
                            o_r = ov[:, :, 0, :]
                            o_i = ov[:, :, 1, :]
                            ta = pRT.tile([128, 256], f32, tag="ta")
                            tb = pRT.tile([128, 256], f32, tag="tb")
                            tav = ta[:].rearrange("p (h d) -> p h d", d=32)
                            tbv = tb[:].rearrange("p (h d) -> p h d", d=32)
                            nc.vector.tensor_tensor(tav, xr, cv, ALU.mult)
                            nc.vector.tensor_tensor(tbv, xi, sv, ALU.mult)
                            nc.vector.tensor_tensor(o_r, tav, tbv, ALU.subtract)
                            nc.vector.tensor_tensor(tav, xr, sv, ALU.mult)
                            nc.vector.tensor_tensor(tbv, xi, cv, ALU.mult)
                            nc.vector.tensor_tensor(o_i, tav, tbv, ALU.add)
                        else:
                            nc.vector.tensor_scalar(sends[m][:, ng * 512:(ng + 1) * 512],
                                                    psq[m][:], d1s[m][:], None, ALU.mult)

                    if ng == 5:
                        # k/v chunks done for all m: scatter + fire the kv AllToAll,
                        # overlapping the q chunks' matmuls
                        for m in range(NT):
                            b = m // 2
                            par = m % 2
                            base = par * 128 * KVW + b * 192
                            for koff in (0, 64):   # k and its duplicate
                                nc.sync.dma_start(
                                    out=_dap(akv_i[:], base + koff,
                                             [[KVW, 128], [SC * KVW, 8], [1, 64]]),
                                    in_=sends[m][:, 2048:2560].rearrange(
                                        "p (j c) -> p j c", j=8))
                            nc.sync.dma_start(
                                out=_dap(akv_i[:], base + 128,
                                         [[KVW, 128], [SC * KVW, 8], [1, 64]]),
                                in_=sends[m][:, 2560:3072].rearrange("p (j c) -> p j c", j=8))
                        nc.gpsimd.collective_compute(
                            "AllToAll", ALU.bypass, replica_groups=[list(range(C))],
                            ins=[akv_i[:].opt()], outs=[akv_o[:].opt()])

                # q scatter + q AllToAll
                for m in range(NT):
                    b = m // 2
                    par = m % 2
                    base = par * 128 * QW + b * 256
                    nc.sync.dma_start(
                        out=_dap(aq_i[:], base,
                                 [[QW, 128], [SC * QW, 8], [1, 256]]),
                        in_=sends[m][:, 0:2048].rearrange("p (j c) -> p j c", j=8))
                nc.gpsimd.collective_compute(
                    "AllToAll", ALU.bypass, replica_groups=[list(range(C))],
                    ins=[aq_i[:].opt()], outs=[aq_o[:].opt()])

            # ================= Stage C: attention =========================
            with ExitStack() as sc:
                pQT = sc.enter_context(tc.tile_pool(name="pQT", bufs=4))
                pKT = sc.enter_context(tc.tile_pool(name="pKT", bufs=2))
                pVA = sc.enter_context(tc.tile_pool(name="pVA", bufs=2 * NKT))
                pEX = sc.enter_context(tc.tile_pool(name="pEX", bufs=NKT))
                pOB = sc.enter_context(tc.tile_pool(name="pOB", bufs=2 * NKT))
                pR = sc.enter_context(tc.tile_pool(name="pR", bufs=8))

                # k/v loads for both batches first (kv collective lands early)
                KBs = []
                vas = []
                for b in range(B):
                    KB = pKT.tile([128, S], bf16, name=f"KB_{b}", tag="kT")
                    nc.sync.dma_start(out=KB[:], in_=akv_o[:, b * 192:b * 192 + 128],
                                      transpose=True)
                    KBs.append(KB)
                    vab = []
                    for kt in range(NKT):
                        t = pVA.tile([128, 65], bf16, name=f"va_{b}_{kt}", tag="va")
                        nc.sync.dma_start(
                            out=t[:, 0:64],
                            in_=akv_o[kt * 128:(kt + 1) * 128, b * 192 + 128:b * 192 + 192])
                        nc.vector.memset(t[:, 64:65], 1.0)
                        vab.append(t)
                    vas.append(vab)

                for b in range(B):
                    KB = KBs[b]
                    vab = vas[b]
                    qTb = []
                    for hp in range(2):
                        t = pQT.tile([128, S], bf16, name=f"qT_{b}_{hp}", tag="qT")
                        nc.sync.dma_start(
                            out=t[:], in_=aq_o[:, b * 256 + hp * 128:b * 256 + (hp + 1) * 128],
                            transpose=True)
                        qTb.append(t)
                    obs = [pOB.tile([128, G * HD], bf16, name=f"ob_{b}_{qt}", tag="ob")
                           for qt in range(NKT)]

                    for hp in range(2):
                        qTx = qTb[hp]
                        for qc in range(4):
                            exs = []
                            for kt in range(4 * qc + 4):
                                dpos = max(0, kt * 128 - qc * 512)
                                st = psS.tile([128, 1024], f32, tag="st",
                                              name=f"st_{b}_{hp}_{qc}_{kt}")
                                nc.tensor.matmul(
                                    st[:, dpos:512],
                                    KB[0:64, kt * 128:(kt + 1) * 128],
                                    qTx[0:64, qc * 512 + dpos:(qc + 1) * 512],
                                    start=True, stop=True)
                                nc.tensor.matmul(
                                    st[:, 512 + dpos:1024],
                                    KB[64:128, kt * 128:(kt + 1) * 128],
                                    qTx[64:128, qc * 512 + dpos:(qc + 1) * 512],
                                    start=True, stop=True, tile_position=(64, 0))
                                ex = pEX.tile([128, 1024], bf16, tag="ex",
                                              name=f"ex_{b}_{hp}_{qc}_{kt}")
                                stv = st[:].rearrange("p (h q) -> p h q", h=2)[:, :, dpos:512]
                                exv = ex[:].rearrange("p (h q) -> p h q", h=2)[:, :, dpos:512]
                                nc.scalar.activation(out=exv, in_=stv, func=FT.Exp, scale=0.125)
                                if kt >= 4 * qc:
                                    for h in range(2):
                                        sl = ex[:, h * 512 + dpos:h * 512 + dpos + 128]
                                        nc.vector.tensor_tensor(sl, sl, trim[:], ALU.mult)
                                exs.append(ex)
                            for h in range(2):
                                for qtl in range(4):
                                    qt = 4 * qc + qtl
                                    op = psA.tile([128, 65], f32, tag="acc",
                                                  name=f"op_{b}_{hp}_{qc}_{h}_{qtl}")
                                    for kt in range(qt + 1):
                                        nc.tensor.matmul(
                                            op[:],
                                            exs[kt][:, h * 512 + qtl * 128:h * 512 + (qtl + 1) * 128],
                                            vab[kt][:],
                                            start=(kt == 0), stop=(kt == qt))
                                    r = pR.tile([128, 1], f32, tag="r")
                                    nc.vector.reciprocal(r[:], op[:, 64:65])
                                    hg = hp * 2 + h
                                    nc.vector.tensor_scalar(
                                        obs[qt][:, hg * 64:(hg + 1) * 64],
                                        op[:, 0:64], r[:], None, ALU.mult)
                    for qt in range(NKT):
                        j = qt // 2
                        rowbase = j * SC + (qt % 2) * 128
                        nc.sync.dma_start(out=a2i[b][rowbase:rowbase + 128, :], in_=obs[qt][:])
                    nc.gpsimd.collective_compute(
                        "AllToAll", ALU.bypass, replica_groups=[list(range(C))],
                        ins=[a2i[b][:].opt()], outs=[a2o[b][:].opt()])

            # ================= Stage D: out projection ====================
            with ExitStack() as sd:
                pD = sd.enter_context(tc.tile_pool(name="pD", bufs=2))
                pDs = sd.enter_context(tc.tile_pool(name="pDs", bufs=4))
                pXT2 = sd.enter_context(tc.tile_pool(name="pXT2", bufs=NHT * 2))
                pW2 = sd.enter_context(tc.tile_pool(name="pW2", bufs=4))
                pO = sd.enter_context(tc.tile_pool(name="pO", bufs=3))

                xq2T = [[None] * 2 for _ in range(NHT)]
                for bh in range(2):
                    for j in range(NHT):
                        xq2T[j][bh] = pXT2.tile([128, 256], bf16,
                                                name=f"xq2T_{j}_{bh}", tag="xq2T")
                for m in range(NT):
                    b = m // 2
                    r0 = (m % 2) * 128
                    x2 = pD.tile([128, H], bf16, tag="x2")
                    nc.sync.dma_start(
                        out=x2[:],
                        in_=_dap(a2o[b][:], r0 * 256,
                                 [[256, 128], [SC * 256, 8], [1, 256]]))
                    mx2 = pDs.tile([128, 1], f32, tag="mx2")
                    nc.vector.tensor_reduce(mx2[:], x2[:], mybir.AxisListType.X, ALU.max,
                                            apply_absolute_value=True)
                    mp2 = pDs.tile([128, 1], f32, tag="mp2")
                    nc.vector.tensor_scalar(mp2[:], mx2[:], 1e-5, None, ALU.max)
                    nc.vector.tensor_tensor(d2s[m][:], mp2[:], sw2b[:], ALU.mult)
                    rm2 = pDs.tile([128, 1], f32, tag="rm2")
                    nc.vector.reciprocal(rm2[:], mp2[:])
                    c2 = pDs.tile([128, 1], f32, tag="c2")
                    nc.vector.tensor_scalar(c2[:], rm2[:], 127.0, None, ALU.mult)
                    t2 = pD.tile([128, H], f32, tag="t2")
                    nc.gpsimd.tensor_scalar(t2[:], x2[:], c2[:], MAGIC, ALU.mult, ALU.add)
                    xq2 = pD.tile([128, H], bf16, tag="xq2")
                    nc.vector.tensor_scalar(xq2[:], t2[:], MAGIC, None, ALU.subtract)
                    for j in range(NHT):
                        tp = psS.tile([128, 128], bf16, tag="st", name=f"tp2_{m}_{j}")
                        nc.tensor.transpose(tp[:], xq2[:, j * 128:(j + 1) * 128], ident[:])
                        nc.vector.tensor_copy(
                            xq2T[j][b][:, (m % 2) * 128:(m % 2 + 1) * 128], tp[:])

                for ng in range(H // 512):
                    ps2 = [psA.tile([128, 512], f32, tag="acc", name=f"ps2_{ng}_{m}")
                           for m in range(NT)]
                    for j in range(NHT):
                        wt = pW2.tile([128, 512], bf16, tag="w2")
                        r0 = (ng * NHT + j) * 128
                        nc.sync.dma_start(out=wt[:], in_=WQ2[r0:r0 + 128, :])
                        for m in range(NT):
                            nc.tensor.matmul(
                                ps2[m][:],
                                xq2T[j][m // 2][:, (m % 2) * 128:(m % 2 + 1) * 128],
                                wt[:], start=(j == 0), stop=(j == NHT - 1))
                    for m in range(NT):
                        ot = pO.tile([128, 512], f32, tag="ot")
                        nc.vector.tensor_scalar(ot[:], ps2[m][:], d2s[m][:], None, ALU.mult)
                        nc.sync.dma_start(
                            out=OUT[m * 128:(m + 1) * 128, ng * 512:(ng + 1) * 512], in_=ot[:])

    nc.compile()
    return nc


_NC_CACHE = {}


def _get_nc():
    if "nc" not in _NC_CACHE:
        _NC_CACHE["nc"] = build_nc()
    return _NC_CACHE["nc"]


def _stripe(wt, nchunk):
    """[H, O] -> [(O//512)*16*128, 512] contiguous (ng, j)-stripe layout."""
    Hh, O = wt.shape
    a = wt.reshape(NHT, 128, O // 512, 512)          # [j, h, ng, c]
    a = a.transpose(2, 0, 1, 3)                      # [ng, j, h, c]
    return np.ascontiguousarray(a.reshape(-1, 512))


def kernel(x, w_norm, w_qkv, w_out):
    x = np.asarray(x, dtype=np.float32)
    w_norm = np.asarray(w_norm, dtype=np.float32)
    w_qkv = np.asarray(w_qkv, dtype=np.float32)
    w_out = np.asarray(w_out, dtype=np.float32)

    def tern(w):
        ws = np.float32(1.0) / np.clip(np.mean(np.abs(w)), np.float32(1e-5), None).astype(np.float32)
        wq = np.clip(np.round(w * ws), -1.0, 1.0).astype(np.float32)
        return wq, (np.float32(1.0) / ws).astype(np.float32)

    wq1, s_w1 = tern(w_qkv)
    wq2, s_w2 = tern(w_out)
    wq1t = _stripe(np.ascontiguousarray(wq1.T), QKV_O // 512).astype(ml_dtypes.bfloat16)
    wq2t = _stripe(np.ascontiguousarray(wq2.T), H // 512).astype(ml_dtypes.bfloat16)

    inv_freq = (1.0 / THETA ** (np.arange(0, HD, 2, dtype=np.float32) / HD)).astype(np.float32)
    t_pos = np.arange(S, dtype=np.float32)
    freqs = t_pos[:, None] * inv_freq[None, :]
    cos_full = np.cos(freqs).astype(np.float32)
    sin_full = np.sin(freqs).astype(np.float32)

    trimask = np.triu(np.ones((128, 128), np.float32)).astype(ml_dtypes.bfloat16)
    sw1 = np.array([[s_w1 / np.float32(127.0)]], dtype=np.float32)
    sw2 = np.array([[s_w2 / np.float32(127.0)]], dtype=np.float32)
    wn2d = w_norm.reshape(1, H)

    in_maps = []
    for i in range(C):
        xc = np.ascontiguousarray(
            np.concatenate([x[0, i * SC:(i + 1) * SC, :], x[1, i * SC:(i + 1) * SC, :]], axis=0))
        in_maps.append({
            "x": xc,
            "wn": wn2d,
            "wq1t": wq1t,
            "wq2t": wq2t,
            "cosb": np.ascontiguousarray(np.tile(cos_full[i * SC:(i + 1) * SC, :], (1, 8))),
            "sinb": np.ascontiguousarray(np.tile(sin_full[i * SC:(i + 1) * SC, :], (1, 8))),
            "trimask": trimask,
            "sw1": sw1,
            "sw2": sw2,
        })

    nc = _get_nc()
    res = bass_utils.run_bass_kernel_spmd(nc, in_maps, core_ids=list(range(C)))

    out = np.empty((B, S, H), dtype=np.float32)
    for i in range(C):
        ci = res.results[i]["out"]
        for b in range(B):
            out[b, i * SC:(i + 1) * SC, :] = ci[b * SC:(b + 1) * SC, :]
    return out


# revision 39
# speedup vs baseline: 1.2528x; 1.0388x over previous
"""Distributed Trainium2 Bass kernel for BitNet-style attention block.

Sharding: sequence-parallel projections + (batch x kv-head) parallel attention,
stitched with per-batch AllToAll collectives (split so comm overlaps compute).

Per core (core i):
  A. RMSNorm + per-token absmax quantization of its 512-token chunk.
  B. qkv projection as exact integer bf16 matmul against host-prequantized
     ternary weights, dequant, RoPE on q/k, scatter into per-batch A2A bufs.
  C. AllToAll #1a/#1b -> core i holds full-sequence q/k/v for kv-head i of
     each batch; causal attention (transposed scores, exp on ACT, ones-column
     rowsums, deferred normalization). Batch 1 attention overlaps A2A #2a.
  D. AllToAll #2a/#2b -> core i holds its token chunk of all 32 heads;
     per-token quantization + integer matmul with ternary output weights.
"""
import sys
sys.path.insert(0, "/opt/trn_rl_repo")
import numpy as np
import ml_dtypes
import concourse.bass as bass
import concourse.tile as tile
from concourse import bacc, mybir
from concourse import bass_utils
from concourse.masks import make_identity

f32 = mybir.dt.float32
bf16 = mybir.dt.bfloat16
FT = mybir.ActivationFunctionType
ALU = mybir.AluOpType

B, S, H = 2, 2048, 2048
NH, NKV, HD = 32, 8, 64
G = NH // NKV                    # 4
QKV_O = (NH + 2 * NKV) * HD      # 3072
EPS = 1e-5
THETA = 10000.0
C = 8
SC = S // C                      # 256 positions per core
TOK = B * SC                     # 512 token rows per core
MAGIC = float(1.5 * 2.0 ** 23)   # RNE integer rounding for |v| < 2^22
NT = TOK // 128                  # 4 token tiles
NHT = H // 128                   # 16 h-tiles
NKT = S // 128                   # 16 kj tiles

# a2a1 split into two column-group collectives:
#   kv: per-batch [k 64 | kdup 64 | v 64] -> [8, 256, 384]
#   q:  per-batch [q 256]                 -> [8, 256, 512]
KVW = 2 * 3 * HD                 # 384
QW = 2 * G * HD                  # 512


def _dap(t_ap, extra, dims):
    return bass.AP(tensor=t_ap.tensor, offset=t_ap.offset + extra, ap=[list(d) for d in dims])


def build_nc():
    nc = bacc.Bacc("TRN2", target_bir_lowering=False, debug=False, num_devices=C)

    x_in = nc.dram_tensor("x", [TOK, H], f32, kind="ExternalInput")
    wn_in = nc.dram_tensor("wn", [1, H], f32, kind="ExternalInput")
    # contiguous stripes: row ((ng*16+j)*128 + h_local), 512 cols each
    wq1t_in = nc.dram_tensor("wq1t", [(QKV_O // 512) * NHT * 128, 512], bf16, kind="ExternalInput")
    wq2t_in = nc.dram_tensor("wq2t", [(H // 512) * NHT * 128, 512], bf16, kind="ExternalInput")
    cos_in = nc.dram_tensor("cosb", [SC, 8 * 32], f32, kind="ExternalInput")
    sin_in = nc.dram_tensor("sinb", [SC, 8 * 32], f32, kind="ExternalInput")
    tri_in = nc.dram_tensor("trimask", [128, 128], bf16, kind="ExternalInput")
    sw1_in = nc.dram_tensor("sw1", [1, 1], f32, kind="ExternalInput")
    sw2_in = nc.dram_tensor("sw2", [1, 1], f32, kind="ExternalInput")
    out_ext = nc.dram_tensor("out", [TOK, H], f32, kind="ExternalOutput")

    X = x_in.ap()
    WQ1 = wq1t_in.ap()
    WQ2 = wq2t_in.ap()
    OUT = out_ext.ap()

    with tile.TileContext(nc) as tc:
        from contextlib import ExitStack
        with ExitStack() as top:
            dram = top.enter_context(tc.tile_pool(name="dram", bufs=1, space="DRAM"))
            const = top.enter_context(tc.tile_pool(name="const", bufs=1))
            smalls = top.enter_context(tc.tile_pool(name="smalls", bufs=1))
            psA = top.enter_context(tc.tile_pool(name="psA", bufs=4, space="PSUM"))
            psS = top.enter_context(tc.tile_pool(name="psS", bufs=2, space="PSUM"))

            # ---------------- DRAM scratch ----------------
            aq_i = dram.tile([C * SC, QW], bf16, name="aq_i")
            aq_o = dram.tile([C * SC, QW], bf16, name="aq_o")
            ak_i = dram.tile([C * SC, 256], bf16, name="ak_i")
            ak_o = dram.tile([C * SC, 256], bf16, name="ak_o")
            av_i = dram.tile([C * SC, 128], bf16, name="av_i")
            av_o = dram.tile([C * SC, 128], bf16, name="av_o")
            a2i = [dram.tile([C * SC, G * HD], bf16, name=f"a2i_{b}") for b in range(B)]
            a2o = [dram.tile([C * SC, G * HD], bf16, name=f"a2o_{b}") for b in range(B)]

            # ---------------- constants ----------------
            wnorm_b = const.tile([128, H], f32)
            nc.sync.dma_start(out=wnorm_b[:], in_=_dap(wn_in.ap(), 0, [[0, 128], [1, H]]))
            trim = const.tile([128, 128], bf16)
            nc.sync.dma_start(out=trim[:], in_=tri_in.ap()[:, :])
            sw1b = const.tile([128, 1], f32)
            nc.sync.dma_start(out=sw1b[:], in_=_dap(sw1_in.ap(), 0, [[0, 128], [1, 1]]))
            sw2b = const.tile([128, 1], f32)
            nc.sync.dma_start(out=sw2b[:], in_=_dap(sw2_in.ap(), 0, [[0, 128], [1, 1]]))
            epsb = const.tile([128, 1], f32)
            nc.vector.memset(epsb[:], EPS)
            ident = const.tile([128, 128], bf16)
            make_identity(nc, ident[:])

            d1s = [smalls.tile([128, 1], f32, name=f"d1_{m}") for m in range(NT)]
            d2s = [smalls.tile([128, 1], f32, name=f"d2_{m}") for m in range(NT)]

            xqT_pool = top.enter_context(tc.tile_pool(name="xqT", bufs=NHT))
            pQT = top.enter_context(tc.tile_pool(name="pQT", bufs=4))
            pKT = top.enter_context(tc.tile_pool(name="pKT", bufs=2))

            # ================= Stage A: RMSNorm + quantize =================
            with ExitStack() as sa:
                pA = sa.enter_context(tc.tile_pool(name="pA", bufs=2))
                pXQ = sa.enter_context(tc.tile_pool(name="pXQ", bufs=NT))
                pSc = sa.enter_context(tc.tile_pool(name="pASc", bufs=4))
                xqms = []
                for m in range(NT):
                    xa = pA.tile([128, H], f32, tag="xa")
                    nc.sync.dma_start(out=xa[:], in_=X[m * 128:(m + 1) * 128, :])
                    sq = pA.tile([128, H], f32, tag="sq")
                    ssq = pSc.tile([128, 1], f32, tag="ssq")
                    nc.scalar.activation(out=sq[:], in_=xa[:], func=FT.Square, accum_out=ssq[:])
                    xw = pA.tile([128, H], f32, tag="xw")
                    nc.vector.tensor_tensor(xw[:], xa[:], wnorm_b[:], ALU.mult)
                    std = pSc.tile([128, 1], f32, tag="std")
                    nc.scalar.activation(out=std[:], in_=ssq[:], func=FT.Sqrt,
                                         bias=epsb[:], scale=1.0 / H)
                    rstd = pSc.tile([128, 1], f32, tag="rstd")
                    nc.vector.reciprocal(rstd[:], std[:])
                    mx = pSc.tile([128, 1], f32, tag="mx")
                    nc.vector.tensor_reduce(mx[:], xw[:], mybir.AxisListType.X, ALU.max,
                                            apply_absolute_value=True)
                    mp = pSc.tile([128, 1], f32, tag="mp")
                    nc.vector.tensor_scalar(mp[:], mx[:], rstd[:], 1e-5, ALU.mult, ALU.max)
                    nc.vector.tensor_tensor(d1s[m][:], mp[:], sw1b[:], ALU.mult)
                    rmp = pSc.tile([128, 1], f32, tag="rmp")
                    nc.vector.reciprocal(rmp[:], mp[:])
                    csc = pSc.tile([128, 1], f32, tag="csc")
                    nc.vector.tensor_scalar(csc[:], rmp[:], rstd[:], 127.0, ALU.mult, ALU.mult)
                    t1 = pA.tile([128, H], f32, tag="t1")
                    nc.gpsimd.tensor_scalar(t1[:], xw[:], csc[:], MAGIC, ALU.mult, ALU.add)
                    xqm = pXQ.tile([128, H], bf16, tag="xqm", name=f"xqm_{m}")
                    nc.vector.tensor_scalar(xqm[:], t1[:], MAGIC, None, ALU.subtract)
                    xqms.append(xqm)

                # transposed activations via PE (keeps the DMA queues free)
                xqT = []
                for j in range(NHT):
                    t = xqT_pool.tile([128, TOK], bf16, name=f"xqT_{j}", tag="xqT")
                    xqT.append(t)
                for m in range(NT):
                    for j in range(NHT):
                        tp = psS.tile([128, 128], bf16, tag="st", name=f"tp_{m}_{j}")
                        nc.tensor.transpose(tp[:], xqms[m][:, j * 128:(j + 1) * 128], ident[:])
                        nc.vector.tensor_copy(xqT[j][:, m * 128:(m + 1) * 128], tp[:])

            # ================= Stage B: qkv matmul + RoPE + scatter ========
            with ExitStack() as sb:
                pW = sb.enter_context(tc.tile_pool(name="pW", bufs=4))
                pQC = sb.enter_context(tc.tile_pool(name="pQC", bufs=3))
                pRT = sb.enter_context(tc.tile_pool(name="pRT", bufs=2))
                pSend = sb.enter_context(tc.tile_pool(name="pSend", bufs=NT))
                pCos = sb.enter_context(tc.tile_pool(name="pCos", bufs=1))

                cosr = []
                sinr = []
                for par in range(2):
                    ct = pCos.tile([128, 8 * 32], f32, name=f"cosr_{par}")
                    nc.sync.dma_start(out=ct[:], in_=cos_in.ap()[par * 128:(par + 1) * 128, :])
                    st_ = pCos.tile([128, 8 * 32], f32, name=f"sinr_{par}")
                    nc.sync.dma_start(out=st_[:], in_=sin_in.ap()[par * 128:(par + 1) * 128, :])
                    cosr.append(ct)
                    sinr.append(st_)

                sends = [pSend.tile([128, QKV_O], bf16, name=f"sends_{m}", tag="sends")
                         for m in range(NT)]

                qTs = [[None] * 2 for _ in range(B)]
                KBs = [None] * B
                NQC = QKV_O // 512   # q chunks 0-3, k chunk 4, v chunk 5
                for ng in range(NQC):
                    psq = [psA.tile([128, 512], f32, tag="acc", name=f"qkvp_{ng}_{m}")
                           for m in range(NT)]
                    for j in range(NHT):
                        wt = pW.tile([128, 512], bf16, tag="w1")
                        r0 = (ng * NHT + j) * 128
                        nc.sync.dma_start(out=wt[:], in_=WQ1[r0:r0 + 128, :])
                        for m in range(NT):
                            nc.tensor.matmul(psq[m][:], xqT[j][:, m * 128:(m + 1) * 128], wt[:],
                                             start=(j == 0), stop=(j == NHT - 1))
                    for m in range(NT):
                        par = m % 2
                        if ng < 5:
                            qc_t = pQC.tile([128, 512], f32, tag="qc")
                            nc.vector.tensor_scalar(qc_t[:], psq[m][:], d1s[m][:], None, ALU.mult)
                            xv = qc_t[:].rearrange("p (h t d) -> p h t d", t=2, d=32)
                            xr = xv[:, :, 0, :]
                            xi = xv[:, :, 1, :]
                            cv = cosr[par][:].rearrange("p (h d) -> p h d", d=32)
                            sv = sinr[par][:].rearrange("p (h d) -> p h d", d=32)
                            ov = sends[m][:, ng * 512:(ng + 1) * 512].rearrange(
                                "p (h t d) -> p h t d", t=2, d=32)
                            o_r = ov[:, :, 0, :]
                            o_i = ov[:, :, 1, :]
                            ta = pRT.tile([128, 256], f32, tag="ta")
                            tb = pRT.tile([128, 256], f32, tag="tb")
                            tav = ta[:].rearrange("p (h d) -> p h d", d=32)
                            tbv = tb[:].rearrange("p (h d) -> p h d", d=32)
                            nc.vector.tensor_tensor(tav, xr, cv, ALU.mult)
                            nc.vector.tensor_tensor(tbv, xi, sv, ALU.mult)
                            nc.vector.tensor_tensor(o_r, tav, tbv, ALU.subtract)
                            nc.vector.tensor_tensor(tav, xr, sv, ALU.mult)
                            nc.vector.tensor_tensor(tbv, xi, cv, ALU.mult)
                            nc.vector.tensor_tensor(o_i, tav, tbv, ALU.add)
                        else:
                            nc.vector.tensor_scalar(sends[m][:, ng * 512:(ng + 1) * 512],
                                                    psq[m][:], d1s[m][:], None, ALU.mult)

                    if ng == 3:
                        # q chunks complete: scatter + fire q AllToAll (overlaps k/v chunks)
                        for m in range(NT):
                            b = m // 2
                            par = m % 2
                            base = par * 128 * QW + b * 256
                            nc.sync.dma_start(
                                out=_dap(aq_i[:], base,
                                         [[QW, 128], [SC * QW, 8], [1, 256]]),
                                in_=sends[m][:, 0:2048].rearrange("p (j c) -> p j c", j=8))
                        nc.gpsimd.collective_compute(
                            "AllToAll", ALU.bypass, replica_groups=[list(range(C))],
                            ins=[aq_i[:].opt()], outs=[aq_o[:].opt()])
                        for b in range(B):
                            for hp in range(2):
                                t = pQT.tile([128, S], bf16, name=f"qT_{b}_{hp}", tag="qT")
                                nc.sync.dma_start(
                                    out=t[:],
                                    in_=aq_o[:, b * 256 + hp * 128:b * 256 + (hp + 1) * 128],
                                    transpose=True)
                                qTs[b][hp] = t
                    elif ng == 4:
                        # k chunk complete: scatter (with duplicate) + k AllToAll
                        for m in range(NT):
                            b = m // 2
                            par = m % 2
                            base = par * 128 * 256 + b * 128
                            for koff in (0, 64):
                                nc.sync.dma_start(
                                    out=_dap(ak_i[:], base + koff,
                                             [[256, 128], [SC * 256, 8], [1, 64]]),
                                    in_=sends[m][:, 2048:2560].rearrange(
                                        "p (j c) -> p j c", j=8))
                        nc.gpsimd.collective_compute(
                            "AllToAll", ALU.bypass, replica_groups=[list(range(C))],
                            ins=[ak_i[:].opt()], outs=[ak_o[:].opt()])
                        for b in range(B):
                            KB = pKT.tile([128, S], bf16, name=f"KB_{b}", tag="kT")
                            nc.sync.dma_start(out=KB[:],
                                              in_=ak_o[:, b * 128:(b + 1) * 128],
                                              transpose=True)
                            KBs[b] = KB
                    elif ng == 5:
                        # v chunk complete: scatter + v AllToAll
                        for m in range(NT):
                            b = m // 2
                            par = m % 2
                            base = par * 128 * 128 + b * 64
                            nc.sync.dma_start(
                                out=_dap(av_i[:], base,
                                         [[128, 128], [SC * 128, 8], [1, 64]]),
                                in_=sends[m][:, 2560:3072].rearrange("p (j c) -> p j c", j=8))
                        nc.gpsimd.collective_compute(
                            "AllToAll", ALU.bypass, replica_groups=[list(range(C))],
                            ins=[av_i[:].opt()], outs=[av_o[:].opt()])

            # ================= Stage C: attention =========================
            with ExitStack() as sc:
                pVA = sc.enter_context(tc.tile_pool(name="pVA", bufs=2 * NKT))
                pEX = sc.enter_context(tc.tile_pool(name="pEX", bufs=NKT))
                pOB = sc.enter_context(tc.tile_pool(name="pOB", bufs=2 * NKT))
                pR = sc.enter_context(tc.tile_pool(name="pR", bufs=8))

                # v loads for both batches
                vas = []
                for b in range(B):
                    vab = []
                    for kt in range(NKT):
                        t = pVA.tile([128, 65], bf16, name=f"va_{b}_{kt}", tag="va")
                        nc.sync.dma_start(
                            out=t[:, 0:64],
                            in_=av_o[kt * 128:(kt + 1) * 128, b * 64:(b + 1) * 64])
                        nc.vector.memset(t[:, 64:65], 1.0)
                        vab.append(t)
                    vas.append(vab)

                for b in range(B):
                    KB = KBs[b]
                    vab = vas[b]
                    obs = [pOB.tile([128, G * HD], bf16, name=f"ob_{b}_{qt}", tag="ob")
                           for qt in range(NKT)]

                    for hp in range(2):
                        qTx = qTs[b][hp]
                        for qc in range(4):
                            exs = []
                            for kt in range(4 * qc + 4):
                                dpos = max(0, kt * 128 - qc * 512)
                                st = psS.tile([128, 1024], f32, tag="st",
                                              name=f"st_{b}_{hp}_{qc}_{kt}")
                                nc.tensor.matmul(
                                    st[:, dpos:512],
                                    KB[0:64, kt * 128:(kt + 1) * 128],
                                    qTx[0:64, qc * 512 + dpos:(qc + 1) * 512],
                                    start=True, stop=True)
                                nc.tensor.matmul(
                                    st[:, 512 + dpos:1024],
                                    KB[64:128, kt * 128:(kt + 1) * 128],
                                    qTx[64:128, qc * 512 + dpos:(qc + 1) * 512],
                                    start=True, stop=True, tile_position=(64, 0))
                                ex = pEX.tile([128, 1024], bf16, tag="ex",
                                              name=f"ex_{b}_{hp}_{qc}_{kt}")
                                stv = st[:].rearrange("p (h q) -> p h q", h=2)[:, :, dpos:512]
                                exv = ex[:].rearrange("p (h q) -> p h q", h=2)[:, :, dpos:512]
                                nc.scalar.activation(out=exv, in_=stv, func=FT.Exp, scale=0.125)
                                if kt >= 4 * qc:
                                    for h in range(2):
                                        sl = ex[:, h * 512 + dpos:h * 512 + dpos + 128]
                                        nc.vector.tensor_tensor(sl, sl, trim[:], ALU.mult)
                                exs.append(ex)
                            for h in range(2):
                                for qtl in range(4):
                                    qt = 4 * qc + qtl
                                    op = psA.tile([128, 65], f32, tag="acc",
                                                  name=f"op_{b}_{hp}_{qc}_{h}_{qtl}")
                                    for kt in range(qt + 1):
                                        nc.tensor.matmul(
                                            op[:],
                                            exs[kt][:, h * 512 + qtl * 128:h * 512 + (qtl + 1) * 128],
                                            vab[kt][:],
                                            start=(kt == 0), stop=(kt == qt))
                                    r = pR.tile([128, 1], f32, tag="r")
                                    nc.vector.reciprocal(r[:], op[:, 64:65])
                                    hg = hp * 2 + h
                                    nc.vector.tensor_scalar(
                                        obs[qt][:, hg * 64:(hg + 1) * 64],
                                        op[:, 0:64], r[:], None, ALU.mult)
                    for qt in range(NKT):
                        j = qt // 2
                        rowbase = j * SC + (qt % 2) * 128
                        nc.sync.dma_start(out=a2i[b][rowbase:rowbase + 128, :], in_=obs[qt][:])
                    nc.gpsimd.collective_compute(
                        "AllToAll", ALU.bypass, replica_groups=[list(range(C))],
                        ins=[a2i[b][:].opt()], outs=[a2o[b][:].opt()])

            # ================= Stage D: out projection ====================
            with ExitStack() as sd:
                pD = sd.enter_context(tc.tile_pool(name="pD", bufs=2))
                pDs = sd.enter_context(tc.tile_pool(name="pDs", bufs=4))
                pXT2 = sd.enter_context(tc.tile_pool(name="pXT2", bufs=NHT * 2))
                pW2 = sd.enter_context(tc.tile_pool(name="pW2", bufs=4))
                pO = sd.enter_context(tc.tile_pool(name="pO", bufs=3))

                xq2T = [[None] * 2 for _ in range(NHT)]
                for bh in range(2):
                    for j in range(NHT):
                        xq2T[j][bh] = pXT2.tile([128, 256], bf16,
                                                name=f"xq2T_{j}_{bh}", tag="xq2T")
                for m in range(NT):
                    b = m // 2
                    r0 = (m % 2) * 128
                    x2 = pD.tile([128, H], bf16, tag="x2")
                    nc.sync.dma_start(
                        out=x2[:],
                        in_=_dap(a2o[b][:], r0 * 256,
                                 [[256, 128], [SC * 256, 8], [1, 256]]))
                    mx2 = pDs.tile([128, 1], f32, tag="mx2")
                    nc.vector.tensor_reduce(mx2[:], x2[:], mybir.AxisListType.X, ALU.max,
                                            apply_absolute_value=True)
                    mp2 = pDs.tile([128, 1], f32, tag="mp2")
                    nc.vector.tensor_scalar(mp2[:], mx2[:], 1e-5, None, ALU.max)
                    nc.vector.tensor_tensor(d2s[m][:], mp2[:], sw2b[:], ALU.mult)
                    rm2 = pDs.tile([128, 1], f32, tag="rm2")
                    nc.vector.reciprocal(rm2[:], mp2[:])
                    c2 = pDs.tile([128, 1], f32, tag="c2")
                    nc.vector.tensor_scalar(c2[:], rm2[:], 127.0, None, ALU.mult)
                    t2 = pD.tile([128, H], f32, tag="t2")
                    nc.gpsimd.tensor_scalar(t2[:], x2[:], c2[:], MAGIC, ALU.mult, ALU.add)
                    xq2 = pD.tile([128, H], bf16, tag="xq2")
                    nc.vector.tensor_scalar(xq2[:], t2[:], MAGIC, None, ALU.subtract)
                    for j in range(NHT):
                        tp = psS.tile([128, 128], bf16, tag="st", name=f"tp2_{m}_{j}")
                        nc.tensor.transpose(tp[:], xq2[:, j * 128:(j + 1) * 128], ident[:])
                        nc.vector.tensor_copy(
                            xq2T[j][b][:, (m % 2) * 128:(m % 2 + 1) * 128], tp[:])

                for ng in range(H // 512):
                    ps2 = [psA.tile([128, 512], f32, tag="acc", name=f"ps2_{ng}_{m}")
                           for m in range(NT)]
                    for j in range(NHT):
                        wt = pW2.tile([128, 512], bf16, tag="w2")
                        r0 = (ng * NHT + j) * 128
                        nc.sync.dma_start(out=wt[:], in_=WQ2[r0:r0 + 128, :])
                        for m in range(NT):
                            nc.tensor.matmul(
                                ps2[m][:],
                                xq2T[j][m // 2][:, (m % 2) * 128:(m % 2 + 1) * 128],
                                wt[:], start=(j == 0), stop=(j == NHT - 1))
                    for m in range(NT):
                        ot = pO.tile([128, 512], f32, tag="ot")
                        nc.vector.tensor_scalar(ot[:], ps2[m][:], d2s[m][:], None, ALU.mult)
                        nc.sync.dma_start(
                            out=OUT[m * 128:(m + 1) * 128, ng * 512:(ng + 1) * 512], in_=ot[:])

    nc.compile()
    return nc


_NC_CACHE = {}


def _get_nc():
    if "nc" not in _NC_CACHE:
        _NC_CACHE["nc"] = build_nc()
    return _NC_CACHE["nc"]


def _stripe(wt, nchunk):
    """[H, O] -> [(O//512)*16*128, 512] contiguous (ng, j)-stripe layout."""
    Hh, O = wt.shape
    a = wt.reshape(NHT, 128, O // 512, 512)          # [j, h, ng, c]
    a = a.transpose(2, 0, 1, 3)                      # [ng, j, h, c]
    return np.ascontiguousarray(a.reshape(-1, 512))


def kernel(x, w_norm, w_qkv, w_out):
    x = np.asarray(x, dtype=np.float32)
    w_norm = np.asarray(w_norm, dtype=np.float32)
    w_qkv = np.asarray(w_qkv, dtype=np.float32)
    w_out = np.asarray(w_out, dtype=np.float32)

    def tern(w):
        ws = np.float32(1.0) / np.clip(np.mean(np.abs(w)), np.float32(1e-5), None).astype(np.float32)
        wq = np.clip(np.round(w * ws), -1.0, 1.0).astype(np.float32)
        return wq, (np.float32(1.0) / ws).astype(np.float32)

    wq1, s_w1 = tern(w_qkv)
    wq2, s_w2 = tern(w_out)
    wq1t = _stripe(np.ascontiguousarray(wq1.T), QKV_O // 512).astype(ml_dtypes.bfloat16)
    wq2t = _stripe(np.ascontiguousarray(wq2.T), H // 512).astype(ml_dtypes.bfloat16)

    inv_freq = (1.0 / THETA ** (np.arange(0, HD, 2, dtype=np.float32) / HD)).astype(np.float32)
    t_pos = np.arange(S, dtype=np.float32)
    freqs = t_pos[:, None] * inv_freq[None, :]
    cos_full = np.cos(freqs).astype(np.float32)
    sin_full = np.sin(freqs).astype(np.float32)

    trimask = np.triu(np.ones((128, 128), np.float32)).astype(ml_dtypes.bfloat16)
    sw1 = np.array([[s_w1 / np.float32(127.0)]], dtype=np.float32)
    sw2 = np.array([[s_w2 / np.float32(127.0)]], dtype=np.float32)
    wn2d = w_norm.reshape(1, H)

    in_maps = []
    for i in range(C):
        xc = np.ascontiguousarray(
            np.concatenate([x[0, i * SC:(i + 1) * SC, :], x[1, i * SC:(i + 1) * SC, :]], axis=0))
        in_maps.append({
            "x": xc,
            "wn": wn2d,
            "wq1t": wq1t,
            "wq2t": wq2t,
            "cosb": np.ascontiguousarray(np.tile(cos_full[i * SC:(i + 1) * SC, :], (1, 8))),
            "sinb": np.ascontiguousarray(np.tile(sin_full[i * SC:(i + 1) * SC, :], (1, 8))),
            "trimask": trimask,
            "sw1": sw1,
            "sw2": sw2,
        })

    nc = _get_nc()
    res = bass_utils.run_bass_kernel_spmd(nc, in_maps, core_ids=list(range(C)))

    out = np.empty((B, S, H), dtype=np.float32)
    for i in range(C):
        ci = res.results[i]["out"]
        for b in range(B):
            out[b, i * SC:(i + 1) * SC, :] = ci[b * SC:(b + 1) * SC, :]
    return out


# revision 49
# speedup vs baseline: 1.3106x; 1.0462x over previous
"""Distributed Trainium2 Bass kernel for BitNet-style attention block.

Sharding: sequence-parallel projections + (batch x kv-head) parallel attention,
stitched with per-batch AllToAll collectives (split so comm overlaps compute).

Per core (core i):
  A. RMSNorm + per-token absmax quantization of its 512-token chunk.
  B. qkv projection as exact integer bf16 matmul against host-prequantized
     ternary weights, dequant, RoPE on q/k, scatter into per-batch A2A bufs.
  C. AllToAll #1a/#1b -> core i holds full-sequence q/k/v for kv-head i of
     each batch; causal attention (transposed scores, exp on ACT, ones-column
     rowsums, deferred normalization). Batch 1 attention overlaps A2A #2a.
  D. AllToAll #2a/#2b -> core i holds its token chunk of all 32 heads;
     per-token quantization + integer matmul with ternary output weights.
"""
import sys
sys.path.insert(0, "/opt/trn_rl_repo")
import numpy as np
import ml_dtypes
import concourse.bass as bass
import concourse.tile as tile
from concourse import bacc, mybir
from concourse import bass_utils
from concourse.masks import make_identity

f32 = mybir.dt.float32
bf16 = mybir.dt.bfloat16
FT = mybir.ActivationFunctionType
ALU = mybir.AluOpType

B, S, H = 2, 2048, 2048
NH, NKV, HD = 32, 8, 64
G = NH // NKV                    # 4
QKV_O = (NH + 2 * NKV) * HD      # 3072
EPS = 1e-5
THETA = 10000.0
C = 8
SC = S // C                      # 256 positions per core
TOK = B * SC                     # 512 token rows per core
MAGIC = float(1.5 * 2.0 ** 23)   # RNE integer rounding for |v| < 2^22
NT = TOK // 128                  # 4 token tiles
NHT = H // 128                   # 16 h-tiles
NKT = S // 128                   # 16 kj tiles

# a2a1 split into four column-group collectives fired as their data completes:
#   k: per-batch [k 64 | kdup 64] -> [8, 256, 256]
#   v: per-batch [v 64]           -> [8, 256, 128]
#   qA/qB: per-batch head-pair [2 heads = 128] -> [8, 256, 256] each
# q heads are permuted host-side: col hp*1024 + dest*128 + hh*64 + d


def _dap(t_ap, extra, dims):
    return bass.AP(tensor=t_ap.tensor, offset=t_ap.offset + extra, ap=[list(d) for d in dims])


def build_nc():
    nc = bacc.Bacc("TRN2", target_bir_lowering=False, debug=False, num_devices=C)

    x_in = nc.dram_tensor("x", [TOK, H], f32, kind="ExternalInput")
    wn_in = nc.dram_tensor("wn", [1, H], f32, kind="ExternalInput")
    # contiguous stripes: row ((ng*16+j)*128 + h_local), 512 cols each
    wq1t_in = nc.dram_tensor("wq1t", [(QKV_O // 512) * NHT * 128, 512], bf16, kind="ExternalInput")
    wq2t_in = nc.dram_tensor("wq2t", [(H // 512) * NHT * 128, 512], bf16, kind="ExternalInput")
    cos_in = nc.dram_tensor("cosb", [SC, 8 * 32], f32, kind="ExternalInput")
    sin_in = nc.dram_tensor("sinb", [SC, 8 * 32], f32, kind="ExternalInput")
    tri_in = nc.dram_tensor("trimask", [128, 128], bf16, kind="ExternalInput")
    sw1_in = nc.dram_tensor("sw1", [1, 1], f32, kind="ExternalInput")
    sw2_in = nc.dram_tensor("sw2", [1, 1], f32, kind="ExternalInput")
    out_ext = nc.dram_tensor("out", [TOK, H], f32, kind="ExternalOutput")

    X = x_in.ap()
    WQ1 = wq1t_in.ap()
    WQ2 = wq2t_in.ap()
    OUT = out_ext.ap()

    with tile.TileContext(nc) as tc:
        from contextlib import ExitStack
        with ExitStack() as top:
            dram = top.enter_context(tc.tile_pool(name="dram", bufs=1, space="DRAM"))
            const = top.enter_context(tc.tile_pool(name="const", bufs=1))
            smalls = top.enter_context(tc.tile_pool(name="smalls", bufs=1))
            psA = top.enter_context(tc.tile_pool(name="psA", bufs=4, space="PSUM"))
            psS = top.enter_context(tc.tile_pool(name="psS", bufs=2, space="PSUM"))

            # ---------------- DRAM scratch ----------------
            aq_i = [dram.tile([C * SC, 256], bf16, name=f"aq_i{hp}") for hp in range(2)]
            aq_o = [dram.tile([C * SC, 256], bf16, name=f"aq_o{hp}") for hp in range(2)]
            ak_i = dram.tile([C * SC, 256], bf16, name="ak_i")
            ak_o = dram.tile([C * SC, 256], bf16, name="ak_o")
            av_i = dram.tile([C * SC, 128], bf16, name="av_i")
            av_o = dram.tile([C * SC, 128], bf16, name="av_o")
            a2i = [dram.tile([C * SC, G * HD], bf16, name=f"a2i_{b}") for b in range(B)]
            a2o = [dram.tile([C * SC, G * HD], bf16, name=f"a2o_{b}") for b in range(B)]

            # ---------------- constants ----------------
            wnorm_b = const.tile([128, H], f32)
            nc.sync.dma_start(out=wnorm_b[:], in_=_dap(wn_in.ap(), 0, [[0, 128], [1, H]]))
            trim = const.tile([128, 128], bf16)
            nc.sync.dma_start(out=trim[:], in_=tri_in.ap()[:, :])
            sw1b = const.tile([128, 1], f32)
            nc.sync.dma_start(out=sw1b[:], in_=_dap(sw1_in.ap(), 0, [[0, 128], [1, 1]]))
            sw2b = const.tile([128, 1], f32)
            nc.sync.dma_start(out=sw2b[:], in_=_dap(sw2_in.ap(), 0, [[0, 128], [1, 1]]))
            epsb = const.tile([128, 1], f32)
            nc.vector.memset(epsb[:], EPS)
            ident = const.tile([128, 128], bf16)
            make_identity(nc, ident[:])

            d1s = [smalls.tile([128, 1], f32, name=f"d1_{m}") for m in range(NT)]
            d2s = [smalls.tile([128, 1], f32, name=f"d2_{m}") for m in range(NT)]

            xqT_pool = top.enter_context(tc.tile_pool(name="xqT", bufs=NHT))
            pQT = top.enter_context(tc.tile_pool(name="pQT", bufs=4))
            pKT = top.enter_context(tc.tile_pool(name="pKT", bufs=2))

            # ================= Stage A: RMSNorm + quantize =================
            with ExitStack() as sa:
                pA = sa.enter_context(tc.tile_pool(name="pA", bufs=2))
                pXQ = sa.enter_context(tc.tile_pool(name="pXQ", bufs=NT))
                pSc = sa.enter_context(tc.tile_pool(name="pASc", bufs=4))
                xqms = []
                for m in range(NT):
                    xa = pA.tile([128, H], f32, tag="xa")
                    nc.sync.dma_start(out=xa[:], in_=X[m * 128:(m + 1) * 128, :])
                    sq = pA.tile([128, H], f32, tag="sq")
                    ssq = pSc.tile([128, 1], f32, tag="ssq")
                    nc.scalar.activation(out=sq[:], in_=xa[:], func=FT.Square, accum_out=ssq[:])
                    xw = pA.tile([128, H], f32, tag="xw")
                    nc.vector.tensor_tensor(xw[:], xa[:], wnorm_b[:], ALU.mult)
                    std = pSc.tile([128, 1], f32, tag="std")
                    nc.scalar.activation(out=std[:], in_=ssq[:], func=FT.Sqrt,
                                         bias=epsb[:], scale=1.0 / H)
                    rstd = pSc.tile([128, 1], f32, tag="rstd")
                    nc.vector.reciprocal(rstd[:], std[:])
                    mx = pSc.tile([128, 1], f32, tag="mx")
                    nc.vector.tensor_reduce(mx[:], xw[:], mybir.AxisListType.X, ALU.max,
                                            apply_absolute_value=True)
                    mp = pSc.tile([128, 1], f32, tag="mp")
                    nc.vector.tensor_scalar(mp[:], mx[:], rstd[:], 1e-5, ALU.mult, ALU.max)
                    nc.vector.tensor_tensor(d1s[m][:], mp[:], sw1b[:], ALU.mult)
                    rmp = pSc.tile([128, 1], f32, tag="rmp")
                    nc.vector.reciprocal(rmp[:], mp[:])
                    csc = pSc.tile([128, 1], f32, tag="csc")
                    nc.vector.tensor_scalar(csc[:], rmp[:], rstd[:], 127.0, ALU.mult, ALU.mult)
                    t1 = pA.tile([128, H], f32, tag="t1")
                    nc.gpsimd.tensor_scalar(t1[:], xw[:], csc[:], MAGIC, ALU.mult, ALU.add)
                    xqm = pXQ.tile([128, H], bf16, tag="xqm", name=f"xqm_{m}")
                    nc.vector.tensor_scalar(xqm[:], t1[:], MAGIC, None, ALU.subtract)
                    xqms.append(xqm)

                # transposed activations via PE (keeps the DMA queues free)
                xqT = []
                for j in range(NHT):
                    t = xqT_pool.tile([128, TOK], bf16, name=f"xqT_{j}", tag="xqT")
                    xqT.append(t)
                for m in range(NT):
                    for j in range(NHT):
                        tp = psS.tile([128, 128], bf16, tag="st", name=f"tp_{m}_{j}")
                        nc.tensor.transpose(tp[:], xqms[m][:, j * 128:(j + 1) * 128], ident[:])
                        nc.vector.tensor_copy(xqT[j][:, m * 128:(m + 1) * 128], tp[:])

            # ================= Stage B: qkv matmul + RoPE + scatter ========
            with ExitStack() as sb:
                pW = sb.enter_context(tc.tile_pool(name="pW", bufs=4))
                pQC = sb.enter_context(tc.tile_pool(name="pQC", bufs=3))
                pRT = sb.enter_context(tc.tile_pool(name="pRT", bufs=2))
                pSend = sb.enter_context(tc.tile_pool(name="pSend", bufs=NT))
                pCos = sb.enter_context(tc.tile_pool(name="pCos", bufs=1))

                cosr = []
                sinr = []
                for par in range(2):
                    ct = pCos.tile([128, 8 * 32], f32, name=f"cosr_{par}")
                    nc.sync.dma_start(out=ct[:], in_=cos_in.ap()[par * 128:(par + 1) * 128, :])
                    st_ = pCos.tile([128, 8 * 32], f32, name=f"sinr_{par}")
                    nc.sync.dma_start(out=st_[:], in_=sin_in.ap()[par * 128:(par + 1) * 128, :])
                    cosr.append(ct)
                    sinr.append(st_)

                sends = [pSend.tile([128, QKV_O], bf16, name=f"sends_{m}", tag="sends")
                         for m in range(NT)]

                qTs = [[None] * 2 for _ in range(B)]
                KBs = [None] * B
                NQC = QKV_O // 512   # process k, v first, then q (qA = chunks 0,1; qB = 2,3)
                for ng in (4, 5, 0, 1, 2, 3):
                    psq = [psA.tile([128, 512], f32, tag="acc", name=f"qkvp_{ng}_{m}")
                           for m in range(NT)]
                    for j in range(NHT):
                        wt = pW.tile([128, 512], bf16, tag="w1")
                        r0 = (ng * NHT + j) * 128
                        nc.sync.dma_start(out=wt[:], in_=WQ1[r0:r0 + 128, :])
                        for m in range(NT):
                            nc.tensor.matmul(psq[m][:], xqT[j][:, m * 128:(m + 1) * 128], wt[:],
                                             start=(j == 0), stop=(j == NHT - 1))
                    for m in range(NT):
                        par = m % 2
                        if ng < 5:
                            qc_t = pQC.tile([128, 512], f32, tag="qc")
                            nc.scalar.mul(qc_t[:], psq[m][:], d1s[m][:])
                            xv = qc_t[:].rearrange("p (h t d) -> p h t d", t=2, d=32)
                            xr = xv[:, :, 0, :]
                            xi = xv[:, :, 1, :]
                            cv = cosr[par][:].rearrange("p (h d) -> p h d", d=32)
                            sv = sinr[par][:].rearrange("p (h d) -> p h d", d=32)
                            ov = sends[m][:, ng * 512:(ng + 1) * 512].rearrange(
                                "p (h t d) -> p h t d", t=2, d=32)
                            o_r = ov[:, :, 0, :]
                            o_i = ov[:, :, 1, :]
                            ta = pRT.tile([128, 256], f32, tag="ta")
                            tb = pRT.tile([128, 256], f32, tag="tb")
                            tav = ta[:].rearrange("p (h d) -> p h d", d=32)
                            tbv = tb[:].rearrange("p (h d) -> p h d", d=32)
                            nc.vector.tensor_tensor(tav, xr, cv, ALU.mult)
                            nc.vector.tensor_tensor(tbv, xi, sv, ALU.mult)
                            nc.vector.tensor_tensor(o_r, tav, tbv, ALU.subtract)
                            nc.vector.tensor_tensor(tav, xr, sv, ALU.mult)
                            nc.vector.tensor_tensor(tbv, xi, cv, ALU.mult)
                            nc.vector.tensor_tensor(o_i, tav, tbv, ALU.add)
                        else:
                            nc.scalar.mul(sends[m][:, ng * 512:(ng + 1) * 512],
                                          psq[m][:], d1s[m][:])

                    if ng in (1, 3):
                        # a head-pair's q chunks complete: scatter + fire its AllToAll
                        hp = ng // 2
                        for m in range(NT):
                            b = m // 2
                            par = m % 2
                            base = par * 128 * 256 + b * 128
                            nc.sync.dma_start(
                                out=_dap(aq_i[hp][:], base,
                                         [[256, 128], [SC * 256, 8], [1, 128]]),
                                in_=sends[m][:, hp * 1024:(hp + 1) * 1024].rearrange(
                                    "p (j c) -> p j c", j=8))
                        nc.gpsimd.collective_compute(
                            "AllToAll", ALU.bypass, replica_groups=[list(range(C))],
                            ins=[aq_i[hp][:].opt()], outs=[aq_o[hp][:].opt()])
                        for b in range(B):
                            t = pQT.tile([128, S], bf16, name=f"qT_{b}_{hp}", tag="qT")
                            nc.sync.dma_start(
                                out=t[:],
                                in_=aq_o[hp][:, b * 128:(b + 1) * 128],
                                transpose=True)
                            qTs[b][hp] = t
                    elif ng == 4:
                        # k chunk complete: scatter (with duplicate) + k AllToAll
                        for m in range(NT):
                            b = m // 2
                            par = m % 2
                            base = par * 128 * 256 + b * 128
                            for koff in (0, 64):
                                nc.sync.dma_start(
                                    out=_dap(ak_i[:], base + koff,
                                             [[256, 128], [SC * 256, 8], [1, 64]]),
                                    in_=sends[m][:, 2048:2560].rearrange(
                                        "p (j c) -> p j c", j=8))
                        nc.gpsimd.collective_compute(
                            "AllToAll", ALU.bypass, replica_groups=[list(range(C))],
                            ins=[ak_i[:].opt()], outs=[ak_o[:].opt()])
                        for b in range(B):
                            KB = pKT.tile([128, S], bf16, name=f"KB_{b}", tag="kT")
                            nc.sync.dma_start(out=KB[:],
                                              in_=ak_o[:, b * 128:(b + 1) * 128],
                                              transpose=True)
                            KBs[b] = KB
                    elif ng == 5:
                        # v chunk complete: scatter + v AllToAll
                        for m in range(NT):
                            b = m // 2
                            par = m % 2
                            base = par * 128 * 128 + b * 64
                            nc.sync.dma_start(
                                out=_dap(av_i[:], base,
                                         [[128, 128], [SC * 128, 8], [1, 64]]),
                                in_=sends[m][:, 2560:3072].rearrange("p (j c) -> p j c", j=8))
                        nc.gpsimd.collective_compute(
                            "AllToAll", ALU.bypass, replica_groups=[list(range(C))],
                            ins=[av_i[:].opt()], outs=[av_o[:].opt()])

            # ================= Stage C: attention =========================
            with ExitStack() as sc:
                pVA = sc.enter_context(tc.tile_pool(name="pVA", bufs=2 * NKT))
                pEX = sc.enter_context(tc.tile_pool(name="pEX", bufs=NKT))
                pOB = sc.enter_context(tc.tile_pool(name="pOB", bufs=2 * NKT))
                pR = sc.enter_context(tc.tile_pool(name="pR", bufs=8))

                # v loads for both batches
                vas = []
                for b in range(B):
                    vab = []
                    for kt in range(NKT):
                        t = pVA.tile([128, 65], bf16, name=f"va_{b}_{kt}", tag="va")
                        nc.sync.dma_start(
                            out=t[:, 0:64],
                            in_=av_o[kt * 128:(kt + 1) * 128, b * 64:(b + 1) * 64])
                        nc.vector.memset(t[:, 64:65], 1.0)
                        vab.append(t)
                    vas.append(vab)

                obs_all = [[pOB.tile([128, G * HD], bf16, name=f"ob_{b}_{qt}", tag="ob")
                            for qt in range(NKT)] for b in range(B)]
                for hp in range(2):
                    for b in range(B):
                        KB = KBs[b]
                        vab = vas[b]
                        obs = obs_all[b]
                        qTx = qTs[b][hp]
                        for qc in range(4):
                            exs = []
                            for kt in range(4 * qc + 4):
                                dpos = max(0, kt * 128 - qc * 512)
                                st = psS.tile([128, 1024], f32, tag="st",
                                              name=f"st_{b}_{hp}_{qc}_{kt}")
                                nc.tensor.matmul(
                                    st[:, dpos:512],
                                    KB[0:64, kt * 128:(kt + 1) * 128],
                                    qTx[0:64, qc * 512 + dpos:(qc + 1) * 512],
                                    start=True, stop=True)
                                nc.tensor.matmul(
                                    st[:, 512 + dpos:1024],
                                    KB[64:128, kt * 128:(kt + 1) * 128],
                                    qTx[64:128, qc * 512 + dpos:(qc + 1) * 512],
                                    start=True, stop=True, tile_position=(64, 0))
                                ex = pEX.tile([128, 1024], bf16, tag="ex",
                                              name=f"ex_{b}_{hp}_{qc}_{kt}")
                                stv = st[:].rearrange("p (h q) -> p h q", h=2)[:, :, dpos:512]
                                exv = ex[:].rearrange("p (h q) -> p h q", h=2)[:, :, dpos:512]
                                nc.scalar.activation(out=exv, in_=stv, func=FT.Exp, scale=0.125)
                                if kt >= 4 * qc:
                                    for h in range(2):
                                        sl = ex[:, h * 512 + dpos:h * 512 + dpos + 128]
                                        nc.vector.tensor_tensor(sl, sl, trim[:], ALU.mult)
                                exs.append(ex)
                            for h in range(2):
                                for qtl in range(4):
                                    qt = 4 * qc + qtl
                                    op = psA.tile([128, 65], f32, tag="acc",
                                                  name=f"op_{b}_{hp}_{qc}_{h}_{qtl}")
                                    for kt in range(qt + 1):
                                        nc.tensor.matmul(
                                            op[:],
                                            exs[kt][:, h * 512 + qtl * 128:h * 512 + (qtl + 1) * 128],
                                            vab[kt][:],
                                            start=(kt == 0), stop=(kt == qt))
                                    r = pR.tile([128, 1], f32, tag="r")
                                    nc.vector.reciprocal(r[:], op[:, 64:65])
                                    hg = hp * 2 + h
                                    nc.vector.tensor_scalar(
                                        obs[qt][:, hg * 64:(hg + 1) * 64],
                                        op[:, 0:64], r[:], None, ALU.mult)
                        if hp == 1:
                            for qt in range(NKT):
                                j = qt // 2
                                rowbase = j * SC + (qt % 2) * 128
                                nc.sync.dma_start(out=a2i[b][rowbase:rowbase + 128, :],
                                                  in_=obs[qt][:])
                            nc.gpsimd.collective_compute(
                                "AllToAll", ALU.bypass, replica_groups=[list(range(C))],
                                ins=[a2i[b][:].opt()], outs=[a2o[b][:].opt()])

            # ================= Stage D: out projection ====================
            with ExitStack() as sd:
                pD = sd.enter_context(tc.tile_pool(name="pD", bufs=2))
                pDs = sd.enter_context(tc.tile_pool(name="pDs", bufs=4))
                pXT2 = sd.enter_context(tc.tile_pool(name="pXT2", bufs=NHT * 2))
                pW2 = sd.enter_context(tc.tile_pool(name="pW2", bufs=4))
                pO = sd.enter_context(tc.tile_pool(name="pO", bufs=3))

                xq2T = [[None] * 2 for _ in range(NHT)]
                for bh in range(2):
                    for j in range(NHT):
                        xq2T[j][bh] = pXT2.tile([128, 256], bf16,
                                                name=f"xq2T_{j}_{bh}", tag="xq2T")
                for m in range(NT):
                    b = m // 2
                    r0 = (m % 2) * 128
                    x2 = pD.tile([128, H], bf16, tag="x2")
                    nc.sync.dma_start(
                        out=x2[:],
                        in_=_dap(a2o[b][:], r0 * 256,
                                 [[256, 128], [SC * 256, 8], [1, 256]]))
                    mx2 = pDs.tile([128, 1], f32, tag="mx2")
                    nc.vector.tensor_reduce(mx2[:], x2[:], mybir.AxisListType.X, ALU.max,
                                            apply_absolute_value=True)
                    mp2 = pDs.tile([128, 1], f32, tag="mp2")
                    nc.vector.tensor_scalar(mp2[:], mx2[:], 1e-5, None, ALU.max)
                    nc.vector.tensor_tensor(d2s[m][:], mp2[:], sw2b[:], ALU.mult)
                    rm2 = pDs.tile([128, 1], f32, tag="rm2")
                    nc.vector.reciprocal(rm2[:], mp2[:])
                    c2 = pDs.tile([128, 1], f32, tag="c2")
                    nc.vector.tensor_scalar(c2[:], rm2[:], 127.0, None, ALU.mult)
                    t2 = pD.tile([128, H], f32, tag="t2")
                    nc.gpsimd.tensor_scalar(t2[:], x2[:], c2[:], MAGIC, ALU.mult, ALU.add)
                    xq2 = pD.tile([128, H], bf16, tag="xq2")
                    nc.vector.tensor_scalar(xq2[:], t2[:], MAGIC, None, ALU.subtract)
                    for j in range(NHT):
                        tp = psS.tile([128, 128], bf16, tag="st", name=f"tp2_{m}_{j}")
                        nc.tensor.transpose(tp[:], xq2[:, j * 128:(j + 1) * 128], ident[:])
                        nc.vector.tensor_copy(
                            xq2T[j][b][:, (m % 2) * 128:(m % 2 + 1) * 128], tp[:])

                for ng in range(H // 512):
                    ps2 = [psA.tile([128, 512], f32, tag="acc", name=f"ps2_{ng}_{m}")
                           for m in range(NT)]
                    for j in range(NHT):
                        wt = pW2.tile([128, 512], bf16, tag="w2")
                        r0 = (ng * NHT + j) * 128
                        nc.sync.dma_start(out=wt[:], in_=WQ2[r0:r0 + 128, :])
                        for m in range(NT):
                            nc.tensor.matmul(
                                ps2[m][:],
                                xq2T[j][m // 2][:, (m % 2) * 128:(m % 2 + 1) * 128],
                                wt[:], start=(j == 0), stop=(j == NHT - 1))
                    for m in range(NT):
                        ot = pO.tile([128, 512], f32, tag="ot")
                        nc.scalar.mul(ot[:], ps2[m][:], d2s[m][:])
                        nc.sync.dma_start(
                            out=OUT[m * 128:(m + 1) * 128, ng * 512:(ng + 1) * 512], in_=ot[:])

    nc.compile()
    return nc


_NC_CACHE = {}


def _get_nc():
    if "nc" not in _NC_CACHE:
        _NC_CACHE["nc"] = build_nc()
    return _NC_CACHE["nc"]


def _stripe(wt, nchunk):
    """[H, O] -> [(O//512)*16*128, 512] contiguous (ng, j)-stripe layout."""
    Hh, O = wt.shape
    a = wt.reshape(NHT, 128, O // 512, 512)          # [j, h, ng, c]
    a = a.transpose(2, 0, 1, 3)                      # [ng, j, h, c]
    return np.ascontiguousarray(a.reshape(-1, 512))


def kernel(x, w_norm, w_qkv, w_out):
    x = np.asarray(x, dtype=np.float32)
    w_norm = np.asarray(w_norm, dtype=np.float32)
    w_qkv = np.asarray(w_qkv, dtype=np.float32)
    w_out = np.asarray(w_out, dtype=np.float32)

    def tern(w):
        ws = np.float32(1.0) / np.clip(np.mean(np.abs(w)), np.float32(1e-5), None).astype(np.float32)
        wq = np.clip(np.round(w * ws), -1.0, 1.0).astype(np.float32)
        return wq, (np.float32(1.0) / ws).astype(np.float32)

    wq1, s_w1 = tern(w_qkv)
    wq2, s_w2 = tern(w_out)
    # permute q head blocks: new col hp*1024 + dest*128 + (h%2)*64 + d
    hperm = np.empty(NH, np.int64)
    for h in range(NH):
        hperm[(h % 4) // 2 * 16 + (h // 4) * 2 + (h % 2)] = h
    qperm = (hperm[:, None] * HD + np.arange(HD)[None, :]).reshape(-1)
    wq1p = wq1.copy()
    wq1p[:NH * HD] = wq1[qperm]
    wq1t = _stripe(np.ascontiguousarray(wq1p.T), QKV_O // 512).astype(ml_dtypes.bfloat16)
    wq2t = _stripe(np.ascontiguousarray(wq2.T), H // 512).astype(ml_dtypes.bfloat16)

    inv_freq = (1.0 / THETA ** (np.arange(0, HD, 2, dtype=np.float32) / HD)).astype(np.float32)
    t_pos = np.arange(S, dtype=np.float32)
    freqs = t_pos[:, None] * inv_freq[None, :]
    cos_full = np.cos(freqs).astype(np.float32)
    sin_full = np.sin(freqs).astype(np.float32)

    trimask = np.triu(np.ones((128, 128), np.float32)).astype(ml_dtypes.bfloat16)
    sw1 = np.array([[s_w1 / np.float32(127.0)]], dtype=np.float32)
    sw2 = np.array([[s_w2 / np.float32(127.0)]], dtype=np.float32)
    wn2d = w_norm.reshape(1, H)

    in_maps = []
    for i in range(C):
        xc = np.ascontiguousarray(
            np.concatenate([x[0, i * SC:(i + 1) * SC, :], x[1, i * SC:(i + 1) * SC, :]], axis=0))
        in_maps.append({
            "x": xc,
            "wn": wn2d,
            "wq1t": wq1t,
            "wq2t": wq2t,
            "cosb": np.ascontiguousarray(np.tile(cos_full[i * SC:(i + 1) * SC, :], (1, 8))),
            "sinb": np.ascontiguousarray(np.tile(sin_full[i * SC:(i + 1) * SC, :], (1, 8))),
            "trimask": trimask,
            "sw1": sw1,
            "sw2": sw2,
        })

    nc = _get_nc()
    res = bass_utils.run_bass_kernel_spmd(nc, in_maps, core_ids=list(range(C)))

    out = np.empty((B, S, H), dtype=np.float32)
    for i in range(C):
        ci = res.results[i]["out"]
        for b in range(B):
            out[b, i * SC:(i + 1) * SC, :] = ci[b * SC:(b + 1) * SC, :]
    return out


# revision 54
# speedup vs baseline: 1.3681x; 1.0438x over previous
"""Distributed Trainium2 Bass kernel for BitNet-style attention block.

Sharding: sequence-parallel projections + (batch x kv-head) parallel attention,
stitched with per-batch AllToAll collectives (split so comm overlaps compute).

Per core (core i):
  A. RMSNorm + per-token absmax quantization of its 512-token chunk.
  B. qkv projection as exact integer bf16 matmul against host-prequantized
     ternary weights, dequant, RoPE on q/k, scatter into per-batch A2A bufs.
  C. AllToAll #1a/#1b -> core i holds full-sequence q/k/v for kv-head i of
     each batch; causal attention (transposed scores, exp on ACT, ones-column
     rowsums, deferred normalization). Batch 1 attention overlaps A2A #2a.
  D. AllToAll #2a/#2b -> core i holds its token chunk of all 32 heads;
     per-token quantization + integer matmul with ternary output weights.
"""
import sys
sys.path.insert(0, "/opt/trn_rl_repo")
import numpy as np
import ml_dtypes
import concourse.bass as bass
import concourse.tile as tile
from concourse import bacc, mybir
from concourse import bass_utils
from concourse.masks import make_identity

f32 = mybir.dt.float32
bf16 = mybir.dt.bfloat16
FT = mybir.ActivationFunctionType
ALU = mybir.AluOpType

B, S, H = 2, 2048, 2048
NH, NKV, HD = 32, 8, 64
G = NH // NKV                    # 4
QKV_O = (NH + 2 * NKV) * HD      # 3072
EPS = 1e-5
THETA = 10000.0
C = 8
SC = S // C                      # 256 positions per core
TOK = B * SC                     # 512 token rows per core
MAGIC = float(1.5 * 2.0 ** 23)   # RNE integer rounding for |v| < 2^22
NT = TOK // 128                  # 4 token tiles
NHT = H // 128                   # 16 h-tiles
NKT = S // 128                   # 16 kj tiles

# a2a1 split into four column-group collectives fired as their data completes:
#   k: per-batch [k 64 | kdup 64] -> [8, 256, 256]
#   v: per-batch [v 64]           -> [8, 256, 128]
#   qA/qB: per-batch head-pair [2 heads = 128] -> [8, 256, 256] each
# q heads are permuted host-side: col hp*1024 + dest*128 + hh*64 + d


def _dap(t_ap, extra, dims):
    return bass.AP(tensor=t_ap.tensor, offset=t_ap.offset + extra, ap=[list(d) for d in dims])


def build_nc():
    nc = bacc.Bacc("TRN2", target_bir_lowering=False, debug=False, num_devices=C)

    x_in = nc.dram_tensor("x", [TOK, H], f32, kind="ExternalInput")
    wn_in = nc.dram_tensor("wn", [1, H], f32, kind="ExternalInput")
    # contiguous stripes: row ((ng*16+j)*128 + h_local), 512 cols each
    wq1t_in = nc.dram_tensor("wq1t", [(QKV_O // 512) * NHT * 128, 512], bf16, kind="ExternalInput")
    wq2t_in = nc.dram_tensor("wq2t", [(H // 512) * NHT * 128, 512], bf16, kind="ExternalInput")
    cos_in = nc.dram_tensor("cosb", [SC, 8 * 32], f32, kind="ExternalInput")
    sin_in = nc.dram_tensor("sinb", [SC, 8 * 32], f32, kind="ExternalInput")
    tri_in = nc.dram_tensor("trimask", [128, 128], bf16, kind="ExternalInput")
    sw1_in = nc.dram_tensor("sw1", [1, 1], f32, kind="ExternalInput")
    sw2_in = nc.dram_tensor("sw2", [1, 1], f32, kind="ExternalInput")
    out_ext = nc.dram_tensor("out", [TOK, H], f32, kind="ExternalOutput")

    X = x_in.ap()
    WQ1 = wq1t_in.ap()
    WQ2 = wq2t_in.ap()
    OUT = out_ext.ap()

    with tile.TileContext(nc) as tc:
        from contextlib import ExitStack
        with ExitStack() as top:
            dram = top.enter_context(tc.tile_pool(name="dram", bufs=1, space="DRAM"))
            const = top.enter_context(tc.tile_pool(name="const", bufs=1))
            smalls = top.enter_context(tc.tile_pool(name="smalls", bufs=1))
            psA = top.enter_context(tc.tile_pool(name="psA", bufs=4, space="PSUM"))
            psS = top.enter_context(tc.tile_pool(name="psS", bufs=2, space="PSUM"))

            # ---------------- DRAM scratch ----------------
            aq_i = [dram.tile([C * SC, 256], bf16, name=f"aq_i{hp}") for hp in range(2)]
            aq_o = [dram.tile([C * SC, 256], bf16, name=f"aq_o{hp}") for hp in range(2)]
            ak_i = dram.tile([C * SC, 256], bf16, name="ak_i")
            ak_o = dram.tile([C * SC, 256], bf16, name="ak_o")
            av_i = dram.tile([C * SC, 128], bf16, name="av_i")
            av_o = dram.tile([C * SC, 128], bf16, name="av_o")
            a2i = [dram.tile([C * SC, G * HD], bf16, name=f"a2i_{b}") for b in range(B)]
            a2o = [dram.tile([C * SC, G * HD], bf16, name=f"a2o_{b}") for b in range(B)]

            # ---------------- constants ----------------
            wnorm_b = const.tile([128, H], f32)
            nc.sync.dma_start(out=wnorm_b[:], in_=_dap(wn_in.ap(), 0, [[0, 128], [1, H]]))
            trim = const.tile([128, 128], bf16)
            nc.sync.dma_start(out=trim[:], in_=tri_in.ap()[:, :])
            sw1b = const.tile([128, 1], f32)
            nc.sync.dma_start(out=sw1b[:], in_=_dap(sw1_in.ap(), 0, [[0, 128], [1, 1]]))
            sw2b = const.tile([128, 1], f32)
            nc.sync.dma_start(out=sw2b[:], in_=_dap(sw2_in.ap(), 0, [[0, 128], [1, 1]]))
            epsb = const.tile([128, 1], f32)
            nc.vector.memset(epsb[:], EPS)
            ident = const.tile([128, 128], bf16)
            make_identity(nc, ident[:])

            d1s = [smalls.tile([128, 1], f32, name=f"d1_{m}") for m in range(NT)]
            d2s = [smalls.tile([128, 1], f32, name=f"d2_{m}") for m in range(NT)]

            xqT_pool = top.enter_context(tc.tile_pool(name="xqT", bufs=NHT))
            pQT = top.enter_context(tc.tile_pool(name="pQT", bufs=4))
            pKT = top.enter_context(tc.tile_pool(name="pKT", bufs=2))

            # ================= Stage A: RMSNorm + quantize =================
            with ExitStack() as sa:
                pA = sa.enter_context(tc.tile_pool(name="pA", bufs=2))
                pXQ = sa.enter_context(tc.tile_pool(name="pXQ", bufs=NT))
                pSc = sa.enter_context(tc.tile_pool(name="pASc", bufs=4))
                xqms = []
                for m in range(NT):
                    xa = pA.tile([128, H], f32, tag="xa")
                    nc.sync.dma_start(out=xa[:], in_=X[m * 128:(m + 1) * 128, :])
                    sq = pA.tile([128, H], f32, tag="sq")
                    ssq = pSc.tile([128, 1], f32, tag="ssq")
                    nc.scalar.activation(out=sq[:], in_=xa[:], func=FT.Square, accum_out=ssq[:])
                    xw = pA.tile([128, H], f32, tag="xw")
                    nc.vector.tensor_tensor(xw[:], xa[:], wnorm_b[:], ALU.mult)
                    std = pSc.tile([128, 1], f32, tag="std")
                    nc.scalar.activation(out=std[:], in_=ssq[:], func=FT.Sqrt,
                                         bias=epsb[:], scale=1.0 / H)
                    rstd = pSc.tile([128, 1], f32, tag="rstd")
                    nc.vector.reciprocal(rstd[:], std[:])
                    mx = pSc.tile([128, 1], f32, tag="mx")
                    nc.vector.tensor_reduce(mx[:], xw[:], mybir.AxisListType.X, ALU.max,
                                            apply_absolute_value=True)
                    mp = pSc.tile([128, 1], f32, tag="mp")
                    nc.vector.tensor_scalar(mp[:], mx[:], rstd[:], 1e-5, ALU.mult, ALU.max)
                    nc.vector.tensor_tensor(d1s[m][:], mp[:], sw1b[:], ALU.mult)
                    rmp = pSc.tile([128, 1], f32, tag="rmp")
                    nc.vector.reciprocal(rmp[:], mp[:])
                    csc = pSc.tile([128, 1], f32, tag="csc")
                    nc.vector.tensor_scalar(csc[:], rmp[:], rstd[:], 127.0, ALU.mult, ALU.mult)
                    t1 = pA.tile([128, H], f32, tag="t1")
                    nc.gpsimd.tensor_scalar(t1[:], xw[:], csc[:], MAGIC, ALU.mult, ALU.add)
                    xqm = pXQ.tile([128, H], bf16, tag="xqm", name=f"xqm_{m}")
                    nc.vector.tensor_scalar(xqm[:], t1[:], MAGIC, None, ALU.subtract)
                    xqms.append(xqm)

                # transposed activations via PE (keeps the DMA queues free)
                xqT = []
                for j in range(NHT):
                    t = xqT_pool.tile([128, TOK], bf16, name=f"xqT_{j}", tag="xqT")
                    xqT.append(t)
                for m in range(NT):
                    for j in range(NHT):
                        tp = psS.tile([128, 128], bf16, tag="st", name=f"tp_{m}_{j}")
                        nc.tensor.transpose(tp[:], xqms[m][:, j * 128:(j + 1) * 128], ident[:])
                        nc.vector.tensor_copy(xqT[j][:, m * 128:(m + 1) * 128], tp[:])

            # ================= Stage B: qkv matmul + RoPE + scatter ========
            with ExitStack() as sb:
                pW = sb.enter_context(tc.tile_pool(name="pW", bufs=12))
                pQC = sb.enter_context(tc.tile_pool(name="pQC", bufs=3))
                pRT = sb.enter_context(tc.tile_pool(name="pRT", bufs=2))
                pSend = sb.enter_context(tc.tile_pool(name="pSend", bufs=NT))
                pCos = sb.enter_context(tc.tile_pool(name="pCos", bufs=1))

                cosr = []
                sinr = []
                for par in range(2):
                    ct = pCos.tile([128, 8 * 32], f32, name=f"cosr_{par}")
                    nc.sync.dma_start(out=ct[:], in_=cos_in.ap()[par * 128:(par + 1) * 128, :])
                    st_ = pCos.tile([128, 8 * 32], f32, name=f"sinr_{par}")
                    nc.sync.dma_start(out=st_[:], in_=sin_in.ap()[par * 128:(par + 1) * 128, :])
                    cosr.append(ct)
                    sinr.append(st_)

                sends = [pSend.tile([128, QKV_O], bf16, name=f"sends_{m}", tag="sends")
                         for m in range(NT)]

                qTs = [[None] * 2 for _ in range(B)]
                KBs = [None] * B
                NQC = QKV_O // 512   # process k, v first, then q (qA = chunks 0,1; qB = 2,3)
                for ng in (4, 5, 0, 1, 2, 3):
                    psq = [psA.tile([128, 512], f32, tag="acc", name=f"qkvp_{ng}_{m}")
                           for m in range(NT)]
                    for j in range(NHT):
                        wt = pW.tile([128, 512], bf16, tag="w1")
                        r0 = (ng * NHT + j) * 128
                        nc.sync.dma_start(out=wt[:], in_=WQ1[r0:r0 + 128, :])
                        for m in range(NT):
                            nc.tensor.matmul(psq[m][:], xqT[j][:, m * 128:(m + 1) * 128], wt[:],
                                             start=(j == 0), stop=(j == NHT - 1))
                    for m in range(NT):
                        par = m % 2
                        if ng < 5:
                            qc_t = pQC.tile([128, 512], f32, tag="qc")
                            nc.scalar.mul(qc_t[:], psq[m][:], d1s[m][:])
                            xv = qc_t[:].rearrange("p (h t d) -> p h t d", t=2, d=32)
                            xr = xv[:, :, 0, :]
                            xi = xv[:, :, 1, :]
                            cv = cosr[par][:].rearrange("p (h d) -> p h d", d=32)
                            sv = sinr[par][:].rearrange("p (h d) -> p h d", d=32)
                            ov = sends[m][:, ng * 512:(ng + 1) * 512].rearrange(
                                "p (h t d) -> p h t d", t=2, d=32)
                            o_r = ov[:, :, 0, :]
                            o_i = ov[:, :, 1, :]
                            ta = pRT.tile([128, 256], f32, tag="ta")
                            tb = pRT.tile([128, 256], f32, tag="tb")
                            tav = ta[:].rearrange("p (h d) -> p h d", d=32)
                            tbv = tb[:].rearrange("p (h d) -> p h d", d=32)
                            tc_ = pRT.tile([128, 256], f32, tag="tc")
                            td = pRT.tile([128, 256], f32, tag="td")
                            tcv = tc_[:].rearrange("p (h d) -> p h d", d=32)
                            tdv = td[:].rearrange("p (h d) -> p h d", d=32)
                            nc.gpsimd.tensor_tensor(tav, xr, cv, ALU.mult)
                            nc.gpsimd.tensor_tensor(tbv, xi, sv, ALU.mult)
                            nc.vector.tensor_tensor(o_r, tav, tbv, ALU.subtract)
                            nc.gpsimd.tensor_tensor(tcv, xr, sv, ALU.mult)
                            nc.gpsimd.tensor_tensor(tdv, xi, cv, ALU.mult)
                            nc.vector.tensor_tensor(o_i, tcv, tdv, ALU.add)
                        else:
                            nc.scalar.mul(sends[m][:, ng * 512:(ng + 1) * 512],
                                          psq[m][:], d1s[m][:])

                    if ng in (1, 3):
                        # a head-pair's q chunks complete: scatter + fire its AllToAll
                        hp = ng // 2
                        for m in range(NT):
                            b = m // 2
                            par = m % 2
                            base = par * 128 * 256 + b * 128
                            nc.gpsimd.dma_start(
                                out=_dap(aq_i[hp][:], base,
                                         [[256, 128], [SC * 256, 8], [1, 128]]),
                                in_=sends[m][:, hp * 1024:(hp + 1) * 1024].rearrange(
                                    "p (j c) -> p j c", j=8))
                        nc.gpsimd.collective_compute(
                            "AllToAll", ALU.bypass, replica_groups=[list(range(C))],
                            ins=[aq_i[hp][:].opt()], outs=[aq_o[hp][:].opt()])
                        for b in range(B):
                            t = pQT.tile([128, S], bf16, name=f"qT_{b}_{hp}", tag="qT")
                            nc.sync.dma_start(
                                out=t[:],
                                in_=aq_o[hp][:, b * 128:(b + 1) * 128],
                                transpose=True)
                            qTs[b][hp] = t
                    elif ng == 4:
                        # k chunk complete: scatter (with duplicate) + k AllToAll
                        for m in range(NT):
                            b = m // 2
                            par = m % 2
                            base = par * 128 * 256 + b * 128
                            for koff in (0, 64):
                                nc.gpsimd.dma_start(
                                    out=_dap(ak_i[:], base + koff,
                                             [[256, 128], [SC * 256, 8], [1, 64]]),
                                    in_=sends[m][:, 2048:2560].rearrange(
                                        "p (j c) -> p j c", j=8))
                        nc.gpsimd.collective_compute(
                            "AllToAll", ALU.bypass, replica_groups=[list(range(C))],
                            ins=[ak_i[:].opt()], outs=[ak_o[:].opt()])
                        for b in range(B):
                            KB = pKT.tile([128, S], bf16, name=f"KB_{b}", tag="kT")
                            nc.sync.dma_start(out=KB[:],
                                              in_=ak_o[:, b * 128:(b + 1) * 128],
                                              transpose=True)
                            KBs[b] = KB
                    elif ng == 5:
                        # v chunk complete: scatter + v AllToAll
                        for m in range(NT):
                            b = m // 2
                            par = m % 2
                            base = par * 128 * 128 + b * 64
                            nc.gpsimd.dma_start(
                                out=_dap(av_i[:], base,
                                         [[128, 128], [SC * 128, 8], [1, 64]]),
                                in_=sends[m][:, 2560:3072].rearrange("p (j c) -> p j c", j=8))
                        nc.gpsimd.collective_compute(
                            "AllToAll", ALU.bypass, replica_groups=[list(range(C))],
                            ins=[av_i[:].opt()], outs=[av_o[:].opt()])

            # ================= Stage C: attention =========================
            with ExitStack() as sc:
                pVA = sc.enter_context(tc.tile_pool(name="pVA", bufs=2 * NKT))
                pEX = sc.enter_context(tc.tile_pool(name="pEX", bufs=NKT))
                pOB = sc.enter_context(tc.tile_pool(name="pOB", bufs=2 * NKT))
                pR = sc.enter_context(tc.tile_pool(name="pR", bufs=8))

                # v loads for both batches
                vas = []
                for b in range(B):
                    vab = []
                    for kt in range(NKT):
                        t = pVA.tile([128, 65], bf16, name=f"va_{b}_{kt}", tag="va")
                        nc.sync.dma_start(
                            out=t[:, 0:64],
                            in_=av_o[kt * 128:(kt + 1) * 128, b * 64:(b + 1) * 64])
                        nc.vector.memset(t[:, 64:65], 1.0)
                        vab.append(t)
                    vas.append(vab)

                obs_all = [[pOB.tile([128, G * HD], bf16, name=f"ob_{b}_{qt}", tag="ob")
                            for qt in range(NKT)] for b in range(B)]
                for hp in range(2):
                    for b in range(B):
                        KB = KBs[b]
                        vab = vas[b]
                        obs = obs_all[b]
                        qTx = qTs[b][hp]
                        for qc in range(4):
                            exs = []
                            for kt in range(4 * qc + 4):
                                dpos = max(0, kt * 128 - qc * 512)
                                st = psS.tile([128, 1024], f32, tag="st",
                                              name=f"st_{b}_{hp}_{qc}_{kt}")
                                nc.tensor.matmul(
                                    st[:, dpos:512],
                                    KB[0:64, kt * 128:(kt + 1) * 128],
                                    qTx[0:64, qc * 512 + dpos:(qc + 1) * 512],
                                    start=True, stop=True)
                                nc.tensor.matmul(
                                    st[:, 512 + dpos:1024],
                                    KB[64:128, kt * 128:(kt + 1) * 128],
                                    qTx[64:128, qc * 512 + dpos:(qc + 1) * 512],
                                    start=True, stop=True, tile_position=(64, 0))
                                ex = pEX.tile([128, 1024], bf16, tag="ex",
                                              name=f"ex_{b}_{hp}_{qc}_{kt}")
                                stv = st[:].rearrange("p (h q) -> p h q", h=2)[:, :, dpos:512]
                                exv = ex[:].rearrange("p (h q) -> p h q", h=2)[:, :, dpos:512]
                                nc.scalar.activation(out=exv, in_=stv, func=FT.Exp, scale=0.125)
                                if kt >= 4 * qc:
                                    for h in range(2):
                                        sl = ex[:, h * 512 + dpos:h * 512 + dpos + 128]
                                        nc.vector.tensor_tensor(sl, sl, trim[:], ALU.mult)
                                exs.append(ex)
                            for h in range(2):
                                for qtl in range(4):
                                    qt = 4 * qc + qtl
                                    op = psA.tile([128, 65], f32, tag="acc",
                                                  name=f"op_{b}_{hp}_{qc}_{h}_{qtl}")
                                    for kt in range(qt + 1):
                                        nc.tensor.matmul(
                                            op[:],
                                            exs[kt][:, h * 512 + qtl * 128:h * 512 + (qtl + 1) * 128],
                                            vab[kt][:],
                                            start=(kt == 0), stop=(kt == qt))
                                    r = pR.tile([128, 1], f32, tag="r")
                                    nc.vector.reciprocal(r[:], op[:, 64:65])
                                    hg = hp * 2 + h
                                    nc.vector.tensor_scalar(
                                        obs[qt][:, hg * 64:(hg + 1) * 64],
                                        op[:, 0:64], r[:], None, ALU.mult)
                        if hp == 1:
                            for qt in range(NKT):
                                j = qt // 2
                                rowbase = j * SC + (qt % 2) * 128
                                nc.gpsimd.dma_start(out=a2i[b][rowbase:rowbase + 128, :],
                                                  in_=obs[qt][:])
                            nc.gpsimd.collective_compute(
                                "AllToAll", ALU.bypass, replica_groups=[list(range(C))],
                                ins=[a2i[b][:].opt()], outs=[a2o[b][:].opt()])

            # ================= Stage D: out projection ====================
            with ExitStack() as sd:
                pD = sd.enter_context(tc.tile_pool(name="pD", bufs=2))
                pDs = sd.enter_context(tc.tile_pool(name="pDs", bufs=4))
                pXT2 = sd.enter_context(tc.tile_pool(name="pXT2", bufs=NHT * 2))
                pW2 = sd.enter_context(tc.tile_pool(name="pW2", bufs=12))
                pO = sd.enter_context(tc.tile_pool(name="pO", bufs=3))

                xq2T = [[None] * 2 for _ in range(NHT)]
                for bh in range(2):
                    for j in range(NHT):
                        xq2T[j][bh] = pXT2.tile([128, 256], bf16,
                                                name=f"xq2T_{j}_{bh}", tag="xq2T")
                for m in range(NT):
                    b = m // 2
                    r0 = (m % 2) * 128
                    x2 = pD.tile([128, H], bf16, tag="x2")
                    nc.sync.dma_start(
                        out=x2[:],
                        in_=_dap(a2o[b][:], r0 * 256,
                                 [[256, 128], [SC * 256, 8], [1, 256]]))
                    mx2 = pDs.tile([128, 1], f32, tag="mx2")
                    nc.vector.tensor_reduce(mx2[:], x2[:], mybir.AxisListType.X, ALU.max,
                                            apply_absolute_value=True)
                    mp2 = pDs.tile([128, 1], f32, tag="mp2")
                    nc.vector.tensor_scalar(mp2[:], mx2[:], 1e-5, None, ALU.max)
                    nc.vector.tensor_tensor(d2s[m][:], mp2[:], sw2b[:], ALU.mult)
                    rm2 = pDs.tile([128, 1], f32, tag="rm2")
                    nc.vector.reciprocal(rm2[:], mp2[:])
                    c2 = pDs.tile([128, 1], f32, tag="c2")
                    nc.vector.tensor_scalar(c2[:], rm2[:], 127.0, None, ALU.mult)
                    t2 = pD.tile([128, H], f32, tag="t2")
                    nc.gpsimd.tensor_scalar(t2[:], x2[:], c2[:], MAGIC, ALU.mult, ALU.add)
                    xq2 = pD.tile([128, H], bf16, tag="xq2")
                    nc.vector.tensor_scalar(xq2[:], t2[:], MAGIC, None, ALU.subtract)
                    for j in range(NHT):
                        tp = psS.tile([128, 128], bf16, tag="st", name=f"tp2_{m}_{j}")
                        nc.tensor.transpose(tp[:], xq2[:, j * 128:(j + 1) * 128], ident[:])
                        nc.vector.tensor_copy(
                            xq2T[j][b][:, (m % 2) * 128:(m % 2 + 1) * 128], tp[:])

                for ng in range(H // 512):
                    ps2 = [psA.tile([128, 512], f32, tag="acc", name=f"ps2_{ng}_{m}")
                           for m in range(NT)]
                    for j in range(NHT):
                        wt = pW2.tile([128, 512], bf16, tag="w2")
                        r0 = (ng * NHT + j) * 128
                        nc.sync.dma_start(out=wt[:], in_=WQ2[r0:r0 + 128, :])
                        for m in range(NT):
                            nc.tensor.matmul(
                                ps2[m][:],
                                xq2T[j][m // 2][:, (m % 2) * 128:(m % 2 + 1) * 128],
                                wt[:], start=(j == 0), stop=(j == NHT - 1))
                    for m in range(NT):
                        ot = pO.tile([128, 512], f32, tag="ot")
                        nc.scalar.mul(ot[:], ps2[m][:], d2s[m][:])
                        nc.sync.dma_start(
                            out=OUT[m * 128:(m + 1) * 128, ng * 512:(ng + 1) * 512], in_=ot[:])

    nc.compile()
    return nc


_NC_CACHE = {}


def _get_nc():
    if "nc" not in _NC_CACHE:
        _NC_CACHE["nc"] = build_nc()
    return _NC_CACHE["nc"]


def _stripe(wt, nchunk):
    """[H, O] -> [(O//512)*16*128, 512] contiguous (ng, j)-stripe layout."""
    Hh, O = wt.shape
    a = wt.reshape(NHT, 128, O // 512, 512)          # [j, h, ng, c]
    a = a.transpose(2, 0, 1, 3)                      # [ng, j, h, c]
    return np.ascontiguousarray(a.reshape(-1, 512))


def kernel(x, w_norm, w_qkv, w_out):
    x = np.asarray(x, dtype=np.float32)
    w_norm = np.asarray(w_norm, dtype=np.float32)
    w_qkv = np.asarray(w_qkv, dtype=np.float32)
    w_out = np.asarray(w_out, dtype=np.float32)

    def tern(w):
        ws = np.float32(1.0) / np.clip(np.mean(np.abs(w)), np.float32(1e-5), None).astype(np.float32)
        wq = np.clip(np.round(w * ws), -1.0, 1.0).astype(np.float32)
        return wq, (np.float32(1.0) / ws).astype(np.float32)

    wq1, s_w1 = tern(w_qkv)
    wq2, s_w2 = tern(w_out)
    # permute q head blocks: new col hp*1024 + dest*128 + (h%2)*64 + d
    hperm = np.empty(NH, np.int64)
    for h in range(NH):
        hperm[(h % 4) // 2 * 16 + (h // 4) * 2 + (h % 2)] = h
    qperm = (hperm[:, None] * HD + np.arange(HD)[None, :]).reshape(-1)
    wq1p = wq1.copy()
    wq1p[:NH * HD] = wq1[qperm]
    wq1t = _stripe(np.ascontiguousarray(wq1p.T), QKV_O // 512).astype(ml_dtypes.bfloat16)
    wq2t = _stripe(np.ascontiguousarray(wq2.T), H // 512).astype(ml_dtypes.bfloat16)

    inv_freq = (1.0 / THETA ** (np.arange(0, HD, 2, dtype=np.float32) / HD)).astype(np.float32)
    t_pos = np.arange(S, dtype=np.float32)
    freqs = t_pos[:, None] * inv_freq[None, :]
    cos_full = np.cos(freqs).astype(np.float32)
    sin_full = np.sin(freqs).astype(np.float32)

    trimask = np.triu(np.ones((128, 128), np.float32)).astype(ml_dtypes.bfloat16)
    sw1 = np.array([[s_w1 / np.float32(127.0)]], dtype=np.float32)
    sw2 = np.array([[s_w2 / np.float32(127.0)]], dtype=np.float32)
    wn2d = w_norm.reshape(1, H)

    in_maps = []
    for i in range(C):
        xc = np.ascontiguousarray(
            np.concatenate([x[0, i * SC:(i + 1) * SC, :], x[1, i * SC:(i + 1) * SC, :]], axis=0))
        in_maps.append({
            "x": xc,
            "wn": wn2d,
            "wq1t": wq1t,
            "wq2t": wq2t,
            "cosb": np.ascontiguousarray(np.tile(cos_full[i * SC:(i + 1) * SC, :], (1, 8))),
            "sinb": np.ascontiguousarray(np.tile(sin_full[i * SC:(i + 1) * SC, :], (1, 8))),
            "trimask": trimask,
            "sw1": sw1,
            "sw2": sw2,
        })

    nc = _get_nc()
    res = bass_utils.run_bass_kernel_spmd(nc, in_maps, core_ids=list(range(C)))

    out = np.empty((B, S, H), dtype=np.float32)
    for i in range(C):
        ci = res.results[i]["out"]
        for b in range(B):
            out[b, i * SC:(i + 1) * SC, :] = ci[b * SC:(b + 1) * SC, :]
    return out


# revision 55
# speedup vs baseline: 1.4060x; 1.0278x over previous
"""Distributed Trainium2 Bass kernel for BitNet-style attention block.

Sharding: sequence-parallel projections + (batch x kv-head) parallel attention,
stitched with per-batch AllToAll collectives (split so comm overlaps compute).

Per core (core i):
  A. RMSNorm + per-token absmax quantization of its 512-token chunk.
  B. qkv projection as exact integer bf16 matmul against host-prequantized
     ternary weights, dequant, RoPE on q/k, scatter into per-batch A2A bufs.
  C. AllToAll #1a/#1b -> core i holds full-sequence q/k/v for kv-head i of
     each batch; causal attention (transposed scores, exp on ACT, ones-column
     rowsums, deferred normalization). Batch 1 attention overlaps A2A #2a.
  D. AllToAll #2a/#2b -> core i holds its token chunk of all 32 heads;
     per-token quantization + integer matmul with ternary output weights.
"""
import sys
sys.path.insert(0, "/opt/trn_rl_repo")
import numpy as np
import ml_dtypes
import concourse.bass as bass
import concourse.tile as tile
from concourse import bacc, mybir
from concourse import bass_utils
from concourse.masks import make_identity

f32 = mybir.dt.float32
bf16 = mybir.dt.bfloat16
FT = mybir.ActivationFunctionType
ALU = mybir.AluOpType

B, S, H = 2, 2048, 2048
NH, NKV, HD = 32, 8, 64
G = NH // NKV                    # 4
QKV_O = (NH + 2 * NKV) * HD      # 3072
EPS = 1e-5
THETA = 10000.0
C = 8
SC = S // C                      # 256 positions per core
TOK = B * SC                     # 512 token rows per core
MAGIC = float(1.5 * 2.0 ** 23)   # RNE integer rounding for |v| < 2^22
NT = TOK // 128                  # 4 token tiles
NHT = H // 128                   # 16 h-tiles
NKT = S // 128                   # 16 kj tiles

# a2a1 split into four column-group collectives fired as their data completes:
#   k: per-batch [k 64 | kdup 64] -> [8, 256, 256]
#   v: per-batch [v 64]           -> [8, 256, 128]
#   qA/qB: per-batch head-pair [2 heads = 128] -> [8, 256, 256] each
# q heads are permuted host-side: col hp*1024 + dest*128 + hh*64 + d


def _dap(t_ap, extra, dims):
    return bass.AP(tensor=t_ap.tensor, offset=t_ap.offset + extra, ap=[list(d) for d in dims])


def build_nc():
    nc = bacc.Bacc("TRN2", target_bir_lowering=False, debug=False, num_devices=C)

    x_in = nc.dram_tensor("x", [TOK, H], f32, kind="ExternalInput")
    wn_in = nc.dram_tensor("wn", [1, H], f32, kind="ExternalInput")
    # contiguous stripes: row ((ng*16+j)*128 + h_local), 512 cols each
    wq1t_in = nc.dram_tensor("wq1t", [(QKV_O // 512) * NHT * 128, 512], bf16, kind="ExternalInput")
    wq2t_in = nc.dram_tensor("wq2t", [(H // 512) * NHT * 128, 512], bf16, kind="ExternalInput")
    cos_in = nc.dram_tensor("cosb", [SC, 8 * 32], f32, kind="ExternalInput")
    sin_in = nc.dram_tensor("sinb", [SC, 8 * 32], f32, kind="ExternalInput")
    tri_in = nc.dram_tensor("trimask", [128, 128], bf16, kind="ExternalInput")
    sw1_in = nc.dram_tensor("sw1", [1, 1], f32, kind="ExternalInput")
    sw2_in = nc.dram_tensor("sw2", [1, 1], f32, kind="ExternalInput")
    out_ext = nc.dram_tensor("out", [TOK, H], f32, kind="ExternalOutput")

    X = x_in.ap()
    WQ1 = wq1t_in.ap()
    WQ2 = wq2t_in.ap()
    OUT = out_ext.ap()

    with tile.TileContext(nc) as tc:
        from contextlib import ExitStack
        with ExitStack() as top:
            dram = top.enter_context(tc.tile_pool(name="dram", bufs=1, space="DRAM"))
            const = top.enter_context(tc.tile_pool(name="const", bufs=1))
            smalls = top.enter_context(tc.tile_pool(name="smalls", bufs=1))
            psA = top.enter_context(tc.tile_pool(name="psA", bufs=4, space="PSUM"))
            psS = top.enter_context(tc.tile_pool(name="psS", bufs=2, space="PSUM"))

            # ---------------- DRAM scratch ----------------
            aq_i = [dram.tile([C * SC, 256], bf16, name=f"aq_i{hp}") for hp in range(2)]
            aq_o = [dram.tile([C * SC, 256], bf16, name=f"aq_o{hp}") for hp in range(2)]
            ak_i = dram.tile([C * SC, 256], bf16, name="ak_i")
            ak_o = dram.tile([C * SC, 256], bf16, name="ak_o")
            av_i = dram.tile([C * SC, 128], bf16, name="av_i")
            av_o = dram.tile([C * SC, 128], bf16, name="av_o")
            a2i = [dram.tile([C * SC, G * HD], bf16, name=f"a2i_{b}") for b in range(B)]
            a2o = [dram.tile([C * SC, G * HD], bf16, name=f"a2o_{b}") for b in range(B)]

            # ---------------- constants ----------------
            wnorm_b = const.tile([128, H], f32)
            nc.sync.dma_start(out=wnorm_b[:], in_=_dap(wn_in.ap(), 0, [[0, 128], [1, H]]))
            trim = const.tile([128, 128], bf16)
            nc.sync.dma_start(out=trim[:], in_=tri_in.ap()[:, :])
            sw1b = const.tile([128, 1], f32)
            nc.sync.dma_start(out=sw1b[:], in_=_dap(sw1_in.ap(), 0, [[0, 128], [1, 1]]))
            sw2b = const.tile([128, 1], f32)
            nc.sync.dma_start(out=sw2b[:], in_=_dap(sw2_in.ap(), 0, [[0, 128], [1, 1]]))
            epsb = const.tile([128, 1], f32)
            nc.vector.memset(epsb[:], EPS)
            ident = const.tile([128, 128], bf16)
            make_identity(nc, ident[:])

            d1s = [smalls.tile([128, 1], f32, name=f"d1_{m}") for m in range(NT)]
            d2s = [smalls.tile([128, 1], f32, name=f"d2_{m}") for m in range(NT)]

            xqT_pool = top.enter_context(tc.tile_pool(name="xqT", bufs=NHT))
            pQT = top.enter_context(tc.tile_pool(name="pQT", bufs=4))
            pKT = top.enter_context(tc.tile_pool(name="pKT", bufs=2))

            # ================= Stage A: RMSNorm + quantize =================
            with ExitStack() as sa:
                pA = sa.enter_context(tc.tile_pool(name="pA", bufs=2))
                pXQ = sa.enter_context(tc.tile_pool(name="pXQ", bufs=NT))
                pSc = sa.enter_context(tc.tile_pool(name="pASc", bufs=4))
                xqms = []
                for m in range(NT):
                    xa = pA.tile([128, H], f32, tag="xa")
                    nc.sync.dma_start(out=xa[:], in_=X[m * 128:(m + 1) * 128, :])
                    sq = pA.tile([128, H], f32, tag="sq")
                    ssq = pSc.tile([128, 1], f32, tag="ssq")
                    nc.scalar.activation(out=sq[:], in_=xa[:], func=FT.Square, accum_out=ssq[:])
                    xw = pA.tile([128, H], f32, tag="xw")
                    nc.vector.tensor_tensor(xw[:], xa[:], wnorm_b[:], ALU.mult)
                    std = pSc.tile([128, 1], f32, tag="std")
                    nc.scalar.activation(out=std[:], in_=ssq[:], func=FT.Sqrt,
                                         bias=epsb[:], scale=1.0 / H)
                    rstd = pSc.tile([128, 1], f32, tag="rstd")
                    nc.vector.reciprocal(rstd[:], std[:])
                    mx = pSc.tile([128, 1], f32, tag="mx")
                    nc.vector.tensor_reduce(mx[:], xw[:], mybir.AxisListType.X, ALU.max,
                                            apply_absolute_value=True)
                    mp = pSc.tile([128, 1], f32, tag="mp")
                    nc.vector.tensor_scalar(mp[:], mx[:], rstd[:], 1e-5, ALU.mult, ALU.max)
                    nc.vector.tensor_tensor(d1s[m][:], mp[:], sw1b[:], ALU.mult)
                    rmp = pSc.tile([128, 1], f32, tag="rmp")
                    nc.vector.reciprocal(rmp[:], mp[:])
                    csc = pSc.tile([128, 1], f32, tag="csc")
                    nc.vector.tensor_scalar(csc[:], rmp[:], rstd[:], 127.0, ALU.mult, ALU.mult)
                    t1 = pA.tile([128, H], f32, tag="t1")
                    nc.gpsimd.tensor_scalar(t1[:], xw[:], csc[:], MAGIC, ALU.mult, ALU.add)
                    xqm = pXQ.tile([128, H], bf16, tag="xqm", name=f"xqm_{m}")
                    nc.vector.tensor_scalar(xqm[:], t1[:], MAGIC, None, ALU.subtract)
                    xqms.append(xqm)

                # transposed activations via PE (keeps the DMA queues free)
                xqT = []
                for j in range(NHT):
                    t = xqT_pool.tile([128, TOK], bf16, name=f"xqT_{j}", tag="xqT")
                    xqT.append(t)
                for m in range(NT):
                    for j in range(NHT):
                        tp = psS.tile([128, 128], bf16, tag="st", name=f"tp_{m}_{j}")
                        nc.tensor.transpose(tp[:], xqms[m][:, j * 128:(j + 1) * 128], ident[:])
                        nc.vector.tensor_copy(xqT[j][:, m * 128:(m + 1) * 128], tp[:])

            # ================= Stage B: qkv matmul + RoPE + scatter ========
            with ExitStack() as sb:
                pW = sb.enter_context(tc.tile_pool(name="pW", bufs=12))
                pQC = sb.enter_context(tc.tile_pool(name="pQC", bufs=10))
                pRT = sb.enter_context(tc.tile_pool(name="pRT", bufs=3))
                pSend = sb.enter_context(tc.tile_pool(name="pSend", bufs=NT))
                pCos = sb.enter_context(tc.tile_pool(name="pCos", bufs=1))

                cosr = []
                sinr = []
                for par in range(2):
                    ct = pCos.tile([128, 8 * 32], f32, name=f"cosr_{par}")
                    nc.sync.dma_start(out=ct[:], in_=cos_in.ap()[par * 128:(par + 1) * 128, :])
                    st_ = pCos.tile([128, 8 * 32], f32, name=f"sinr_{par}")
                    nc.sync.dma_start(out=st_[:], in_=sin_in.ap()[par * 128:(par + 1) * 128, :])
                    cosr.append(ct)
                    sinr.append(st_)

                sends = [pSend.tile([128, QKV_O], bf16, name=f"sends_{m}", tag="sends")
                         for m in range(NT)]

                qTs = [[None] * 2 for _ in range(B)]
                KBs = [None] * B
                NQC = QKV_O // 512   # process k, v first, then q (qA = chunks 0,1; qB = 2,3)
                for ng in (4, 5, 0, 1, 2, 3):
                    psq = [psA.tile([128, 512], f32, tag="acc", name=f"qkvp_{ng}_{m}")
                           for m in range(NT)]
                    for j in range(NHT):
                        wt = pW.tile([128, 512], bf16, tag="w1")
                        r0 = (ng * NHT + j) * 128
                        nc.sync.dma_start(out=wt[:], in_=WQ1[r0:r0 + 128, :])
                        for m in range(NT):
                            nc.tensor.matmul(psq[m][:], xqT[j][:, m * 128:(m + 1) * 128], wt[:],
                                             start=(j == 0), stop=(j == NHT - 1))
                    for m in range(NT):
                        par = m % 2
                        if ng < 5:
                            qc_t = pQC.tile([128, 512], f32, tag="qc")
                            nc.scalar.mul(qc_t[:], psq[m][:], d1s[m][:])
                            xv = qc_t[:].rearrange("p (h t d) -> p h t d", t=2, d=32)
                            xr = xv[:, :, 0, :]
                            xi = xv[:, :, 1, :]
                            cv = cosr[par][:].rearrange("p (h d) -> p h d", d=32)
                            sv = sinr[par][:].rearrange("p (h d) -> p h d", d=32)
                            ov = sends[m][:, ng * 512:(ng + 1) * 512].rearrange(
                                "p (h t d) -> p h t d", t=2, d=32)
                            o_r = ov[:, :, 0, :]
                            o_i = ov[:, :, 1, :]
                            ta = pRT.tile([128, 256], f32, tag="ta")
                            tb = pRT.tile([128, 256], f32, tag="tb")
                            tav = ta[:].rearrange("p (h d) -> p h d", d=32)
                            tbv = tb[:].rearrange("p (h d) -> p h d", d=32)
                            tc_ = pRT.tile([128, 256], f32, tag="tc")
                            td = pRT.tile([128, 256], f32, tag="td")
                            tcv = tc_[:].rearrange("p (h d) -> p h d", d=32)
                            tdv = td[:].rearrange("p (h d) -> p h d", d=32)
                            nc.gpsimd.tensor_tensor(tav, xr, cv, ALU.mult)
                            nc.gpsimd.tensor_tensor(tbv, xi, sv, ALU.mult)
                            nc.vector.tensor_tensor(o_r, tav, tbv, ALU.subtract)
                            nc.gpsimd.tensor_tensor(tcv, xr, sv, ALU.mult)
                            nc.gpsimd.tensor_tensor(tdv, xi, cv, ALU.mult)
                            nc.vector.tensor_tensor(o_i, tcv, tdv, ALU.add)
                        else:
                            nc.scalar.mul(sends[m][:, ng * 512:(ng + 1) * 512],
                                          psq[m][:], d1s[m][:])

                    if ng in (1, 3):
                        # a head-pair's q chunks complete: scatter + fire its AllToAll
                        hp = ng // 2
                        for m in range(NT):
                            b = m // 2
                            par = m % 2
                            base = par * 128 * 256 + b * 128
                            nc.gpsimd.dma_start(
                                out=_dap(aq_i[hp][:], base,
                                         [[256, 128], [SC * 256, 8], [1, 128]]),
                                in_=sends[m][:, hp * 1024:(hp + 1) * 1024].rearrange(
                                    "p (j c) -> p j c", j=8))
                        nc.gpsimd.collective_compute(
                            "AllToAll", ALU.bypass, replica_groups=[list(range(C))],
                            ins=[aq_i[hp][:].opt()], outs=[aq_o[hp][:].opt()])
                        for b in range(B):
                            t = pQT.tile([128, S], bf16, name=f"qT_{b}_{hp}", tag="qT")
                            nc.sync.dma_start(
                                out=t[:],
                                in_=aq_o[hp][:, b * 128:(b + 1) * 128],
                                transpose=True)
                            qTs[b][hp] = t
                    elif ng == 4:
                        # k chunk complete: scatter (with duplicate) + k AllToAll
                        for m in range(NT):
                            b = m // 2
                            par = m % 2
                            base = par * 128 * 256 + b * 128
                            for koff in (0, 64):
                                nc.gpsimd.dma_start(
                                    out=_dap(ak_i[:], base + koff,
                                             [[256, 128], [SC * 256, 8], [1, 64]]),
                                    in_=sends[m][:, 2048:2560].rearrange(
                                        "p (j c) -> p j c", j=8))
                        nc.gpsimd.collective_compute(
                            "AllToAll", ALU.bypass, replica_groups=[list(range(C))],
                            ins=[ak_i[:].opt()], outs=[ak_o[:].opt()])
                        for b in range(B):
                            KB = pKT.tile([128, S], bf16, name=f"KB_{b}", tag="kT")
                            nc.sync.dma_start(out=KB[:],
                                              in_=ak_o[:, b * 128:(b + 1) * 128],
                                              transpose=True)
                            KBs[b] = KB
                    elif ng == 5:
                        # v chunk complete: scatter + v AllToAll
                        for m in range(NT):
                            b = m // 2
                            par = m % 2
                            base = par * 128 * 128 + b * 64
                            nc.gpsimd.dma_start(
                                out=_dap(av_i[:], base,
                                         [[128, 128], [SC * 128, 8], [1, 64]]),
                                in_=sends[m][:, 2560:3072].rearrange("p (j c) -> p j c", j=8))
                        nc.gpsimd.collective_compute(
                            "AllToAll", ALU.bypass, replica_groups=[list(range(C))],
                            ins=[av_i[:].opt()], outs=[av_o[:].opt()])

            # ================= Stage C: attention =========================
            with ExitStack() as sc:
                pVA = sc.enter_context(tc.tile_pool(name="pVA", bufs=2 * NKT))
                pEX = sc.enter_context(tc.tile_pool(name="pEX", bufs=NKT))
                pOB = sc.enter_context(tc.tile_pool(name="pOB", bufs=2 * NKT))
                pR = sc.enter_context(tc.tile_pool(name="pR", bufs=8))

                # v loads for both batches
                vas = []
                for b in range(B):
                    vab = []
                    for kt in range(NKT):
                        t = pVA.tile([128, 65], bf16, name=f"va_{b}_{kt}", tag="va")
                        nc.sync.dma_start(
                            out=t[:, 0:64],
                            in_=av_o[kt * 128:(kt + 1) * 128, b * 64:(b + 1) * 64])
                        nc.vector.memset(t[:, 64:65], 1.0)
                        vab.append(t)
                    vas.append(vab)

                obs_all = [[pOB.tile([128, G * HD], bf16, name=f"ob_{b}_{qt}", tag="ob")
                            for qt in range(NKT)] for b in range(B)]
                for hp in range(2):
                    for b in range(B):
                        KB = KBs[b]
                        vab = vas[b]
                        obs = obs_all[b]
                        qTx = qTs[b][hp]
                        for qc in range(4):
                            exs = []
                            for kt in range(4 * qc + 4):
                                dpos = max(0, kt * 128 - qc * 512)
                                st = psS.tile([128, 1024], f32, tag="st",
                                              name=f"st_{b}_{hp}_{qc}_{kt}")
                                nc.tensor.matmul(
                                    st[:, dpos:512],
                                    KB[0:64, kt * 128:(kt + 1) * 128],
                                    qTx[0:64, qc * 512 + dpos:(qc + 1) * 512],
                                    start=True, stop=True)
                                nc.tensor.matmul(
                                    st[:, 512 + dpos:1024],
                                    KB[64:128, kt * 128:(kt + 1) * 128],
                                    qTx[64:128, qc * 512 + dpos:(qc + 1) * 512],
                                    start=True, stop=True, tile_position=(64, 0))
                                ex = pEX.tile([128, 1024], bf16, tag="ex",
                                              name=f"ex_{b}_{hp}_{qc}_{kt}")
                                stv = st[:].rearrange("p (h q) -> p h q", h=2)[:, :, dpos:512]
                                exv = ex[:].rearrange("p (h q) -> p h q", h=2)[:, :, dpos:512]
                                nc.scalar.activation(out=exv, in_=stv, func=FT.Exp, scale=0.125)
                                if kt >= 4 * qc:
                                    for h in range(2):
                                        sl = ex[:, h * 512 + dpos:h * 512 + dpos + 128]
                                        nc.vector.tensor_tensor(sl, sl, trim[:], ALU.mult)
                                exs.append(ex)
                            for h in range(2):
                                for qtl in range(4):
                                    qt = 4 * qc + qtl
                                    op = psA.tile([128, 65], f32, tag="acc",
                                                  name=f"op_{b}_{hp}_{qc}_{h}_{qtl}")
                                    for kt in range(qt + 1):
                                        nc.tensor.matmul(
                                            op[:],
                                            exs[kt][:, h * 512 + qtl * 128:h * 512 + (qtl + 1) * 128],
                                            vab[kt][:],
                                            start=(kt == 0), stop=(kt == qt))
                                    r = pR.tile([128, 1], f32, tag="r")
                                    nc.vector.reciprocal(r[:], op[:, 64:65])
                                    hg = hp * 2 + h
                                    nc.vector.tensor_scalar(
                                        obs[qt][:, hg * 64:(hg + 1) * 64],
                                        op[:, 0:64], r[:], None, ALU.mult)
                        if hp == 1:
                            for qt in range(NKT):
                                j = qt // 2
                                rowbase = j * SC + (qt % 2) * 128
                                nc.gpsimd.dma_start(out=a2i[b][rowbase:rowbase + 128, :],
                                                  in_=obs[qt][:])
                            nc.gpsimd.collective_compute(
                                "AllToAll", ALU.bypass, replica_groups=[list(range(C))],
                                ins=[a2i[b][:].opt()], outs=[a2o[b][:].opt()])

            # ================= Stage D: out projection ====================
            with ExitStack() as sd:
                pD = sd.enter_context(tc.tile_pool(name="pD", bufs=2))
                pDs = sd.enter_context(tc.tile_pool(name="pDs", bufs=4))
                pXT2 = sd.enter_context(tc.tile_pool(name="pXT2", bufs=NHT * 2))
                pW2 = sd.enter_context(tc.tile_pool(name="pW2", bufs=12))
                pO = sd.enter_context(tc.tile_pool(name="pO", bufs=3))

                xq2T = [[None] * 2 for _ in range(NHT)]
                for bh in range(2):
                    for j in range(NHT):
                        xq2T[j][bh] = pXT2.tile([128, 256], bf16,
                                                name=f"xq2T_{j}_{bh}", tag="xq2T")
                for m in range(NT):
                    b = m // 2
                    r0 = (m % 2) * 128
                    x2 = pD.tile([128, H], bf16, tag="x2")
                    nc.sync.dma_start(
                        out=x2[:],
                        in_=_dap(a2o[b][:], r0 * 256,
                                 [[256, 128], [SC * 256, 8], [1, 256]]))
                    mx2 = pDs.tile([128, 1], f32, tag="mx2")
                    nc.vector.tensor_reduce(mx2[:], x2[:], mybir.AxisListType.X, ALU.max,
                                            apply_absolute_value=True)
                    mp2 = pDs.tile([128, 1], f32, tag="mp2")
                    nc.vector.tensor_scalar(mp2[:], mx2[:], 1e-5, None, ALU.max)
                    nc.vector.tensor_tensor(d2s[m][:], mp2[:], sw2b[:], ALU.mult)
                    rm2 = pDs.tile([128, 1], f32, tag="rm2")
                    nc.vector.reciprocal(rm2[:], mp2[:])
                    c2 = pDs.tile([128, 1], f32, tag="c2")
                    nc.vector.tensor_scalar(c2[:], rm2[:], 127.0, None, ALU.mult)
                    t2 = pD.tile([128, H], f32, tag="t2")
                    nc.gpsimd.tensor_scalar(t2[:], x2[:], c2[:], MAGIC, ALU.mult, ALU.add)
                    xq2 = pD.tile([128, H], bf16, tag="xq2")
                    nc.vector.tensor_scalar(xq2[:], t2[:], MAGIC, None, ALU.subtract)
                    for j in range(NHT):
                        tp = psS.tile([128, 128], bf16, tag="st", name=f"tp2_{m}_{j}")
                        nc.tensor.transpose(tp[:], xq2[:, j * 128:(j + 1) * 128], ident[:])
                        nc.vector.tensor_copy(
                            xq2T[j][b][:, (m % 2) * 128:(m % 2 + 1) * 128], tp[:])

                for ng in range(H // 512):
                    ps2 = [psA.tile([128, 512], f32, tag="acc", name=f"ps2_{ng}_{m}")
                           for m in range(NT)]
                    for j in range(NHT):
                        wt = pW2.tile([128, 512], bf16, tag="w2")
                        r0 = (ng * NHT + j) * 128
                        nc.sync.dma_start(out=wt[:], in_=WQ2[r0:r0 + 128, :])
                        for m in range(NT):
                            nc.tensor.matmul(
                                ps2[m][:],
                                xq2T[j][m // 2][:, (m % 2) * 128:(m % 2 + 1) * 128],
                                wt[:], start=(j == 0), stop=(j == NHT - 1))
                    for m in range(NT):
                        ot = pO.tile([128, 512], f32, tag="ot")
                        nc.scalar.mul(ot[:], ps2[m][:], d2s[m][:])
                        nc.sync.dma_start(
                            out=OUT[m * 128:(m + 1) * 128, ng * 512:(ng + 1) * 512], in_=ot[:])

    nc.compile()
    return nc


_NC_CACHE = {}


def _get_nc():
    if "nc" not in _NC_CACHE:
        _NC_CACHE["nc"] = build_nc()
    return _NC_CACHE["nc"]


def _stripe(wt, nchunk):
    """[H, O] -> [(O//512)*16*128, 512] contiguous (ng, j)-stripe layout."""
    Hh, O = wt.shape
    a = wt.reshape(NHT, 128, O // 512, 512)          # [j, h, ng, c]
    a = a.transpose(2, 0, 1, 3)                      # [ng, j, h, c]
    return np.ascontiguousarray(a.reshape(-1, 512))


def kernel(x, w_norm, w_qkv, w_out):
    x = np.asarray(x, dtype=np.float32)
    w_norm = np.asarray(w_norm, dtype=np.float32)
    w_qkv = np.asarray(w_qkv, dtype=np.float32)
    w_out = np.asarray(w_out, dtype=np.float32)

    def tern(w):
        ws = np.float32(1.0) / np.clip(np.mean(np.abs(w)), np.float32(1e-5), None).astype(np.float32)
        wq = np.clip(np.round(w * ws), -1.0, 1.0).astype(np.float32)
        return wq, (np.float32(1.0) / ws).astype(np.float32)

    wq1, s_w1 = tern(w_qkv)
    wq2, s_w2 = tern(w_out)
    # permute q head blocks: new col hp*1024 + dest*128 + (h%2)*64 + d
    hperm = np.empty(NH, np.int64)
    for h in range(NH):
        hperm[(h % 4) // 2 * 16 + (h // 4) * 2 + (h % 2)] = h
    qperm = (hperm[:, None] * HD + np.arange(HD)[None, :]).reshape(-1)
    wq1p = wq1.copy()
    wq1p[:NH * HD] = wq1[qperm]
    wq1t = _stripe(np.ascontiguousarray(wq1p.T), QKV_O // 512).astype(ml_dtypes.bfloat16)
    wq2t = _stripe(np.ascontiguousarray(wq2.T), H // 512).astype(ml_dtypes.bfloat16)

    inv_freq = (1.0 / THETA ** (np.arange(0, HD, 2, dtype=np.float32) / HD)).astype(np.float32)
    t_pos = np.arange(S, dtype=np.float32)
    freqs = t_pos[:, None] * inv_freq[None, :]
    cos_full = np.cos(freqs).astype(np.float32)
    sin_full = np.sin(freqs).astype(np.float32)

    trimask = np.triu(np.ones((128, 128), np.float32)).astype(ml_dtypes.bfloat16)
    sw1 = np.array([[s_w1 / np.float32(127.0)]], dtype=np.float32)
    sw2 = np.array([[s_w2 / np.float32(127.0)]], dtype=np.float32)
    wn2d = w_norm.reshape(1, H)

    in_maps = []
    for i in range(C):
        xc = np.ascontiguousarray(
            np.concatenate([x[0, i * SC:(i + 1) * SC, :], x[1, i * SC:(i + 1) * SC, :]], axis=0))
        in_maps.append({
            "x": xc,
            "wn": wn2d,
            "wq1t": wq1t,
            "wq2t": wq2t,
            "cosb": np.ascontiguousarray(np.tile(cos_full[i * SC:(i + 1) * SC, :], (1, 8))),
            "sinb": np.ascontiguousarray(np.tile(sin_full[i * SC:(i + 1) * SC, :], (1, 8))),
            "trimask": trimask,
            "sw1": sw1,
            "sw2": sw2,
        })

    nc = _get_nc()
    res = bass_utils.run_bass_kernel_spmd(nc, in_maps, core_ids=list(range(C)))

    out = np.empty((B, S, H), dtype=np.float32)
    for i in range(C):
        ci = res.results[i]["out"]
        for b in range(B):
            out[b, i * SC:(i + 1) * SC, :] = ci[b * SC:(b + 1) * SC, :]
    return out


# revision 56
# speedup vs baseline: 1.4229x; 1.0120x over previous
"""Distributed Trainium2 Bass kernel for BitNet-style attention block.

Sharding: sequence-parallel projections + (batch x kv-head) parallel attention,
stitched with per-batch AllToAll collectives (split so comm overlaps compute).

Per core (core i):
  A. RMSNorm + per-token absmax quantization of its 512-token chunk.
  B. qkv projection as exact integer bf16 matmul against host-prequantized
     ternary weights, dequant, RoPE on q/k, scatter into per-batch A2A bufs.
  C. AllToAll #1a/#1b -> core i holds full-sequence q/k/v for kv-head i of
     each batch; causal attention (transposed scores, exp on ACT, ones-column
     rowsums, deferred normalization). Batch 1 attention overlaps A2A #2a.
  D. AllToAll #2a/#2b -> core i holds its token chunk of all 32 heads;
     per-token quantization + integer matmul with ternary output weights.
"""
import sys
sys.path.insert(0, "/opt/trn_rl_repo")
import numpy as np
import ml_dtypes
import concourse.bass as bass
import concourse.tile as tile
from concourse import bacc, mybir
from concourse import bass_utils
from concourse.masks import make_identity

f32 = mybir.dt.float32
bf16 = mybir.dt.bfloat16
FT = mybir.ActivationFunctionType
ALU = mybir.AluOpType

B, S, H = 2, 2048, 2048
NH, NKV, HD = 32, 8, 64
G = NH // NKV                    # 4
QKV_O = (NH + 2 * NKV) * HD      # 3072
EPS = 1e-5
THETA = 10000.0
C = 8
SC = S // C                      # 256 positions per core
TOK = B * SC                     # 512 token rows per core
MAGIC = float(1.5 * 2.0 ** 23)   # RNE integer rounding for |v| < 2^22
NT = TOK // 128                  # 4 token tiles
NHT = H // 128                   # 16 h-tiles
NKT = S // 128                   # 16 kj tiles

# a2a1 split into four column-group collectives fired as their data completes:
#   k: per-batch [k 64 | kdup 64] -> [8, 256, 256]
#   v: per-batch [v 64]           -> [8, 256, 128]
#   qA/qB: per-batch head-pair [2 heads = 128] -> [8, 256, 256] each
# q heads are permuted host-side: col hp*1024 + dest*128 + hh*64 + d


def _dap(t_ap, extra, dims):
    return bass.AP(tensor=t_ap.tensor, offset=t_ap.offset + extra, ap=[list(d) for d in dims])


def build_nc():
    nc = bacc.Bacc("TRN2", target_bir_lowering=False, debug=False, num_devices=C)

    x_in = nc.dram_tensor("x", [TOK, H], f32, kind="ExternalInput")
    wn_in = nc.dram_tensor("wn", [1, H], f32, kind="ExternalInput")
    # contiguous stripes: row ((ng*16+j)*128 + h_local), 512 cols each
    wq1t_in = nc.dram_tensor("wq1t", [(QKV_O // 512) * NHT * 128, 512], bf16, kind="ExternalInput")
    wq2t_in = nc.dram_tensor("wq2t", [(H // 512) * NHT * 128, 512], bf16, kind="ExternalInput")
    cos_in = nc.dram_tensor("cosb", [SC, 8 * 32], f32, kind="ExternalInput")
    sin_in = nc.dram_tensor("sinb", [SC, 8 * 32], f32, kind="ExternalInput")
    tri_in = nc.dram_tensor("trimask", [128, 128], bf16, kind="ExternalInput")
    sw1_in = nc.dram_tensor("sw1", [1, 1], f32, kind="ExternalInput")
    sw2_in = nc.dram_tensor("sw2", [1, 1], f32, kind="ExternalInput")
    out_ext = nc.dram_tensor("out", [TOK, H], f32, kind="ExternalOutput")

    X = x_in.ap()
    WQ1 = wq1t_in.ap()
    WQ2 = wq2t_in.ap()
    OUT = out_ext.ap()

    with tile.TileContext(nc) as tc:
        from contextlib import ExitStack
        with ExitStack() as top:
            dram = top.enter_context(tc.tile_pool(name="dram", bufs=1, space="DRAM"))
            const = top.enter_context(tc.tile_pool(name="const", bufs=1))
            smalls = top.enter_context(tc.tile_pool(name="smalls", bufs=1))
            psA = top.enter_context(tc.tile_pool(name="psA", bufs=4, space="PSUM"))
            psS = top.enter_context(tc.tile_pool(name="psS", bufs=2, space="PSUM"))

            # ---------------- DRAM scratch ----------------
            aq_i = [dram.tile([C * SC, 256], bf16, name=f"aq_i{hp}") for hp in range(2)]
            aq_o = [dram.tile([C * SC, 256], bf16, name=f"aq_o{hp}") for hp in range(2)]
            ak_i = dram.tile([C * SC, 256], bf16, name="ak_i")
            ak_o = dram.tile([C * SC, 256], bf16, name="ak_o")
            av_i = dram.tile([C * SC, 128], bf16, name="av_i")
            av_o = dram.tile([C * SC, 128], bf16, name="av_o")
            a2i = [dram.tile([C * SC, G * HD], bf16, name=f"a2i_{b}") for b in range(B)]
            a2o = [dram.tile([C * SC, G * HD], bf16, name=f"a2o_{b}") for b in range(B)]

            # ---------------- constants ----------------
            wnorm_b = const.tile([128, H], f32)
            nc.sync.dma_start(out=wnorm_b[:], in_=_dap(wn_in.ap(), 0, [[0, 128], [1, H]]))
            trim = const.tile([128, 128], bf16)
            nc.sync.dma_start(out=trim[:], in_=tri_in.ap()[:, :])
            sw1b = const.tile([128, 1], f32)
            nc.sync.dma_start(out=sw1b[:], in_=_dap(sw1_in.ap(), 0, [[0, 128], [1, 1]]))
            sw2b = const.tile([128, 1], f32)
            nc.sync.dma_start(out=sw2b[:], in_=_dap(sw2_in.ap(), 0, [[0, 128], [1, 1]]))
            epsb = const.tile([128, 1], f32)
            nc.vector.memset(epsb[:], EPS)
            ident = const.tile([128, 128], bf16)
            make_identity(nc, ident[:])

            d1s = [smalls.tile([128, 1], f32, name=f"d1_{m}") for m in range(NT)]
            d2s = [smalls.tile([128, 1], f32, name=f"d2_{m}") for m in range(NT)]

            xqT_pool = top.enter_context(tc.tile_pool(name="xqT", bufs=NHT))
            pQT = top.enter_context(tc.tile_pool(name="pQT", bufs=4))
            pKT = top.enter_context(tc.tile_pool(name="pKT", bufs=2))

            # ================= Stage A: RMSNorm + quantize =================
            with ExitStack() as sa:
                pA = sa.enter_context(tc.tile_pool(name="pA", bufs=2))
                pXQ = sa.enter_context(tc.tile_pool(name="pXQ", bufs=NT))
                pSc = sa.enter_context(tc.tile_pool(name="pASc", bufs=4))
                xqms = []
                for m in range(NT):
                    xa = pA.tile([128, H], f32, tag="xa")
                    nc.sync.dma_start(out=xa[:], in_=X[m * 128:(m + 1) * 128, :])
                    sq = pA.tile([128, H], f32, tag="sq")
                    ssq = pSc.tile([128, 1], f32, tag="ssq")
                    nc.scalar.activation(out=sq[:], in_=xa[:], func=FT.Square, accum_out=ssq[:])
                    xw = pA.tile([128, H], f32, tag="xw")
                    nc.vector.tensor_tensor(xw[:], xa[:], wnorm_b[:], ALU.mult)
                    std = pSc.tile([128, 1], f32, tag="std")
                    nc.scalar.activation(out=std[:], in_=ssq[:], func=FT.Sqrt,
                                         bias=epsb[:], scale=1.0 / H)
                    rstd = pSc.tile([128, 1], f32, tag="rstd")
                    nc.vector.reciprocal(rstd[:], std[:])
                    mx = pSc.tile([128, 1], f32, tag="mx")
                    nc.vector.tensor_reduce(mx[:], xw[:], mybir.AxisListType.X, ALU.max,
                                            apply_absolute_value=True)
                    mp = pSc.tile([128, 1], f32, tag="mp")
                    nc.vector.tensor_scalar(mp[:], mx[:], rstd[:], 1e-5, ALU.mult, ALU.max)
                    nc.vector.tensor_tensor(d1s[m][:], mp[:], sw1b[:], ALU.mult)
                    rmp = pSc.tile([128, 1], f32, tag="rmp")
                    nc.vector.reciprocal(rmp[:], mp[:])
                    csc = pSc.tile([128, 1], f32, tag="csc")
                    nc.vector.tensor_scalar(csc[:], rmp[:], rstd[:], 127.0, ALU.mult, ALU.mult)
                    t1 = pA.tile([128, H], f32, tag="t1")
                    nc.gpsimd.tensor_scalar(t1[:], xw[:], csc[:], MAGIC, ALU.mult, ALU.add)
                    xqm = pXQ.tile([128, H], bf16, tag="xqm", name=f"xqm_{m}")
                    nc.vector.tensor_scalar(xqm[:], t1[:], MAGIC, None, ALU.subtract)
                    xqms.append(xqm)

                # transposed activations via PE (keeps the DMA queues free)
                xqT = []
                for j in range(NHT):
                    t = xqT_pool.tile([128, TOK], bf16, name=f"xqT_{j}", tag="xqT")
                    xqT.append(t)
                for m in range(NT):
                    for j in range(NHT):
                        tp = psS.tile([128, 128], bf16, tag="st", name=f"tp_{m}_{j}")
                        nc.tensor.transpose(tp[:], xqms[m][:, j * 128:(j + 1) * 128], ident[:])
                        nc.vector.tensor_copy(xqT[j][:, m * 128:(m + 1) * 128], tp[:])

            # ================= Stage B: qkv matmul + RoPE + scatter ========
            with ExitStack() as sb:
                pW = sb.enter_context(tc.tile_pool(name="pW", bufs=12))
                pQC = sb.enter_context(tc.tile_pool(name="pQC", bufs=10))
                pRT = sb.enter_context(tc.tile_pool(name="pRT", bufs=3))
                pSend = sb.enter_context(tc.tile_pool(name="pSend", bufs=NT))
                pCos = sb.enter_context(tc.tile_pool(name="pCos", bufs=1))

                cosr = []
                sinr = []
                for par in range(2):
                    ct = pCos.tile([128, 8 * 32], f32, name=f"cosr_{par}")
                    nc.sync.dma_start(out=ct[:], in_=cos_in.ap()[par * 128:(par + 1) * 128, :])
                    st_ = pCos.tile([128, 8 * 32], f32, name=f"sinr_{par}")
                    nc.sync.dma_start(out=st_[:], in_=sin_in.ap()[par * 128:(par + 1) * 128, :])
                    cosr.append(ct)
                    sinr.append(st_)

                sends = [pSend.tile([128, QKV_O], bf16, name=f"sends_{m}", tag="sends")
                         for m in range(NT)]

                qTs = [[None] * 2 for _ in range(B)]
                KBs = [None] * B
                NQC = QKV_O // 512   # process k, v first, then q (qA = chunks 0,1; qB = 2,3)
                for ng in (4, 5, 0, 1, 2, 3):
                    psq = [psA.tile([128, 512], f32, tag="acc", name=f"qkvp_{ng}_{m}")
                           for m in range(NT)]
                    for j in range(NHT):
                        wt = pW.tile([128, 512], bf16, tag="w1")
                        r0 = (ng * NHT + j) * 128
                        nc.sync.dma_start(out=wt[:], in_=WQ1[r0:r0 + 128, :])
                        for m in range(NT):
                            nc.tensor.matmul(psq[m][:], xqT[j][:, m * 128:(m + 1) * 128], wt[:],
                                             start=(j == 0), stop=(j == NHT - 1))
                    for m in range(NT):
                        par = m % 2
                        if ng < 5:
                            qc_t = pQC.tile([128, 512], f32, tag="qc")
                            nc.scalar.mul(qc_t[:], psq[m][:], d1s[m][:])
                            xv = qc_t[:].rearrange("p (h t d) -> p h t d", t=2, d=32)
                            xr = xv[:, :, 0, :]
                            xi = xv[:, :, 1, :]
                            cv = cosr[par][:].rearrange("p (h d) -> p h d", d=32)
                            sv = sinr[par][:].rearrange("p (h d) -> p h d", d=32)
                            ov = sends[m][:, ng * 512:(ng + 1) * 512].rearrange(
                                "p (h t d) -> p h t d", t=2, d=32)
                            o_r = ov[:, :, 0, :]
                            o_i = ov[:, :, 1, :]
                            ta = pRT.tile([128, 256], f32, tag="ta")
                            tb = pRT.tile([128, 256], f32, tag="tb")
                            tav = ta[:].rearrange("p (h d) -> p h d", d=32)
                            tbv = tb[:].rearrange("p (h d) -> p h d", d=32)
                            tc_ = pRT.tile([128, 256], f32, tag="tc")
                            td = pRT.tile([128, 256], f32, tag="td")
                            tcv = tc_[:].rearrange("p (h d) -> p h d", d=32)
                            tdv = td[:].rearrange("p (h d) -> p h d", d=32)
                            nc.vector.tensor_tensor(tav, xr, cv, ALU.mult)
                            nc.vector.tensor_tensor(tbv, xi, sv, ALU.mult)
                            nc.vector.tensor_tensor(o_r, tav, tbv, ALU.subtract)
                            nc.vector.tensor_tensor(tcv, xr, sv, ALU.mult)
                            nc.vector.tensor_tensor(tdv, xi, cv, ALU.mult)
                            nc.vector.tensor_tensor(o_i, tcv, tdv, ALU.add)
                        else:
                            nc.scalar.mul(sends[m][:, ng * 512:(ng + 1) * 512],
                                          psq[m][:], d1s[m][:])

                    if ng in (1, 3):
                        # a head-pair's q chunks complete: scatter + fire its AllToAll
                        hp = ng // 2
                        for m in range(NT):
                            b = m // 2
                            par = m % 2
                            base = par * 128 * 256 + b * 128
                            nc.sync.dma_start(
                                out=_dap(aq_i[hp][:], base,
                                         [[256, 128], [SC * 256, 8], [1, 128]]),
                                in_=sends[m][:, hp * 1024:(hp + 1) * 1024].rearrange(
                                    "p (j c) -> p j c", j=8))
                        nc.gpsimd.collective_compute(
                            "AllToAll", ALU.bypass, replica_groups=[list(range(C))],
                            ins=[aq_i[hp][:].opt()], outs=[aq_o[hp][:].opt()])
                        for b in range(B):
                            t = pQT.tile([128, S], bf16, name=f"qT_{b}_{hp}", tag="qT")
                            nc.sync.dma_start(
                                out=t[:],
                                in_=aq_o[hp][:, b * 128:(b + 1) * 128],
                                transpose=True)
                            qTs[b][hp] = t
                    elif ng == 4:
                        # k chunk complete: scatter (with duplicate) + k AllToAll
                        for m in range(NT):
                            b = m // 2
                            par = m % 2
                            base = par * 128 * 256 + b * 128
                            for koff in (0, 64):
                                nc.sync.dma_start(
                                    out=_dap(ak_i[:], base + koff,
                                             [[256, 128], [SC * 256, 8], [1, 64]]),
                                    in_=sends[m][:, 2048:2560].rearrange(
                                        "p (j c) -> p j c", j=8))
                        nc.gpsimd.collective_compute(
                            "AllToAll", ALU.bypass, replica_groups=[list(range(C))],
                            ins=[ak_i[:].opt()], outs=[ak_o[:].opt()])
                        for b in range(B):
                            KB = pKT.tile([128, S], bf16, name=f"KB_{b}", tag="kT")
                            nc.sync.dma_start(out=KB[:],
                                              in_=ak_o[:, b * 128:(b + 1) * 128],
                                              transpose=True)
                            KBs[b] = KB
                    elif ng == 5:
                        # v chunk complete: scatter + v AllToAll
                        for m in range(NT):
                            b = m // 2
                            par = m % 2
                            base = par * 128 * 128 + b * 64
                            nc.sync.dma_start(
                                out=_dap(av_i[:], base,
                                         [[128, 128], [SC * 128, 8], [1, 64]]),
                                in_=sends[m][:, 2560:3072].rearrange("p (j c) -> p j c", j=8))
                        nc.gpsimd.collective_compute(
                            "AllToAll", ALU.bypass, replica_groups=[list(range(C))],
                            ins=[av_i[:].opt()], outs=[av_o[:].opt()])

            # ================= Stage C: attention =========================
            with ExitStack() as sc:
                pVA = sc.enter_context(tc.tile_pool(name="pVA", bufs=2 * NKT))
                pEX = sc.enter_context(tc.tile_pool(name="pEX", bufs=NKT))
                pOB = sc.enter_context(tc.tile_pool(name="pOB", bufs=2 * NKT))
                pR = sc.enter_context(tc.tile_pool(name="pR", bufs=8))

                # v loads for both batches
                vas = []
                for b in range(B):
                    vab = []
                    for kt in range(NKT):
                        t = pVA.tile([128, 65], bf16, name=f"va_{b}_{kt}", tag="va")
                        nc.sync.dma_start(
                            out=t[:, 0:64],
                            in_=av_o[kt * 128:(kt + 1) * 128, b * 64:(b + 1) * 64])
                        nc.vector.memset(t[:, 64:65], 1.0)
                        vab.append(t)
                    vas.append(vab)

                obs_all = [[pOB.tile([128, G * HD], bf16, name=f"ob_{b}_{qt}", tag="ob")
                            for qt in range(NKT)] for b in range(B)]
                for hp in range(2):
                    for b in range(B):
                        KB = KBs[b]
                        vab = vas[b]
                        obs = obs_all[b]
                        qTx = qTs[b][hp]
                        for qc in range(4):
                            exs = []
                            for kt in range(4 * qc + 4):
                                dpos = max(0, kt * 128 - qc * 512)
                                st = psS.tile([128, 1024], f32, tag="st",
                                              name=f"st_{b}_{hp}_{qc}_{kt}")
                                nc.tensor.matmul(
                                    st[:, dpos:512],
                                    KB[0:64, kt * 128:(kt + 1) * 128],
                                    qTx[0:64, qc * 512 + dpos:(qc + 1) * 512],
                                    start=True, stop=True)
                                nc.tensor.matmul(
                                    st[:, 512 + dpos:1024],
                                    KB[64:128, kt * 128:(kt + 1) * 128],
                                    qTx[64:128, qc * 512 + dpos:(qc + 1) * 512],
                                    start=True, stop=True, tile_position=(64, 0))
                                ex = pEX.tile([128, 1024], bf16, tag="ex",
                                              name=f"ex_{b}_{hp}_{qc}_{kt}")
                                stv = st[:].rearrange("p (h q) -> p h q", h=2)[:, :, dpos:512]
                                exv = ex[:].rearrange("p (h q) -> p h q", h=2)[:, :, dpos:512]
                                nc.scalar.activation(out=exv, in_=stv, func=FT.Exp, scale=0.125)
                                if kt >= 4 * qc:
                                    for h in range(2):
                                        sl = ex[:, h * 512 + dpos:h * 512 + dpos + 128]
                                        nc.vector.tensor_tensor(sl, sl, trim[:], ALU.mult)
                                exs.append(ex)
                            for h in range(2):
                                for qtl in range(4):
                                    qt = 4 * qc + qtl
                                    op = psA.tile([128, 65], f32, tag="acc",
                                                  name=f"op_{b}_{hp}_{qc}_{h}_{qtl}")
                                    for kt in range(qt + 1):
                                        nc.tensor.matmul(
                                            op[:],
                                            exs[kt][:, h * 512 + qtl * 128:h * 512 + (qtl + 1) * 128],
                                            vab[kt][:],
                                            start=(kt == 0), stop=(kt == qt))
                                    r = pR.tile([128, 1], f32, tag="r")
                                    nc.vector.reciprocal(r[:], op[:, 64:65])
                                    hg = hp * 2 + h
                                    nc.vector.tensor_scalar(
                                        obs[qt][:, hg * 64:(hg + 1) * 64],
                                        op[:, 0:64], r[:], None, ALU.mult)
                        if hp == 1:
                            for qt in range(NKT):
                                j = qt // 2
                                rowbase = j * SC + (qt % 2) * 128
                                nc.sync.dma_start(out=a2i[b][rowbase:rowbase + 128, :],
                                                  in_=obs[qt][:])
                            nc.gpsimd.collective_compute(
                                "AllToAll", ALU.bypass, replica_groups=[list(range(C))],
                                ins=[a2i[b][:].opt()], outs=[a2o[b][:].opt()])

            # ================= Stage D: out projection ====================
            with ExitStack() as sd:
                pD = sd.enter_context(tc.tile_pool(name="pD", bufs=2))
                pDs = sd.enter_context(tc.tile_pool(name="pDs", bufs=4))
                pXT2 = sd.enter_context(tc.tile_pool(name="pXT2", bufs=NHT * 2))
                pW2 = sd.enter_context(tc.tile_pool(name="pW2", bufs=12))
                pO = sd.enter_context(tc.tile_pool(name="pO", bufs=3))

                xq2T = [[None] * 2 for _ in range(NHT)]
                for bh in range(2):
                    for j in range(NHT):
                        xq2T[j][bh] = pXT2.tile([128, 256], bf16,
                                                name=f"xq2T_{j}_{bh}", tag="xq2T")
                for m in range(NT):
                    b = m // 2
                    r0 = (m % 2) * 128
                    x2 = pD.tile([128, H], bf16, tag="x2")
                    nc.sync.dma_start(
                        out=x2[:],
                        in_=_dap(a2o[b][:], r0 * 256,
                                 [[256, 128], [SC * 256, 8], [1, 256]]))
                    mx2 = pDs.tile([128, 1], f32, tag="mx2")
                    nc.vector.tensor_reduce(mx2[:], x2[:], mybir.AxisListType.X, ALU.max,
                                            apply_absolute_value=True)
                    mp2 = pDs.tile([128, 1], f32, tag="mp2")
                    nc.vector.tensor_scalar(mp2[:], mx2[:], 1e-5, None, ALU.max)
                    nc.vector.tensor_tensor(d2s[m][:], mp2[:], sw2b[:], ALU.mult)
                    rm2 = pDs.tile([128, 1], f32, tag="rm2")
                    nc.vector.reciprocal(rm2[:], mp2[:])
                    c2 = pDs.tile([128, 1], f32, tag="c2")
                    nc.vector.tensor_scalar(c2[:], rm2[:], 127.0, None, ALU.mult)
                    t2 = pD.tile([128, H], f32, tag="t2")
                    nc.gpsimd.tensor_scalar(t2[:], x2[:], c2[:], MAGIC, ALU.mult, ALU.add)
                    xq2 = pD.tile([128, H], bf16, tag="xq2")
                    nc.vector.tensor_scalar(xq2[:], t2[:], MAGIC, None, ALU.subtract)
                    for j in range(NHT):
                        tp = psS.tile([128, 128], bf16, tag="st", name=f"tp2_{m}_{j}")
                        nc.tensor.transpose(tp[:], xq2[:, j * 128:(j + 1) * 128], ident[:])
                        nc.vector.tensor_copy(
                            xq2T[j][b][:, (m % 2) * 128:(m % 2 + 1) * 128], tp[:])

                for ng in range(H // 512):
                    ps2 = [psA.tile([128, 512], f32, tag="acc", name=f"ps2_{ng}_{m}")
                           for m in range(NT)]
                    for j in range(NHT):
                        wt = pW2.tile([128, 512], bf16, tag="w2")
                        r0 = (ng * NHT + j) * 128
                        nc.sync.dma_start(out=wt[:], in_=WQ2[r0:r0 + 128, :])
                        for m in range(NT):
                            nc.tensor.matmul(
                                ps2[m][:],
                                xq2T[j][m // 2][:, (m % 2) * 128:(m % 2 + 1) * 128],
                                wt[:], start=(j == 0), stop=(j == NHT - 1))
                    for m in range(NT):
                        ot = pO.tile([128, 512], f32, tag="ot")
                        nc.scalar.mul(ot[:], ps2[m][:], d2s[m][:])
                        nc.sync.dma_start(
                            out=OUT[m * 128:(m + 1) * 128, ng * 512:(ng + 1) * 512], in_=ot[:])

    nc.compile()
    return nc


_NC_CACHE = {}


def _get_nc():
    if "nc" not in _NC_CACHE:
        _NC_CACHE["nc"] = build_nc()
    return _NC_CACHE["nc"]


def _stripe(wt, nchunk):
    """[H, O] -> [(O//512)*16*128, 512] contiguous (ng, j)-stripe layout."""
    Hh, O = wt.shape
    a = wt.reshape(NHT, 128, O // 512, 512)          # [j, h, ng, c]
    a = a.transpose(2, 0, 1, 3)                      # [ng, j, h, c]
    return np.ascontiguousarray(a.reshape(-1, 512))


def kernel(x, w_norm, w_qkv, w_out):
    x = np.asarray(x, dtype=np.float32)
    w_norm = np.asarray(w_norm, dtype=np.float32)
    w_qkv = np.asarray(w_qkv, dtype=np.float32)
    w_out = np.asarray(w_out, dtype=np.float32)

    def tern(w):
        ws = np.float32(1.0) / np.clip(np.mean(np.abs(w)), np.float32(1e-5), None).astype(np.float32)
        wq = np.clip(np.round(w * ws), -1.0, 1.0).astype(np.float32)
        return wq, (np.float32(1.0) / ws).astype(np.float32)

    wq1, s_w1 = tern(w_qkv)
    wq2, s_w2 = tern(w_out)
    # permute q head blocks: new col hp*1024 + dest*128 + (h%2)*64 + d
    hperm = np.empty(NH, np.int64)
    for h in range(NH):
        hperm[(h % 4) // 2 * 16 + (h // 4) * 2 + (h % 2)] = h
    qperm = (hperm[:, None] * HD + np.arange(HD)[None, :]).reshape(-1)
    wq1p = wq1.copy()
    wq1p[:NH * HD] = wq1[qperm]
    wq1t = _stripe(np.ascontiguousarray(wq1p.T), QKV_O // 512).astype(ml_dtypes.bfloat16)
    wq2t = _stripe(np.ascontiguousarray(wq2.T), H // 512).astype(ml_dtypes.bfloat16)

    inv_freq = (1.0 / THETA ** (np.arange(0, HD, 2, dtype=np.float32) / HD)).astype(np.float32)
    t_pos = np.arange(S, dtype=np.float32)
    freqs = t_pos[:, None] * inv_freq[None, :]
    cos_full = np.cos(freqs).astype(np.float32)
    sin_full = np.sin(freqs).astype(np.float32)

    trimask = np.triu(np.ones((128, 128), np.float32)).astype(ml_dtypes.bfloat16)
    sw1 = np.array([[s_w1 / np.float32(127.0)]], dtype=np.float32)
    sw2 = np.array([[s_w2 / np.float32(127.0)]], dtype=np.float32)
    wn2d = w_norm.reshape(1, H)

    in_maps = []
    for i in range(C):
        xc = np.ascontiguousarray(
            np.concatenate([x[0, i * SC:(i + 1) * SC, :], x[1, i * SC:(i + 1) * SC, :]], axis=0))
        in_maps.append({
            "x": xc,
            "wn": wn2d,
            "wq1t": wq1t,
            "wq2t": wq2t,
            "cosb": np.ascontiguousarray(np.tile(cos_full[i * SC:(i + 1) * SC, :], (1, 8))),
            "sinb": np.ascontiguousarray(np.tile(sin_full[i * SC:(i + 1) * SC, :], (1, 8))),
            "trimask": trimask,
            "sw1": sw1,
            "sw2": sw2,
        })

    nc = _get_nc()
    res = bass_utils.run_bass_kernel_spmd(nc, in_maps, core_ids=list(range(C)))

    out = np.empty((B, S, H), dtype=np.float32)
    for i in range(C):
        ci = res.results[i]["out"]
        for b in range(B):
            out[b, i * SC:(i + 1) * SC, :] = ci[b * SC:(b + 1) * SC, :]
    return out


# revision 59
# speedup vs baseline: 1.4700x; 1.0331x over previous
"""Distributed Trainium2 Bass kernel for BitNet-style attention block.

Sharding: sequence-parallel projections + (batch x kv-head) parallel attention,
stitched with per-batch AllToAll collectives (split so comm overlaps compute).

Per core (core i):
  A. RMSNorm + per-token absmax quantization of its 512-token chunk.
  B. qkv projection as exact integer bf16 matmul against host-prequantized
     ternary weights, dequant, RoPE on q/k, scatter into per-batch A2A bufs.
  C. AllToAll #1a/#1b -> core i holds full-sequence q/k/v for kv-head i of
     each batch; causal attention (transposed scores, exp on ACT, ones-column
     rowsums, deferred normalization). Batch 1 attention overlaps A2A #2a.
  D. AllToAll #2a/#2b -> core i holds its token chunk of all 32 heads;
     per-token quantization + integer matmul with ternary output weights.
"""
import sys
sys.path.insert(0, "/opt/trn_rl_repo")
import numpy as np
import ml_dtypes
import concourse.bass as bass
import concourse.tile as tile
from concourse import bacc, mybir
from concourse import bass_utils
from concourse.masks import make_identity

f32 = mybir.dt.float32
bf16 = mybir.dt.bfloat16
FT = mybir.ActivationFunctionType
ALU = mybir.AluOpType

B, S, H = 2, 2048, 2048
NH, NKV, HD = 32, 8, 64
G = NH // NKV                    # 4
QKV_O = (NH + 2 * NKV) * HD      # 3072
EPS = 1e-5
THETA = 10000.0
C = 8
SC = S // C                      # 256 positions per core
TOK = B * SC                     # 512 token rows per core
MAGIC = float(1.5 * 2.0 ** 23)   # RNE integer rounding for |v| < 2^22
NT = TOK // 128                  # 4 token tiles
NHT = H // 128                   # 16 h-tiles
NKT = S // 128                   # 16 kj tiles

# a2a1 split into four column-group collectives fired as their data completes:
#   k: per-batch [k 64 | kdup 64] -> [8, 256, 256]
#   v: per-batch [v 64]           -> [8, 256, 128]
#   qA/qB: per-batch head-pair [2 heads = 128] -> [8, 256, 256] each
# q heads are permuted host-side: col hp*1024 + dest*128 + hh*64 + d


def _dap(t_ap, extra, dims):
    return bass.AP(tensor=t_ap.tensor, offset=t_ap.offset + extra, ap=[list(d) for d in dims])


def build_nc():
    nc = bacc.Bacc("TRN2", target_bir_lowering=False, debug=False, num_devices=C)

    x_in = nc.dram_tensor("x", [TOK, H], f32, kind="ExternalInput")
    wn_in = nc.dram_tensor("wn", [1, H], f32, kind="ExternalInput")
    # contiguous stripes: row ((ng*16+j)*128 + h_local), 512 cols each
    wq1t_in = nc.dram_tensor("wq1t", [(QKV_O // 512) * NHT * 128, 512], bf16, kind="ExternalInput")
    wq2t_in = nc.dram_tensor("wq2t", [(H // 512) * NHT * 128, 512], bf16, kind="ExternalInput")
    cos_in = nc.dram_tensor("cosb", [SC, 8 * 32], f32, kind="ExternalInput")
    sin_in = nc.dram_tensor("sinb", [SC, 8 * 32], f32, kind="ExternalInput")
    tri_in = nc.dram_tensor("trimask", [128, 128], bf16, kind="ExternalInput")
    sw1_in = nc.dram_tensor("sw1", [1, 1], f32, kind="ExternalInput")
    sw2_in = nc.dram_tensor("sw2", [1, 1], f32, kind="ExternalInput")
    out_ext = nc.dram_tensor("out", [TOK, H], f32, kind="ExternalOutput")

    X = x_in.ap()
    WQ1 = wq1t_in.ap()
    WQ2 = wq2t_in.ap()
    OUT = out_ext.ap()

    with tile.TileContext(nc) as tc:
        from contextlib import ExitStack
        with ExitStack() as top:
            dram = top.enter_context(tc.tile_pool(name="dram", bufs=1, space="DRAM"))
            const = top.enter_context(tc.tile_pool(name="const", bufs=1))
            smalls = top.enter_context(tc.tile_pool(name="smalls", bufs=1))
            psA = top.enter_context(tc.tile_pool(name="psA", bufs=4, space="PSUM"))
            psS = top.enter_context(tc.tile_pool(name="psS", bufs=2, space="PSUM"))

            # ---------------- DRAM scratch ----------------
            aq_i = [dram.tile([C * SC, 256], bf16, name=f"aq_i{hp}") for hp in range(2)]
            aq_o = [dram.tile([C * SC, 256], bf16, name=f"aq_o{hp}") for hp in range(2)]
            ak_i = dram.tile([C * SC, 256], bf16, name="ak_i")
            ak_o = dram.tile([C * SC, 256], bf16, name="ak_o")
            av_i = dram.tile([C * SC, 128], bf16, name="av_i")
            av_o = dram.tile([C * SC, 128], bf16, name="av_o")
            a2i = [dram.tile([C * SC, G * HD], bf16, name=f"a2i_{b}") for b in range(B)]
            a2o = [dram.tile([C * SC, G * HD], bf16, name=f"a2o_{b}") for b in range(B)]

            # ---------------- constants ----------------
            wnorm_b = const.tile([128, H], f32)
            nc.sync.dma_start(out=wnorm_b[:], in_=_dap(wn_in.ap(), 0, [[0, 128], [1, H]]))
            trim = const.tile([128, 128], bf16)
            nc.sync.dma_start(out=trim[:], in_=tri_in.ap()[:, :])
            sw1b = const.tile([128, 1], f32)
            nc.sync.dma_start(out=sw1b[:], in_=_dap(sw1_in.ap(), 0, [[0, 128], [1, 1]]))
            sw2b = const.tile([128, 1], f32)
            nc.sync.dma_start(out=sw2b[:], in_=_dap(sw2_in.ap(), 0, [[0, 128], [1, 1]]))
            epsb = const.tile([128, 1], f32)
            nc.vector.memset(epsb[:], EPS)
            ident = const.tile([128, 128], bf16)
            make_identity(nc, ident[:])

            d1s = [smalls.tile([128, 1], f32, name=f"d1_{m}") for m in range(NT)]
            d2s = [smalls.tile([128, 1], f32, name=f"d2_{m}") for m in range(NT)]

            xqT_pool = top.enter_context(tc.tile_pool(name="xqT", bufs=NHT))
            pQT = top.enter_context(tc.tile_pool(name="pQT", bufs=4))
            pKT = top.enter_context(tc.tile_pool(name="pKT", bufs=2))

            # ================= Stage A: RMSNorm + quantize =================
            with ExitStack() as sa:
                pA = sa.enter_context(tc.tile_pool(name="pA", bufs=2))
                pXQ = sa.enter_context(tc.tile_pool(name="pXQ", bufs=NT))
                pSc = sa.enter_context(tc.tile_pool(name="pASc", bufs=4))
                xqms = []
                for m in range(NT):
                    xa = pA.tile([128, H], f32, tag="xa")
                    nc.sync.dma_start(out=xa[:], in_=X[m * 128:(m + 1) * 128, :])
                    sq = pA.tile([128, H], f32, tag="sq")
                    ssq = pSc.tile([128, 1], f32, tag="ssq")
                    nc.scalar.activation(out=sq[:], in_=xa[:], func=FT.Square, accum_out=ssq[:])
                    xw = pA.tile([128, H], f32, tag="xw")
                    nc.vector.tensor_tensor(xw[:], xa[:], wnorm_b[:], ALU.mult)
                    std = pSc.tile([128, 1], f32, tag="std")
                    nc.scalar.activation(out=std[:], in_=ssq[:], func=FT.Sqrt,
                                         bias=epsb[:], scale=1.0 / H)
                    rstd = pSc.tile([128, 1], f32, tag="rstd")
                    nc.vector.reciprocal(rstd[:], std[:])
                    mx = pSc.tile([128, 1], f32, tag="mx")
                    nc.vector.tensor_reduce(mx[:], xw[:], mybir.AxisListType.X, ALU.max,
                                            apply_absolute_value=True)
                    mp = pSc.tile([128, 1], f32, tag="mp")
                    nc.vector.tensor_scalar(mp[:], mx[:], rstd[:], 1e-5, ALU.mult, ALU.max)
                    nc.vector.tensor_tensor(d1s[m][:], mp[:], sw1b[:], ALU.mult)
                    rmp = pSc.tile([128, 1], f32, tag="rmp")
                    nc.vector.reciprocal(rmp[:], mp[:])
                    csc = pSc.tile([128, 1], f32, tag="csc")
                    nc.vector.tensor_scalar(csc[:], rmp[:], rstd[:], 127.0, ALU.mult, ALU.mult)
                    t1 = pA.tile([128, H], f32, tag="t1")
                    nc.gpsimd.tensor_scalar(t1[:], xw[:], csc[:], MAGIC, ALU.mult, ALU.add)
                    xqm = pXQ.tile([128, H], bf16, tag="xqm", name=f"xqm_{m}")
                    nc.vector.tensor_scalar(xqm[:], t1[:], MAGIC, None, ALU.subtract)
                    xqms.append(xqm)

                # transposed activations via PE (keeps the DMA queues free)
                xqT = []
                for j in range(NHT):
                    t = xqT_pool.tile([128, TOK], bf16, name=f"xqT_{j}", tag="xqT")
                    xqT.append(t)
                for m in range(NT):
                    for j in range(NHT):
                        tp = psS.tile([128, 128], bf16, tag="st", name=f"tp_{m}_{j}")
                        nc.tensor.transpose(tp[:], xqms[m][:, j * 128:(j + 1) * 128], ident[:])
                        nc.vector.tensor_copy(xqT[j][:, m * 128:(m + 1) * 128], tp[:])

            # ================= Stage B: qkv matmul + RoPE + scatter ========
            with ExitStack() as sb:
                pW = sb.enter_context(tc.tile_pool(name="pW", bufs=12))
                pQC = sb.enter_context(tc.tile_pool(name="pQC", bufs=10))
                pRT = sb.enter_context(tc.tile_pool(name="pRT", bufs=3))
                pSend = sb.enter_context(tc.tile_pool(name="pSend", bufs=NT))
                pCos = sb.enter_context(tc.tile_pool(name="pCos", bufs=1))

                cosr = []
                sinr = []
                for par in range(2):
                    ct = pCos.tile([128, 8 * 32], f32, name=f"cosr_{par}")
                    nc.sync.dma_start(out=ct[:], in_=cos_in.ap()[par * 128:(par + 1) * 128, :])
                    st_ = pCos.tile([128, 8 * 32], f32, name=f"sinr_{par}")
                    nc.sync.dma_start(out=st_[:], in_=sin_in.ap()[par * 128:(par + 1) * 128, :])
                    cosr.append(ct)
                    sinr.append(st_)

                sends = [pSend.tile([128, QKV_O], bf16, name=f"sends_{m}", tag="sends")
                         for m in range(NT)]

                qTs = [[None] * 2 for _ in range(B)]
                KBs = [None] * B
                NQC = QKV_O // 512   # process k, v first, then q (qA = chunks 0,1; qB = 2,3)
                for ng in (4, 5, 0, 1, 2, 3):
                    psq = [psA.tile([128, 512], f32, tag="acc", name=f"qkvp_{ng}_{m}")
                           for m in range(NT)]
                    for j in range(NHT):
                        wt = pW.tile([128, 512], bf16, tag="w1")
                        r0 = (ng * NHT + j) * 128
                        nc.sync.dma_start(out=wt[:], in_=WQ1[r0:r0 + 128, :])
                        for m in range(NT):
                            nc.tensor.matmul(psq[m][:], xqT[j][:, m * 128:(m + 1) * 128], wt[:],
                                             start=(j == 0), stop=(j == NHT - 1))
                    for m in range(NT):
                        par = m % 2
                        if ng < 5:
                            qc_t = pQC.tile([128, 512], f32, tag="qc")
                            nc.scalar.mul(qc_t[:], psq[m][:], d1s[m][:])
                            xv = qc_t[:].rearrange("p (h t d) -> p h t d", t=2, d=32)
                            xr = xv[:, :, 0, :]
                            xi = xv[:, :, 1, :]
                            cv = cosr[par][:].rearrange("p (h d) -> p h d", d=32)
                            sv = sinr[par][:].rearrange("p (h d) -> p h d", d=32)
                            ov = sends[m][:, ng * 512:(ng + 1) * 512].rearrange(
                                "p (h t d) -> p h t d", t=2, d=32)
                            o_r = ov[:, :, 0, :]
                            o_i = ov[:, :, 1, :]
                            ta = pRT.tile([128, 256], f32, tag="ta")
                            tb = pRT.tile([128, 256], f32, tag="tb")
                            tav = ta[:].rearrange("p (h d) -> p h d", d=32)
                            tbv = tb[:].rearrange("p (h d) -> p h d", d=32)
                            tc_ = pRT.tile([128, 256], f32, tag="tc")
                            td = pRT.tile([128, 256], f32, tag="td")
                            tcv = tc_[:].rearrange("p (h d) -> p h d", d=32)
                            tdv = td[:].rearrange("p (h d) -> p h d", d=32)
                            nc.vector.tensor_tensor(tav, xr, cv, ALU.mult)
                            nc.vector.tensor_tensor(tbv, xi, sv, ALU.mult)
                            nc.vector.tensor_tensor(o_r, tav, tbv, ALU.subtract)
                            nc.vector.tensor_tensor(tcv, xr, sv, ALU.mult)
                            nc.vector.tensor_tensor(tdv, xi, cv, ALU.mult)
                            nc.vector.tensor_tensor(o_i, tcv, tdv, ALU.add)
                        else:
                            nc.scalar.mul(sends[m][:, ng * 512:(ng + 1) * 512],
                                          psq[m][:], d1s[m][:])

                    if ng in (1, 3):
                        # a head-pair's q chunks complete: scatter + fire its AllToAll
                        hp = ng // 2
                        for m in range(NT):
                            b = m // 2
                            par = m % 2
                            base = par * 128 * 256 + b * 128
                            nc.sync.dma_start(
                                out=_dap(aq_i[hp][:], base,
                                         [[256, 128], [SC * 256, 8], [1, 128]]),
                                in_=sends[m][:, hp * 1024:(hp + 1) * 1024].rearrange(
                                    "p (j c) -> p j c", j=8))
                        nc.gpsimd.collective_compute(
                            "AllToAll", ALU.bypass, replica_groups=[list(range(C))],
                            ins=[aq_i[hp][:].opt()], outs=[aq_o[hp][:].opt()])
                    elif ng == 4:
                        # k chunk complete: scatter (with duplicate) + k AllToAll
                        for m in range(NT):
                            b = m // 2
                            par = m % 2
                            base = par * 128 * 256 + b * 128
                            for koff in (0, 64):
                                nc.sync.dma_start(
                                    out=_dap(ak_i[:], base + koff,
                                             [[256, 128], [SC * 256, 8], [1, 64]]),
                                    in_=sends[m][:, 2048:2560].rearrange(
                                        "p (j c) -> p j c", j=8))
                        nc.gpsimd.collective_compute(
                            "AllToAll", ALU.bypass, replica_groups=[list(range(C))],
                            ins=[ak_i[:].opt()], outs=[ak_o[:].opt()])
                    elif ng == 5:
                        # v chunk complete: scatter + v AllToAll
                        for m in range(NT):
                            b = m // 2
                            par = m % 2
                            base = par * 128 * 128 + b * 64
                            nc.sync.dma_start(
                                out=_dap(av_i[:], base,
                                         [[128, 128], [SC * 128, 8], [1, 64]]),
                                in_=sends[m][:, 2560:3072].rearrange("p (j c) -> p j c", j=8))
                        nc.gpsimd.collective_compute(
                            "AllToAll", ALU.bypass, replica_groups=[list(range(C))],
                            ins=[av_i[:].opt()], outs=[av_o[:].opt()])

                # transposed loads AFTER all weight-stripe DMAs are issued, so the
                # serialized transpose queue never starves the matmul pipeline
                for b in range(B):
                    KB = pKT.tile([128, S], bf16, name=f"KB_{b}", tag="kT")
                    nc.sync.dma_start(out=KB[:], in_=ak_o[:, b * 128:(b + 1) * 128],
                                      transpose=True)
                    KBs[b] = KB
                for hp in range(2):
                    for b in range(B):
                        t = pQT.tile([128, S], bf16, name=f"qT_{b}_{hp}", tag="qT")
                        nc.sync.dma_start(out=t[:], in_=aq_o[hp][:, b * 128:(b + 1) * 128],
                                          transpose=True)
                        qTs[b][hp] = t

            # ================= Stage C: attention =========================
            with ExitStack() as sc:
                pVA = sc.enter_context(tc.tile_pool(name="pVA", bufs=2 * NKT))
                pEX = sc.enter_context(tc.tile_pool(name="pEX", bufs=NKT))
                pOB = sc.enter_context(tc.tile_pool(name="pOB", bufs=2 * NKT))
                pR = sc.enter_context(tc.tile_pool(name="pR", bufs=8))

                # v loads for both batches
                vas = []
                for b in range(B):
                    vab = []
                    for kt in range(NKT):
                        t = pVA.tile([128, 65], bf16, name=f"va_{b}_{kt}", tag="va")
                        nc.sync.dma_start(
                            out=t[:, 0:64],
                            in_=av_o[kt * 128:(kt + 1) * 128, b * 64:(b + 1) * 64])
                        nc.vector.memset(t[:, 64:65], 1.0)
                        vab.append(t)
                    vas.append(vab)

                obs_all = [[pOB.tile([128, G * HD], bf16, name=f"ob_{b}_{qt}", tag="ob")
                            for qt in range(NKT)] for b in range(B)]
                for hp in range(2):
                    for b in range(B):
                        KB = KBs[b]
                        vab = vas[b]
                        obs = obs_all[b]
                        qTx = qTs[b][hp]
                        for qc in range(4):
                            exs = []
                            for kt in range(4 * qc + 4):
                                dpos = max(0, kt * 128 - qc * 512)
                                st = psS.tile([128, 1024], f32, tag="st",
                                              name=f"st_{b}_{hp}_{qc}_{kt}")
                                nc.tensor.matmul(
                                    st[:, dpos:512],
                                    KB[0:64, kt * 128:(kt + 1) * 128],
                                    qTx[0:64, qc * 512 + dpos:(qc + 1) * 512],
                                    start=True, stop=True)
                                nc.tensor.matmul(
                                    st[:, 512 + dpos:1024],
                                    KB[64:128, kt * 128:(kt + 1) * 128],
                                    qTx[64:128, qc * 512 + dpos:(qc + 1) * 512],
                                    start=True, stop=True, tile_position=(64, 0))
                                ex = pEX.tile([128, 1024], bf16, tag="ex",
                                              name=f"ex_{b}_{hp}_{qc}_{kt}")
                                stv = st[:].rearrange("p (h q) -> p h q", h=2)[:, :, dpos:512]
                                exv = ex[:].rearrange("p (h q) -> p h q", h=2)[:, :, dpos:512]
                                nc.scalar.activation(out=exv, in_=stv, func=FT.Exp, scale=0.125)
                                if kt >= 4 * qc:
                                    for h in range(2):
                                        sl = ex[:, h * 512 + dpos:h * 512 + dpos + 128]
                                        nc.vector.tensor_tensor(sl, sl, trim[:], ALU.mult)
                                exs.append(ex)
                            for h in range(2):
                                for qtl in range(4):
                                    qt = 4 * qc + qtl
                                    op = psA.tile([128, 65], f32, tag="acc",
                                                  name=f"op_{b}_{hp}_{qc}_{h}_{qtl}")
                                    for kt in range(qt + 1):
                                        nc.tensor.matmul(
                                            op[:],
                                            exs[kt][:, h * 512 + qtl * 128:h * 512 + (qtl + 1) * 128],
                                            vab[kt][:],
                                            start=(kt == 0), stop=(kt == qt))
                                    r = pR.tile([128, 1], f32, tag="r")
                                    nc.vector.reciprocal(r[:], op[:, 64:65])
                                    hg = hp * 2 + h
                                    nc.vector.tensor_scalar(
                                        obs[qt][:, hg * 64:(hg + 1) * 64],
                                        op[:, 0:64], r[:], None, ALU.mult)
                        if hp == 1:
                            for qt in range(NKT):
                                j = qt // 2
                                rowbase = j * SC + (qt % 2) * 128
                                nc.sync.dma_start(out=a2i[b][rowbase:rowbase + 128, :],
                                                  in_=obs[qt][:])
                            nc.gpsimd.collective_compute(
                                "AllToAll", ALU.bypass, replica_groups=[list(range(C))],
                                ins=[a2i[b][:].opt()], outs=[a2o[b][:].opt()])

            # ================= Stage D: out projection ====================
            with ExitStack() as sd:
                pD = sd.enter_context(tc.tile_pool(name="pD", bufs=2))
                pDs = sd.enter_context(tc.tile_pool(name="pDs", bufs=4))
                pXT2 = sd.enter_context(tc.tile_pool(name="pXT2", bufs=NHT * 2))
                pW2 = sd.enter_context(tc.tile_pool(name="pW2", bufs=12))
                pO = sd.enter_context(tc.tile_pool(name="pO", bufs=3))

                xq2T = [[None] * 2 for _ in range(NHT)]
                for bh in range(2):
                    for j in range(NHT):
                        xq2T[j][bh] = pXT2.tile([128, 256], bf16,
                                                name=f"xq2T_{j}_{bh}", tag="xq2T")
                for m in range(NT):
                    b = m // 2
                    r0 = (m % 2) * 128
                    x2 = pD.tile([128, H], bf16, tag="x2")
                    nc.sync.dma_start(
                        out=x2[:],
                        in_=_dap(a2o[b][:], r0 * 256,
                                 [[256, 128], [SC * 256, 8], [1, 256]]))
                    mx2 = pDs.tile([128, 1], f32, tag="mx2")
                    nc.vector.tensor_reduce(mx2[:], x2[:], mybir.AxisListType.X, ALU.max,
                                            apply_absolute_value=True)
                    mp2 = pDs.tile([128, 1], f32, tag="mp2")
                    nc.vector.tensor_scalar(mp2[:], mx2[:], 1e-5, None, ALU.max)
                    nc.vector.tensor_tensor(d2s[m][:], mp2[:], sw2b[:], ALU.mult)
                    rm2 = pDs.tile([128, 1], f32, tag="rm2")
                    nc.vector.reciprocal(rm2[:], mp2[:])
                    c2 = pDs.tile([128, 1], f32, tag="c2")
                    nc.vector.tensor_scalar(c2[:], rm2[:], 127.0, None, ALU.mult)
                    t2 = pD.tile([128, H], f32, tag="t2")
                    nc.gpsimd.tensor_scalar(t2[:], x2[:], c2[:], MAGIC, ALU.mult, ALU.add)
                    xq2 = pD.tile([128, H], bf16, tag="xq2")
                    nc.vector.tensor_scalar(xq2[:], t2[:], MAGIC, None, ALU.subtract)
                    for j in range(NHT):
                        tp = psS.tile([128, 128], bf16, tag="st", name=f"tp2_{m}_{j}")
                        nc.tensor.transpose(tp[:], xq2[:, j * 128:(j + 1) * 128], ident[:])
                        nc.vector.tensor_copy(
                            xq2T[j][b][:, (m % 2) * 128:(m % 2 + 1) * 128], tp[:])

                for ng in range(H // 512):
                    ps2 = [psA.tile([128, 512], f32, tag="acc", name=f"ps2_{ng}_{m}")
                           for m in range(NT)]
                    for j in range(NHT):
                        wt = pW2.tile([128, 512], bf16, tag="w2")
                        r0 = (ng * NHT + j) * 128
                        nc.sync.dma_start(out=wt[:], in_=WQ2[r0:r0 + 128, :])
                        for m in range(NT):
                            nc.tensor.matmul(
                                ps2[m][:],
                                xq2T[j][m // 2][:, (m % 2) * 128:(m % 2 + 1) * 128],
                                wt[:], start=(j == 0), stop=(j == NHT - 1))
                    for m in range(NT):
                        ot = pO.tile([128, 512], f32, tag="ot")
                        nc.scalar.mul(ot[:], ps2[m][:], d2s[m][:])
                        nc.sync.dma_start(
                            out=OUT[m * 128:(m + 1) * 128, ng * 512:(ng + 1) * 512], in_=ot[:])

    nc.compile()
    return nc


_NC_CACHE = {}


def _get_nc():
    if "nc" not in _NC_CACHE:
        _NC_CACHE["nc"] = build_nc()
    return _NC_CACHE["nc"]


def _stripe(wt, nchunk):
    """[H, O] -> [(O//512)*16*128, 512] contiguous (ng, j)-stripe layout."""
    Hh, O = wt.shape
    a = wt.reshape(NHT, 128, O // 512, 512)          # [j, h, ng, c]
    a = a.transpose(2, 0, 1, 3)                      # [ng, j, h, c]
    return np.ascontiguousarray(a.reshape(-1, 512))


def kernel(x, w_norm, w_qkv, w_out):
    x = np.asarray(x, dtype=np.float32)
    w_norm = np.asarray(w_norm, dtype=np.float32)
    w_qkv = np.asarray(w_qkv, dtype=np.float32)
    w_out = np.asarray(w_out, dtype=np.float32)

    def tern(w):
        ws = np.float32(1.0) / np.clip(np.mean(np.abs(w)), np.float32(1e-5), None).astype(np.float32)
        wq = np.clip(np.round(w * ws), -1.0, 1.0).astype(np.float32)
        return wq, (np.float32(1.0) / ws).astype(np.float32)

    wq1, s_w1 = tern(w_qkv)
    wq2, s_w2 = tern(w_out)
    # permute q head blocks: new col hp*1024 + dest*128 + (h%2)*64 + d
    hperm = np.empty(NH, np.int64)
    for h in range(NH):
        hperm[(h % 4) // 2 * 16 + (h // 4) * 2 + (h % 2)] = h
    qperm = (hperm[:, None] * HD + np.arange(HD)[None, :]).reshape(-1)
    wq1p = wq1.copy()
    wq1p[:NH * HD] = wq1[qperm]
    wq1t = _stripe(np.ascontiguousarray(wq1p.T), QKV_O // 512).astype(ml_dtypes.bfloat16)
    wq2t = _stripe(np.ascontiguousarray(wq2.T), H // 512).astype(ml_dtypes.bfloat16)

    inv_freq = (1.0 / THETA ** (np.arange(0, HD, 2, dtype=np.float32) / HD)).astype(np.float32)
    t_pos = np.arange(S, dtype=np.float32)
    freqs = t_pos[:, None] * inv_freq[None, :]
    cos_full = np.cos(freqs).astype(np.float32)
    sin_full = np.sin(freqs).astype(np.float32)

    trimask = np.triu(np.ones((128, 128), np.float32)).astype(ml_dtypes.bfloat16)
    sw1 = np.array([[s_w1 / np.float32(127.0)]], dtype=np.float32)
    sw2 = np.array([[s_w2 / np.float32(127.0)]], dtype=np.float32)
    wn2d = w_norm.reshape(1, H)

    in_maps = []
    for i in range(C):
        xc = np.ascontiguousarray(
            np.concatenate([x[0, i * SC:(i + 1) * SC, :], x[1, i * SC:(i + 1) * SC, :]], axis=0))
        in_maps.append({
            "x": xc,
            "wn": wn2d,
            "wq1t": wq1t,
            "wq2t": wq2t,
            "cosb": np.ascontiguousarray(np.tile(cos_full[i * SC:(i + 1) * SC, :], (1, 8))),
            "sinb": np.ascontiguousarray(np.tile(sin_full[i * SC:(i + 1) * SC, :], (1, 8))),
            "trimask": trimask,
            "sw1": sw1,
            "sw2": sw2,
        })

    nc = _get_nc()
    res = bass_utils.run_bass_kernel_spmd(nc, in_maps, core_ids=list(range(C)))

    out = np.empty((B, S, H), dtype=np.float32)
    for i in range(C):
        ci = res.results[i]["out"]
        for b in range(B):
            out[b, i * SC:(i + 1) * SC, :] = ci[b * SC:(b + 1) * SC, :]
    return out
